# revision 1
# baseline (speedup 1.0000x reference)
# Trainium2 Bass kernel for nn_NeuralPromptProducerLayer (moe_routing).
# v2: fp8-e4m3 DoubleRow MLP with resident weights, LayerNorms folded into
# matmuls via rank-1 corrections (normalized activations never materialized),
# fused softmax denominator via an augmented-V ones column, batched DMAs.
import sys
sys.path.insert(0, '/opt/trn_rl_repo')

import numpy as np

B, T, D = 8, 704, 1024
RULES, KSLOT, HD = 16, 4, 64
SD, E = 32, 3
DC = D // 128                        # 8 d-chunks
CH = [(0, 128), (128, 256), (256, 384), (384, 512), (512, 640), (640, 704)]
NCH = len(CH)
HALVES = [(0, 352), (352, 704)]
SEGS = [(0, 128), (128, 192), (192, 704)]
SEG_LENS = [128.0, 64.0, 512.0]
FD1 = 512
FD2 = 4096
C2N = FD2 // 128                     # 32
J1N = FD1 // 128                     # 4

# consts-blob column layout (f32, [128 partitions, CBW])
CB_IDENT = 0          # [128, 128] identity
CB_WENT = 128         # [128, 8*32]  W_ent chunks
CB_SEGW = 384         # [128, 18]    seg/len weights per (m, e)
CB_E4 = 402           # [4, 64]
CB_E4T = 466          # [64, 4]
CB_IOTA = 470         # [64, 1]
CB_WQER = 471         # [64, 32]
CB_WKER = 503         # [32, 32]
CB_WQES = 535         # [64, 64]
CB_WKES = 599         # [32, 16]
CB_RET = 615          # [64, 64]
CB_REF = 679          # [64, 64]
CB_BENT = 743         # [32, 1]
CB_CSE = 744          # [1, 32]
CB_CS1 = 776          # [1, 512]
CB_R1 = 1288          # [1, 512]
CB_OC = 1800          # [128, 1] ones column
CB_OR128 = 1801       # [1, 128] ones row
CB_ORT = 1929         # [1, 704] ones row
CB_UR = 2633          # [1, 260] V-aug ones-col selector
CBW = 2896

_RUNNERS = {}


# ---------------------------------------------------------------------------
# TileContext subclass: this walrus build accepts at most ONE sync-wait per
# instruction; split excess waits onto injected NoOps / extra drains.
# ---------------------------------------------------------------------------
def _make_tile_cls():
    from concourse import tile as _tile
    from concourse import mybir as _mybir
    from concourse.vector_clock import ScopedClock

    class TileContextSplit(_tile.TileContext):
        def _lower_ordered_insts(self, ordered):
            for bb_name in list(ordered.keys()):
                insts = ordered[bb_name]
                out = []
                n_new = 0
                for inst in insts:
                    si = getattr(inst, 'sync_info', None)
                    waits = list(si.on_wait) if (si is not None and si.on_wait) else []
                    if len(waits) > 1:
                        for w in waits[:-1]:
                            nop = _mybir.InstNoOp(name=f"{inst.name}-w{n_new}",
                                                  ins=[], outs=[])
                            nop.engine = inst.engine
                            nop.sync_info = _mybir.SyncInfo(on_wait=[w], on_update=[])
                            out.append(nop)
                            n_new += 1
                        si.on_wait = waits[-1:]
                    out.append(inst)
                ordered[bb_name] = out
            return super()._lower_ordered_insts(ordered)

        def _drain_and_barrier(self, tick_clock, wait_clock):
            nc = self.nc
            drain_inst = nc.sync.drain()
            wait_clock.add_sem_waits(
                drain_inst.ins, ScopedClock({None: tick_clock.global_clock}))
            waits = list(drain_inst.ins.sync_info.on_wait or [])
            if len(waits) > 1:
                drain_inst.ins.sync_info.on_wait = waits[:1]
                rest = waits[1:]
                while rest:
                    extra = nc.sync.drain()
                    extra.ins.sync_info = _mybir.SyncInfo(on_wait=rest[:1],
                                                          on_update=[])
                    rest = rest[1:]
            nc.all_engine_barrier()
            assert self.sems is not None
            popped = nc._tile_sem_poison_stack.pop()
            assert popped is self._sem_poison
            nc.clear_and_free_semaphores(list(self.sems.allocated().values()))
            nc.all_engine_barrier()

    return TileContextSplit


# ---------------------------------------------------------------------------
# Program emission
# ---------------------------------------------------------------------------
def _emit(use_mask, ln1_aff, ln2_aff, repeat=1):
    import concourse.bass as bass
    import concourse.mybir as mybir
    from contextlib import ExitStack

    f32 = mybir.dt.float32
    f32r = mybir.dt.float32r
    bf16 = mybir.dt.bfloat16
    fp8 = mybir.dt.float8e4
    i32 = mybir.dt.int32
    AF = mybir.ActivationFunctionType
    AL = mybir.AluOpType
    DR = mybir.MatmulPerfMode.DoubleRow
    X = mybir.AxisListType.X
    TileContextSplit = _make_tile_cls()

    nc = bass.Bass("TRN2", target_bir_lowering=False, num_devices=B)

    xT_d = nc.declare_dram_parameter("xT", [128, DC * T], f32r, isOutput=False)
    xN_d = nc.declare_dram_parameter("xN", [128, NCH * 1024], f32, isOutput=False)
    Wqkv_d = nc.declare_dram_parameter("Wqkv_g", [RULES, 3 * D * HD], bf16,
                                       isOutput=False)
    bcs_d = nc.declare_dram_parameter("bcs_g", [RULES, 576], f32r, isOutput=False)
    mlpw_d = nc.declare_dram_parameter("mlpw", [128, 40960], fp8, isOutput=False)
    cb_d = nc.declare_dram_parameter("cblob", [128, CBW], f32r, isOutput=False)
    wo_d = nc.declare_dram_parameter("Wo_aug", [HD + 1, D], bf16, isOutput=False)
    if use_mask:
        maskT_d = nc.declare_dram_parameter("maskT", [T, T], f32, isOutput=False)
    out_d = nc.declare_dram_parameter("out", [T, D], f32, isOutput=True)

    with ExitStack() as ctx:
        tc = ctx.enter_context(TileContextSplit(nc, pool_alloc_mode="queue"))
        P = ctx.enter_context(tc.tile_pool(name="main", bufs=1))

        def tile(shape, dt, tag):
            return P.tile(shape, dt, tag=tag, name=f"{tag}_u{nc.next_id()}")

        # ================= phase 0: DMAs + consts =================
        xTc = [tile([128, T], f32r, f"XT{c}") for c in range(DC)]
        for c in range(DC):
            nc.sync.dma_start(xTc[c][:], xT_d[:, c * T:(c + 1) * T])
        CB = tile([128, CBW], f32r, "CB")
        nc.sync.dma_start(CB[:], cb_d[:])
        IDF = tile([128, 128], f32, "IDF")
        nc.sync.dma_start(IDF[:], cb_d[:, CB_IDENT:CB_IDENT + 128].bitcast(f32))
        MW = tile([128, 40960], fp8, "MW")
        nc.sync.dma_start(MW[:], mlpw_d[:])
        WO = tile([HD + 1, D], bf16, "WO")
        nc.sync.dma_start(WO[:], wo_d[:])
        XN = tile([128, NCH * 1024], f32, "XN")
        nc.sync.dma_start(XN[:], xN_d[:])
        if use_mask:
            maskT = [tile([128, T], f32, f"MK{m}") for m in range(NCH)]
            for m, (t0, t1) in enumerate(CH):
                nc.sync.dma_start(maskT[m][0:t1 - t0, :], maskT_d[t0:t1, :])


        # const views from the blob (f32r) + small memsets
        OC = CB[0:128, CB_OC:CB_OC + 1]
        OR128 = CB[0:1, CB_OR128:CB_OR128 + 128]
        ORT = CB[0:1, CB_ORT:CB_ORT + T]
        UR = CB[0:1, CB_UR:CB_UR + KSLOT * 65]
        ones4 = CB[0:KSLOT, CB_OC:CB_OC + 1]
        OB = tile([128, 1], bf16, "OB")       # ones column bf16
        nc.gpsimd.memset(OB[:], 1.0)
        O8 = tile([128, 1], fp8, "O8")        # ones column fp8
        nc.gpsimd.memset(O8[:], 1.0)
        eps1 = tile([1, 1], f32, "EPS")
        nc.gpsimd.memset(eps1[:], 1e-5)

        # MLP weight views
        W1v = MW[:, 0:4096].rearrange("p (s i m) -> p s i m", s=4, i=2, m=512)
        W4v = MW[:, 4096:8192].rearrange("p (s i m) -> p s i m", s=2, i=2, m=1024)
        W2v = MW[:, 8192:24576].rearrange("p (c s i m) -> p c s i m",
                                          c=C2N, s=2, i=2, m=128)
        W3v = MW[:, 24576:40960].rearrange("p (j r i m) -> p j r i m",
                                           j=J1N, r=16, i=2, m=128)

        # ================= phase 1: LN1 stats + xe =================
        sqt = [tile([128, T], bf16, f"SQ{c % 3}") for c in range(DC)]
        MUR = tile([1, T], f32, "MUR")
        INVR = tile([1, T], f32r, "INVR")
        SIGR = tile([1, T], f32, "T3")
        VARR = tile([1, T], f32, "T2")
        MUSQ = tile([1, T], f32, "T1")
        MIR = tile([1, T], f32, "T1")
        NMS = tile([1, T], f32r, "NMS")
        XE = tile([SD, T], f32, "XE")
        XET = tile([128, NCH * SD], f32, "XET")
        IVT = tile([128, NCH], f32, "IVT")
        W3C = tile([128, 18], f32, "W3C")
        NEGC = tile([1, E], f32, "NEGC")

        with tc.tile_pool(name="ps_ln1", bufs=1, space="PSUM") as ps_ln1:
            ps_st = ps_ln1
            psx = [ps_ln1.tile([1, h1 - h0], f32, tag=f"px{h}",
                               name=f"px{h}_u{nc.next_id()}")
                   for h, (h0, h1) in enumerate(HALVES)]
            psq = [ps_ln1.tile([1, h1 - h0], f32, tag=f"pq{h}",
                               name=f"pq{h}_u{nc.next_id()}")
                   for h, (h0, h1) in enumerate(HALVES)]
            psxe = [ps_ln1.tile([SD, h1 - h0], f32, tag=f"pe{h}",
                                name=f"pe{h}_u{nc.next_id()}")
                    for h, (h0, h1) in enumerate(HALVES)]
            for c in range(DC):
                went_c = CB[0:128, CB_WENT + SD * c:CB_WENT + SD * (c + 1)]
                for h, (h0, h1) in enumerate(HALVES):
                    nc.tensor.matmul(psx[h][:], OC,
                                     xTc[c][:, h0:h1],
                                     start=(c == 0), stop=(c == DC - 1))
                    nc.tensor.matmul(psxe[h][:], went_c,
                                     xTc[c][:, h0:h1],
                                     start=(c == 0), stop=(c == DC - 1))
                nc.gpsimd.tensor_mul(sqt[c][:], xTc[c][:].bitcast(f32),
                                     xTc[c][:].bitcast(f32))
            for c in range(DC):
                for h, (h0, h1) in enumerate(HALVES):
                    nc.tensor.matmul(psq[h][:], OB[:], sqt[c][:, h0:h1],
                                     start=(c == 0), stop=(c == DC - 1))
            # rows: mu, var, sig, inv, mu*inv
            for h, (h0, h1) in enumerate(HALVES):
                nc.vector.tensor_scalar_mul(MUR[0:1, h0:h1], psx[h][:], 1.0 / D)
                nc.vector.tensor_mul(MUSQ[0:1, h0:h1], MUR[0:1, h0:h1],
                                     MUR[0:1, h0:h1])
                nc.vector.scalar_tensor_tensor(VARR[0:1, h0:h1], psq[h][:],
                                               1.0 / D, MUSQ[0:1, h0:h1],
                                               AL.mult, AL.subtract)
                nc.scalar.activation(SIGR[0:1, h0:h1], VARR[0:1, h0:h1],
                                     AF.Sqrt, bias=eps1[:])
                with nc.allow_low_precision("f32r rounding is fine here"):
                    nc.vector.reciprocal(INVR[0:1, h0:h1], SIGR[0:1, h0:h1])
                nc.vector.tensor_mul(MIR[0:1, h0:h1], MUR[0:1, h0:h1],
                                     INVR[0:1, h0:h1].bitcast(f32))
            # xe -> SBUF, then per-chunk transposes
            for h, (h0, h1) in enumerate(HALVES):
                nc.vector.tensor_copy(XE[:, h0:h1], psxe[h][:])
            # inv transposed to columns per t-chunk + seg weights
            for m, (t0, t1) in enumerate(CH):
                ts_ = t1 - t0
                pst = ps_st.tile([128, SD], f32, tag="pst",
                                 name=f"pst_u{nc.next_id()}")
                nc.tensor.transpose(pst[0:ts_, 0:SD], XE[:, t0:t1],
                                    IDF[0:SD, 0:SD])
                nc.vector.tensor_copy(XET[0:ts_, m * SD:(m + 1) * SD],
                                      pst[0:ts_, 0:SD])
                psi = ps_st.tile([128, 1], f32, tag="psi",
                                 name=f"psi_u{nc.next_id()}")
                nc.tensor.transpose(psi[0:ts_, 0:1],
                                    INVR[0:1, t0:t1].bitcast(f32),
                                    IDF[0:1, 0:1])
                nc.vector.tensor_copy(IVT[0:ts_, m:m + 1], psi[0:ts_, 0:1])
                nc.vector.tensor_scalar(
                    W3C[:, 3 * m:3 * (m + 1)],
                    CB[0:128, CB_SEGW + 3 * m:CB_SEGW + 3 * (m + 1)].bitcast(f32),
                    IVT[:, m:m + 1], None, AL.mult)
            # neg corr row [1, E]
            for e, (s0, s1) in enumerate(SEGS):
                nc.vector.reduce_sum(NEGC[0:1, e:e + 1], MIR[0:1, s0:s1], axis=X)
                nc.vector.tensor_scalar_mul(NEGC[0:1, e:e + 1],
                                            NEGC[0:1, e:e + 1],
                                            -1.0 / SEG_LENS[e])

        # ============ phase 2: entities + routing (fp32) ============
        entT = tile([SD, E], f32, "ENTT")
        GR = tile([1, T], f32r, "GR")
        SR = tile([1, T], f32r, "SR")
        SBC = tile([128, T], f32, "SBC")
        ridx_i = tile([1, KSLOT], i32, "RIDXI")
        with tc.tile_pool(name="ps_rt", bufs=2, space="PSUM") as ps_rt:
            def rpt(shape, tag="r"):
                return ps_rt.tile(shape, f32, tag=tag, name=f"rt_u{nc.next_id()}")

            ps_ent = rpt([SD, E])
            for m, (t0, t1) in enumerate(CH):
                ts_ = t1 - t0
                nc.tensor.matmul(ps_ent[:], XET[0:ts_, m * SD:(m + 1) * SD],
                                 W3C[0:ts_, 3 * m:3 * (m + 1)],
                                 start=(m == 0), stop=False)
            nc.tensor.matmul(ps_ent[:], CB[0:1, CB_CSE:CB_CSE + SD].bitcast(f32),
                             NEGC[0:1, :], start=False, stop=True)
            nc.vector.tensor_scalar(entT[:], ps_ent[:],
                                    CB[0:SD, CB_BENT:CB_BENT + 1].bitcast(f32),
                                    None, AL.add)

            cb_qer = CB[0:HD, CB_WQER:CB_WQER + SD]
            cb_ker = CB[0:SD, CB_WKER:CB_WKER + SD]
            cb_qes = CB[0:HD, CB_WQES:CB_WQES + 64]
            cb_kes = CB[0:SD, CB_WKES:CB_WKES + 16]
            cb_reT = CB[0:HD, CB_RET:CB_RET + 64]
            cb_ref = CB[0:64, CB_REF:CB_REF + HD]
            cb_E4 = CB[0:KSLOT, CB_E4:CB_E4 + 64]
            cb_E4T = CB[0:64, CB_E4T:CB_E4T + KSLOT]
            cb_iota = CB[0:64, CB_IOTA:CB_IOTA + 1]

            ps_qer = rpt([SD, 64])
            nc.tensor.matmul(ps_qer[:], cb_qer, cb_reT, start=True, stop=True)
            q_erT = tile([SD, 64], f32, "q_erT")
            nc.vector.tensor_copy(q_erT[:], ps_qer[:])

            ps_ker = rpt([SD, E])
            nc.tensor.matmul(ps_ker[:], cb_ker.bitcast(f32), entT[:],
                             start=True, stop=True)
            k_erT = tile([SD, E], f32, "k_erT")
            nc.vector.tensor_copy(k_erT[:], ps_ker[:])

            ps_ser = rpt([64, E])
            nc.tensor.matmul(ps_ser[:], q_erT[:], k_erT[:], start=True, stop=True)
            s_er = tile([64, E], f32, "s_er")
            nc.vector.tensor_scalar_mul(s_er[:], ps_ser[:],
                                        float(1.0 / np.sqrt(SD)))

            m1 = tile([64, 1], f32, "m1")
            nc.vector.reduce_max(m1[:], s_er[:], axis=X)
            ps_m1T = rpt([1, 64])
            nc.tensor.transpose(ps_m1T[:], m1[:], IDF[0:64, 0:64])
            m1T = tile([1, 64], f32, "m1T")
            nc.vector.tensor_copy(m1T[:], ps_m1T[:])
            mk = tile([1, KSLOT], f32, "mk")
            for k in range(KSLOT):
                nc.vector.reduce_max(mk[:, k:k + 1], m1T[:, k * 16:(k + 1) * 16],
                                     axis=X)
            ps_mkT = rpt([KSLOT, 1])
            nc.tensor.transpose(ps_mkT[:], mk[:], IDF[0:1, 0:1])
            mkT = tile([KSLOT, 1], f32, "mkT")
            nc.vector.tensor_copy(mkT[:], ps_mkT[:])
            ps_Mcol = rpt([64, 1])
            nc.tensor.matmul(ps_Mcol[:], cb_E4.bitcast(f32), mkT[:],
                             start=True, stop=True)
            Mcol = tile([64, 1], f32, "Mcol")
            nc.vector.tensor_copy(Mcol[:], ps_Mcol[:])

            onehot = tile([64, E], f32, "onehot")
            nc.vector.tensor_scalar(onehot[:], s_er[:], Mcol[:], None, AL.is_equal)
            rm = tile([64, 1], f32, "rm")
            nc.vector.reduce_sum(rm[:], onehot[:], axis=X)
            rm4 = tile([64, KSLOT], f32, "rm4")
            nc.vector.tensor_scalar(rm4[:], cb_E4T.bitcast(f32), rm[:], None, AL.mult)

            ps_sel = rpt([KSLOT, 64])
            nc.tensor.matmul(ps_sel[:], rm4[:], cb_ref.bitcast(f32),
                             start=True, stop=True)
            sel = tile([KSLOT, 64], f32, "sel")
            nc.vector.tensor_copy(sel[:], ps_sel[:])
            ps_selT = rpt([64, KSLOT])
            nc.tensor.transpose(ps_selT[:], sel[:], IDF[0:KSLOT, 0:KSLOT])
            selT = tile([64, KSLOT], f32, "selT")
            nc.vector.tensor_copy(selT[:], ps_selT[:])

            ps_qes = rpt([16, KSLOT])
            for k in range(KSLOT):
                nc.tensor.matmul(ps_qes[:, k:k + 1],
                                 cb_qes[:, k * 16:(k + 1) * 16].bitcast(f32),
                                 selT[:, k:k + 1], start=(k == 0),
                                 stop=(k == KSLOT - 1))
            q_esT = tile([16, KSLOT], f32, "q_esT")
            nc.vector.tensor_copy(q_esT[:], ps_qes[:])
            ps_kes = rpt([16, E])
            nc.tensor.matmul(ps_kes[:], cb_kes.bitcast(f32), entT[:],
                             start=True, stop=True)
            k_esT = tile([16, E], f32, "k_esT")
            nc.vector.tensor_copy(k_esT[:], ps_kes[:])
            ps_sesT = rpt([E, KSLOT])
            nc.tensor.matmul(ps_sesT[:], k_esT[:], q_esT[:], start=True, stop=True)
            s_esT = tile([E, KSLOT], f32, "s_esT")
            nc.vector.tensor_scalar_mul(s_esT[:], ps_sesT[:],
                                        float(1.0 / np.sqrt(SD // 2)))
            ps_ses = rpt([KSLOT, E])
            nc.tensor.transpose(ps_ses[:], s_esT[:], IDF[0:E, 0:E])
            s_es = tile([KSLOT, E], f32, "s_es")
            nc.vector.tensor_copy(s_es[:], ps_ses[:])
            em1 = tile([KSLOT, 1], f32, "em1")
            nc.vector.reduce_max(em1[:], s_es[:], axis=X)
            eoh = tile([KSLOT, E], f32, "eoh")
            nc.vector.tensor_scalar(eoh[:], s_es[:], em1[:], None, AL.is_equal)

            ps_crow = rpt([1, E])
            nc.tensor.matmul(ps_crow[:], ones4.bitcast(f32), eoh[:],
                             start=True, stop=True)
            c_row = tile([1, E], f32, "c_row")
            nc.vector.tensor_copy(c_row[:], ps_crow[:])

            ps_ridx = rpt([KSLOT, 1])
            nc.tensor.matmul(ps_ridx[:], rm4[:], cb_iota.bitcast(f32),
                             start=True, stop=True)
            ridx_col = tile([KSLOT, 1], f32, "ridx_col")
            nc.vector.tensor_copy(ridx_col[:], ps_ridx[:])
            ps_ridxT = rpt([1, KSLOT])
            nc.tensor.transpose(ps_ridxT[:], ridx_col[:], IDF[0:KSLOT, 0:KSLOT])
            ridx_f = tile([1, KSLOT], f32, "ridx_f")
            nc.vector.tensor_copy(ridx_f[:], ps_ridxT[:])
            nc.vector.tensor_copy(ridx_i[:], ridx_f[:])

            # gate-count row, s row = gate * inv, nms row = -mu * s
            for e, (s0, s1) in enumerate(SEGS):
                nc.vector.tensor_scalar(GR[0:1, s0:s1],
                                        ORT[0:1, s0:s1].bitcast(f32),
                                        c_row[0:1, e:e + 1], None, AL.mult)
            nc.vector.tensor_mul(SR[0:1, :], GR[0:1, :].bitcast(f32),
                                 INVR[0:1, :].bitcast(f32))
            nc.vector.scalar_tensor_tensor(NMS[0:1, :], MUR[0:1, :], -1.0,
                                           SR[0:1, :].bitcast(f32),
                                           AL.mult, AL.mult)

        # ============ phase 3: gather + s_bc + xs ============
        wqkv = [tile([128, 1536], bf16, f"G{k}") for k in range(KSLOT)]
        bcs = [tile([1, 576], f32r, f"BCS{k}") for k in range(KSLOT)]
        for k in range(KSLOT):
            reg = nc.gpsimd.alloc_register(f"ridx{k}")
            nc.gpsimd.reg_load(reg, ridx_i[0:1, k:k + 1])
            off = nc.gpsimd.snap(reg, donate=True, min_val=0, max_val=RULES - 1)
            src_w = Wqkv_d[bass.ds(off, 1), :].rearrange(
                "a (p f) -> (a p) f", p=128, f=1536)
            nc.gpsimd.dma_start(wqkv[k][:], src_w)
            nc.gpsimd.dma_start(bcs[k][:], bcs_d[bass.ds(off, 1), :])

        with tc.tile_pool(name="ps_bc", bufs=1, space="PSUM") as ps_bc:
            for h, (h0, h1) in enumerate(HALVES):
                psb = ps_bc.tile([128, h1 - h0], f32, tag=f"sb{h}",
                                 name=f"sb{h}_u{nc.next_id()}")
                nc.tensor.matmul(psb[:], OR128, SR[0:1, h0:h1],
                                 start=True, stop=True)
                nc.vector.tensor_copy(SBC[:, h0:h1], psb[:])

        xs = [tile([128, T], bf16, f"XS{c}") for c in range(DC)]
        for c in range(DC):
            nc.vector.tensor_mul(xs[c][:], xTc[c][:].bitcast(f32), SBC[:])

        # wv_all [128, c=8, k*65+j] from gathered v-parts; csv row; bvsum col
        WVA = tile([128, DC * KSLOT * 65], bf16, "WVA")
        nc.gpsimd.memset(WVA[:], 0.0)
        for k in range(KSLOT):
            nc.vector.tensor_copy(
                WVA.rearrange("p (c k u) -> p c k u", c=DC, k=KSLOT, u=65)
                [:, :, k, 0:64],
                wqkv[k].rearrange("p (c three j) -> p c three j",
                                  c=DC, three=3, j=HD)[:, :, 2, :])
        CSV = tile([1, KSLOT * 65], f32r, "CSV")
        nc.gpsimd.memset(CSV[:].bitcast(f32), 0.0)
        for k in range(KSLOT):
            nc.vector.tensor_copy(CSV[0:1, k * 65:k * 65 + 64],
                                  bcs[k][0:1, 320:384].bitcast(f32))
        if ln1_aff:
            CBV = tile([1, KSLOT * 65], f32r, "CBV")
            nc.gpsimd.memset(CBV[:].bitcast(f32), 0.0)
            for k in range(KSLOT):
                nc.vector.tensor_copy(CBV[0:1, k * 65:k * 65 + 64],
                                      bcs[k][0:1, 512:576].bitcast(f32))
        bv01 = tile([1, HD], f32, "BV01")
        nc.vector.tensor_add(bv01[:], bcs[0][0:1, 128:192].bitcast(f32),
                             bcs[1][0:1, 128:192].bitcast(f32))
        bv23 = tile([1, HD], f32, "BV23")
        nc.vector.tensor_add(bv23[:], bcs[2][0:1, 128:192].bitcast(f32),
                             bcs[3][0:1, 128:192].bitcast(f32))
        bvr = tile([1, HD], f32, "BVR")
        nc.vector.tensor_add(bvr[:], bv01[:], bv23[:])
        BVS = tile([HD, 1], f32, "BVS")
        with tc.tile_pool(name="ps_bv", bufs=1, space="PSUM") as ps_bv:
            psv = ps_bv.tile([HD, 1], f32, tag="bv", name=f"bv_u{nc.next_id()}")
            nc.tensor.transpose(psv[:], bvr[:], IDF[0:1, 0:1])
            nc.vector.tensor_copy(BVS[:], psv[:])

        # ============ phase 4: QKV + V ============
        qT = [tile([HD, T], bf16, f"QT{k}") for k in range(KSLOT)]
        kT = [tile([HD, T], bf16, f"KT{k}") for k in range(KSLOT)]
        V_aug = [tile([128, KSLOT * 65], bf16, f"VA{m}") for m in range(NCH)]
        with tc.tile_pool(name="ps_qkv", bufs=2, space="PSUM") as ps_qkv, \
             tc.tile_pool(name="ps_v", bufs=2, space="PSUM") as ps_v:
            for k in range(KSLOT):
                for h, (h0, h1) in enumerate(HALVES):
                    ps = ps_qkv.tile([128, h1 - h0], f32, tag="qk",
                                     name=f"psqk_u{nc.next_id()}")
                    for c in range(DC):
                        lhs_qk = wqkv[k].rearrange(
                            "p (c three j) -> p c three j",
                            c=DC, three=3, j=HD)[:, c, 0:2, :]
                        nc.tensor.matmul(ps[:], lhs_qk, xs[c][:, h0:h1],
                                         start=(c == 0), stop=False)
                    nc.tensor.matmul(ps[:], bcs[k][0:1, 0:128],
                                     ORT[0:1, h0:h1],
                                     start=False, stop=False)
                    nc.tensor.matmul(ps[:], bcs[k][0:1, 192:320],
                                     NMS[0:1, h0:h1],
                                     start=False, stop=(not ln1_aff))
                    if ln1_aff:
                        nc.tensor.matmul(ps[:], bcs[k][0:1, 384:512],
                                         GR[0:1, h0:h1],
                                         start=False, stop=True)
                    nc.vector.tensor_copy(qT[k][:, h0:h1], ps[0:HD, :])
                    nc.vector.tensor_copy(kT[k][:, h0:h1], ps[HD:128, :])
            for m, (t0, t1) in enumerate(CH):
                ts_ = t1 - t0
                ps = ps_v.tile([128, KSLOT * 65], f32, tag="v",
                               name=f"psv_u{nc.next_id()}")
                for c in range(DC):
                    nc.tensor.matmul(ps[0:ts_, :], xs[c][:, t0:t1],
                                     WVA.rearrange("p (c u) -> p c u",
                                                   c=DC, u=KSLOT * 65)[:, c, :],
                                     start=(c == 0), stop=False)
                nc.tensor.matmul(ps[0:ts_, :], NMS[0:1, t0:t1],
                                 CSV[:], start=False, stop=False)
                nc.tensor.matmul(ps[0:ts_, :], ORT[0:1, t0:t1],
                                 UR, start=False,
                                 stop=(not ln1_aff))
                if ln1_aff:
                    nc.tensor.matmul(ps[0:ts_, :], GR[0:1, t0:t1],
                                     CBV[:], start=False, stop=True)
                nc.scalar.copy(V_aug[m][0:ts_, :], ps[0:ts_, :])

        # ============ phase 5: attention ============
        exp_tags = ["G0", "G1", "G2", "G3", "WVA", "XE"]
        expT = [tile([128, T], bf16, exp_tags[m]) for m in range(NCH)]
        aoT = tile([HD + 1, T], bf16, "AOT")
        nc.gpsimd.memset(aoT[HD:HD + 1, :], 4.0)
        with tc.tile_pool(name="ps_sc", bufs=2, space="PSUM") as ps_sc, \
             tc.tile_pool(name="ps_av", bufs=2, space="PSUM") as ps_av, \
             tc.tile_pool(name="ps_rb", bufs=2, space="PSUM") as ps_rb:
            for h, (h0, h1) in enumerate(HALVES):
                n = h1 - h0
                tmps = []
                for k in range(KSLOT):
                    for m, (t0, t1) in enumerate(CH):
                        ts_ = t1 - t0
                        ps = ps_sc.tile([128, n], f32, tag="sc",
                                        name=f"sc_u{nc.next_id()}")
                        if use_mask:
                            nc.vector.tensor_scalar(ps[0:ts_, :],
                                                    maskT[m][0:ts_, h0:h1],
                                                    8.0, None, AL.mult)
                            first = False
                        else:
                            first = True
                        nc.tensor.matmul(ps[0:ts_, :], kT[k][:, t0:t1],
                                         qT[k][:, h0:h1], start=first, stop=True)
                        nc.scalar.activation(expT[m][0:ts_, h0:h1], ps[0:ts_, :],
                                             AF.Exp, scale=0.125)
                    av = ps_av.tile([HD + 1, n], f32, tag="av",
                                    name=f"av_u{nc.next_id()}")
                    for m, (t0, t1) in enumerate(CH):
                        ts_ = t1 - t0
                        nc.tensor.matmul(av[:], V_aug[m][0:ts_, k * 65:(k + 1) * 65],
                                         expT[m][0:ts_, h0:h1],
                                         start=(m == 0), stop=(m == NCH - 1))
                    rec = tile([1, n], f32r, f"REC{k % 2}")
                    with nc.allow_low_precision("softmax renorm"):
                        nc.vector.reciprocal(rec[:], av[HD:HD + 1, :])
                    rbp = ps_rb.tile([HD, n], f32, tag="rb",
                                     name=f"rb_u{nc.next_id()}")
                    nc.tensor.matmul(rbp[:], OR128[0:1, 0:HD],
                                     rec[:], start=True, stop=True)
                    rb = tile([HD, n], f32, f"RB{k % 2}")
                    nc.scalar.copy(rb[:], rbp[:])
                    tmp = tile([HD, n], f32, f"TMP{k}")
                    nc.vector.tensor_mul(tmp[:], av[0:HD, :], rb[:])
                    tmps.append(tmp)
                nc.gpsimd.tensor_add(tmps[0][:], tmps[0][:], tmps[1][:])
                nc.gpsimd.tensor_add(tmps[2][:], tmps[2][:], tmps[3][:])
                nc.vector.tensor_add(tmps[0][:], tmps[0][:], tmps[2][:])
                nc.vector.tensor_scalar(aoT[0:HD, h0:h1], tmps[0][:], BVS[:],
                                        None, AL.add)

        # ============ phase 6: Wo + residuals (h2q fp8, h2N f32) ============
        h2q = [tile([128, 2, T], fp8, f"KT{p}") for p in range(4)]
        with tc.tile_pool(name="ps_wo", bufs=2, space="PSUM") as ps_wo:
            for c in range(DC):
                for h, (h0, h1) in enumerate(HALVES):
                    ps = ps_wo.tile([128, h1 - h0], f32, tag="woT",
                                    name=f"woT_u{nc.next_id()}")
                    nc.tensor.matmul(ps[:], WO[:, c * 128:(c + 1) * 128],
                                     aoT[:, h0:h1], start=True, stop=True)
                    nc.vector.tensor_add(h2q[c // 2][:, c % 2, h0:h1],
                                         xTc[c][:, h0:h1].bitcast(f32), ps[:])
            for m, (t0, t1) in enumerate(CH):
                ts_ = t1 - t0
                for dh in range(2):
                    d0 = dh * 512
                    ps = ps_wo.tile([128, 512], f32, tag="woN",
                                    name=f"woN_u{nc.next_id()}")
                    nc.tensor.matmul(ps[0:ts_, :], aoT[:, t0:t1],
                                     WO[:, d0:d0 + 512], start=True, stop=True)
                    nc.vector.tensor_add(
                        XN[0:ts_, m * 1024 + d0:m * 1024 + d0 + 512],
                        XN[0:ts_, m * 1024 + d0:m * 1024 + d0 + 512],
                        ps[0:ts_, :])

        # ============ phase 7: LN2 stats on h2q ============
        MU2 = tile([1, T], f32, "MUR")
        NM2 = tile([1, T], f32r, "NMS")
        SG2 = tile([1, T], f32, "T3")
        VA2 = tile([1, T], f32, "T2")
        MQ2 = tile([1, T], f32, "T1")
        IV2 = tile([1, T], f32r, "INVR")
        sq2 = [tile([128, T], bf16, f"SQ{i % 3}") for i in range(DC)]
        with tc.tile_pool(name="ps_ln2", bufs=1, space="PSUM") as ps_ln2:
            px2 = [ps_ln2.tile([1, h1 - h0], f32, tag=f"p2x{h}",
                               name=f"p2x{h}_u{nc.next_id()}")
                   for h, (h0, h1) in enumerate(HALVES)]
            pq2 = [ps_ln2.tile([1, h1 - h0], f32, tag=f"p2q{h}",
                               name=f"p2q{h}_u{nc.next_id()}")
                   for h, (h0, h1) in enumerate(HALVES)]
            for c in range(DC):
                src = h2q[c // 2][:, c % 2, :]
                for h, (h0, h1) in enumerate(HALVES):
                    nc.tensor.matmul(px2[h][:], O8[:], src[:, h0:h1],
                                     start=(c == 0), stop=(c == DC - 1))
                nc.scalar.activation(sq2[c][:], src, AF.Square)
            for c in range(DC):
                for h, (h0, h1) in enumerate(HALVES):
                    nc.tensor.matmul(pq2[h][:], OB[:], sq2[c][:, h0:h1],
                                     start=(c == 0), stop=(c == DC - 1))
            for h, (h0, h1) in enumerate(HALVES):
                nc.vector.tensor_scalar_mul(MU2[0:1, h0:h1], px2[h][:], 1.0 / D)
                nc.vector.tensor_mul(MQ2[0:1, h0:h1], MU2[0:1, h0:h1],
                                     MU2[0:1, h0:h1])
                nc.vector.scalar_tensor_tensor(VA2[0:1, h0:h1], pq2[h][:],
                                               1.0 / D, MQ2[0:1, h0:h1],
                                               AL.mult, AL.subtract)
                nc.scalar.activation(SG2[0:1, h0:h1], VA2[0:1, h0:h1],
                                     AF.Sqrt, bias=eps1[:])
                with nc.allow_low_precision("f32r rounding is fine here"):
                    nc.vector.reciprocal(IV2[0:1, h0:h1], SG2[0:1, h0:h1])
                nc.vector.tensor_scalar_mul(NM2[0:1, h0:h1], MU2[0:1, h0:h1],
                                            -1.0)
        # ============ phase 8: MLP (fp8 DoubleRow) ============
        y1q = [tile([128, 2, T], fp8, f"SQ{p}") for p in range(2)]
        y2_tags = ([f"XS{c}" for c in range(DC)] + [f"G{k}" for k in range(KSLOT)]
                   + ["WVA", "XE", "SBC", "Y2F"])
        if use_mask:
            y2_tags = ([f"MK{m}" for m in range(NCH)]
                       + [f"XS{c}" for c in range(DC)] + ["G0", "G1"])
        y2q = [tile([128, 2, T], fp8, y2_tags[p]) for p in range(16)]
        y3q = [tile([128, 2, T], fp8, f"QT{p}") for p in range(2)]

        with tc.tile_pool(name="ps_i2", bufs=1, space="PSUM") as ps_i2, \
             tc.tile_pool(name="ps_y1", bufs=2, space="PSUM") as ps_y1:
            i2bc = []
            for h, (h0, h1) in enumerate(HALVES):
                pi = ps_i2.tile([128, h1 - h0], f32, tag=f"i2{h}",
                                name=f"i2{h}_u{nc.next_id()}")
                nc.tensor.matmul(pi[:], OR128, IV2[0:1, h0:h1],
                                 start=True, stop=True)
                i2bc.append(pi)
            for j in range(J1N):
                for h, (h0, h1) in enumerate(HALVES):
                    ps = ps_y1.tile([128, h1 - h0], f32, tag="y1",
                                    name=f"y1_u{nc.next_id()}")
                    for s in range(4):
                        nc.tensor.matmul(ps[:], W1v[:, s, :, j * 128:(j + 1) * 128],
                                         h2q[s][:, :, h0:h1], perf_mode=DR,
                                         start=(s == 0), stop=False)
                    nc.tensor.matmul(
                        ps[:], CB[0:1, CB_CS1 + j * 128:CB_CS1 + (j + 1) * 128],
                        NM2[0:1, h0:h1], start=False, stop=(not ln2_aff))
                    if ln2_aff:
                        nc.tensor.matmul(
                            ps[:], CB[0:1, CB_R1 + j * 128:CB_R1 + (j + 1) * 128],
                            SG2[0:1, h0:h1].bitcast(f32r),
                            start=False, stop=True)
                    nc.scalar.activation(y1q[j // 2][:, j % 2, h0:h1], ps[:],
                                         AF.Relu)
            for p in range(2):
                for i in range(2):
                    for h, (h0, h1) in enumerate(HALVES):
                        nc.vector.tensor_mul(y1q[p][:, i, h0:h1],
                                             y1q[p][:, i, h0:h1], i2bc[h][:])

        with tc.tile_pool(name="ps_y2", bufs=3, space="PSUM") as ps_y2, \
             tc.tile_pool(name="ps_y3", bufs=2, space="PSUM") as ps_y3, \
             tc.tile_pool(name="ps_y4", bufs=2, space="PSUM") as ps_y4:
            for c2 in range(C2N):
                for h, (h0, h1) in enumerate(HALVES):
                    ps = ps_y2.tile([128, h1 - h0], f32, tag="y2",
                                    name=f"y2_u{nc.next_id()}")
                    for s in range(2):
                        nc.tensor.matmul(ps[:], W2v[:, c2, s],
                                         y1q[s][:, :, h0:h1], perf_mode=DR,
                                         start=(s == 0), stop=(s == 1))
                    nc.scalar.activation(y2q[c2 // 2][:, c2 % 2, h0:h1], ps[:],
                                         AF.Gelu)
            for j in range(J1N):
                for h, (h0, h1) in enumerate(HALVES):
                    ps = ps_y3.tile([128, h1 - h0], f32, tag="y3",
                                    name=f"y3_u{nc.next_id()}")
                    for r in range(16):
                        nc.tensor.matmul(ps[:], W3v[:, j, r],
                                         y2q[r][:, :, h0:h1], perf_mode=DR,
                                         start=(r == 0), stop=(r == 15))
                    nc.scalar.activation(y3q[j // 2][:, j % 2, h0:h1], ps[:],
                                         AF.Relu)
            for m, (t0, t1) in enumerate(CH):
                ts_ = t1 - t0
                for dh in range(2):
                    d0 = dh * 512
                    ps = ps_y4.tile([128, 512], f32, tag="y4",
                                    name=f"y4_u{nc.next_id()}")
                    for p in range(2):
                        nc.tensor.matmul(ps[0:ts_, :], y3q[p][:, :, t0:t1],
                                         W4v[:, p, :, d0:d0 + 512], perf_mode=DR,
                                         start=(p == 0), stop=(p == 1))
                    nc.vector.tensor_add(
                        XN[0:ts_, m * 1024 + d0:m * 1024 + d0 + 512],
                        XN[0:ts_, m * 1024 + d0:m * 1024 + d0 + 512],
                        ps[0:ts_, :])
                nc.sync.dma_start(out_d[t0:t1, :],
                                  XN[0:ts_, m * 1024:(m + 1) * 1024])

    return nc


# ---------------------------------------------------------------------------
# Host-side input prep
# ---------------------------------------------------------------------------
def _host_consts(inputs, ln1_aff, ln2_aff):
    """Batch-independent tensors (weights), computed once."""
    import ml_dtypes
    f = np.float32
    e4 = ml_dtypes.float8_e4m3
    d = {}

    g1 = np.asarray(inputs["ln1_g"], f) if ln1_aff else None
    g2 = np.asarray(inputs["ln2_g"], f) if ln2_aff else None
    b1 = np.asarray(inputs["ln1_b"], f) if ln1_aff else None
    b2 = np.asarray(inputs["ln2_b"], f) if ln2_aff else None

    # gathered per-rule QKV weights (g1 folded in if affine)
    blks = []
    for Wn in ("Wq", "Wk", "Wv"):
        W = np.asarray(inputs[Wn], f)
        if ln1_aff:
            W = W * g1[:, None]
        blks.append(W.reshape(DC, 128, RULES, HD).transpose(2, 0, 1, 3))
    d["Wqkv_g"] = np.ascontiguousarray(
        np.stack(blks, axis=2).transpose(0, 3, 1, 2, 4)
        .reshape(RULES, 3 * D * HD)).astype(ml_dtypes.bfloat16)

    # bcs row per rule: [bq bk bv csq csk csv cbq cbk cbv] (9*64 = 576)
    bias = np.concatenate(
        [np.asarray(inputs[bn], f).reshape(RULES, HD) for bn in ("bq", "bk", "bv")],
        axis=1)                                             # [R, 192]
    # blks[i] is [R, DC, 128, HD]; column sums over d per rule head
    csums = np.concatenate(
        [blk.reshape(RULES, D, HD).sum(1) for blk in blks], axis=1)  # [R, 192]
    if ln1_aff:
        cb = np.concatenate(
            [np.einsum('d,drh->rh', b1,
                       (np.asarray(inputs[Wn], f) * g1[:, None])
                       .reshape(D, RULES, HD))
             for Wn in ("Wq", "Wk", "Wv")], axis=1)
    else:
        cb = np.zeros((RULES, 192), f)
    d["bcs_g"] = np.ascontiguousarray(np.concatenate([bias, csums, cb], axis=1))

    # Wo_aug bf16
    d["Wo_aug"] = np.ascontiguousarray(np.concatenate(
        [np.asarray(inputs["Wo"], f), np.asarray(inputs["bo"], f)[None, :]],
        0)).astype(ml_dtypes.bfloat16)

    # MLP weights fp8, DoubleRow layouts
    W1 = np.asarray(inputs["fc1_w1"], f)
    if ln2_aff:
        W1 = W1 * g2[:, None]
    W1q = W1.astype(e4)
    W2q = np.asarray(inputs["fc1_w2"], f).astype(e4)
    W3q = np.asarray(inputs["fc2_w1"], f).astype(e4)
    W4q = np.asarray(inputs["fc2_w2"], f).astype(e4)
    # W1 [1024, 512] -> [p, s, i, m]
    w1 = W1q.reshape(4, 2, 128, FD1).transpose(2, 0, 1, 3).reshape(128, 4096)
    # W4 [512, 1024] -> [p, s, i, d]
    w4 = W4q.reshape(2, 2, 128, D).transpose(2, 0, 1, 3).reshape(128, 4096)
    # W2 [512, 4096] -> [p, c2, s, i, m]
    w2 = (W2q.reshape(2, 2, 128, C2N, 128).transpose(2, 3, 0, 1, 4)
          .reshape(128, 16384))
    # W3 [4096, 512] -> [p, j, r, i, m]
    w3 = (W3q.reshape(16, 2, 128, J1N, 128).transpose(2, 3, 0, 1, 4)
          .reshape(128, 16384))
    d["mlpw"] = np.ascontiguousarray(np.concatenate([w1, w4, w2, w3], axis=1))

    # consts blob
    cb_arr = np.zeros((128, CBW), f)
    cb_arr[:, CB_IDENT:CB_IDENT + 128] = np.eye(128, dtype=f)
    went = np.asarray(inputs["W_ent"], f)
    if ln1_aff:
        went = went * g1[:, None]
    cb_arr[:, CB_WENT:CB_WENT + DC * SD] = (
        went.reshape(DC, 128, SD).transpose(1, 0, 2).reshape(128, DC * SD))
    segw = np.zeros((128, NCH * E), f)
    for m, (t0, t1) in enumerate(CH):
        for p in range(t1 - t0):
            t = t0 + p
            for e, (s0, s1) in enumerate(SEGS):
                if s0 <= t < s1:
                    segw[p, m * E + e] = 1.0 / SEG_LENS[e]
    cb_arr[:, CB_SEGW:CB_SEGW + NCH * E] = segw
    E4 = np.kron(np.eye(KSLOT, dtype=f), np.ones((1, RULES), f))
    cb_arr[0:KSLOT, CB_E4:CB_E4 + 64] = E4
    cb_arr[0:64, CB_E4T:CB_E4T + KSLOT] = E4.T
    cb_arr[0:64, CB_IOTA] = (np.arange(64) % RULES).astype(f)
    cb_arr[0:HD, CB_WQER:CB_WQER + SD] = np.asarray(inputs["Wq_er"], f)
    cb_arr[0:SD, CB_WKER:CB_WKER + SD] = np.asarray(inputs["Wk_er"], f)
    Wqes = np.asarray(inputs["Wq_es"], f)          # [K, HD, SD//2]
    cb_arr[0:HD, CB_WQES:CB_WQES + 64] = (
        Wqes.transpose(1, 0, 2).reshape(HD, KSLOT * (SD // 2)))
    cb_arr[0:SD, CB_WKES:CB_WKES + 16] = np.asarray(inputs["Wk_es"], f)
    re = np.asarray(inputs["rules_embed"], f)      # [K, R, HD]
    cb_arr[0:HD, CB_RET:CB_RET + 64] = re.transpose(2, 0, 1).reshape(HD, 64)
    cb_arr[0:64, CB_REF:CB_REF + HD] = re.reshape(64, HD)
    bent = np.asarray(inputs["b_ent"], f)
    if ln1_aff:
        bent = bent + b1 @ went
    cb_arr[0:SD, CB_BENT] = bent
    cb_arr[0, CB_CSE:CB_CSE + SD] = went.sum(0)
    cb_arr[0, CB_CS1:CB_CS1 + FD1] = W1q.astype(f).sum(0)
    if ln2_aff:
        cb_arr[0, CB_R1:CB_R1 + FD1] = b2 @ W1
    cb_arr[:, CB_OC] = 1.0
    cb_arr[0, CB_OR128:CB_OR128 + 128] = 1.0
    cb_arr[0, CB_ORT:CB_ORT + T] = 1.0
    ur = np.zeros(KSLOT * 65, f)
    ur[64::65] = 1.0
    cb_arr[0, CB_UR:CB_UR + KSLOT * 65] = ur
    d["cblob"] = np.ascontiguousarray(cb_arr)
    return d


def _prep_core_inputs(inputs, b, use_mask, ln1_aff, ln2_aff, consts=None):
    f = np.float32
    if consts is None:
        consts = _host_consts(inputs, ln1_aff, ln2_aff)
    d = dict(consts)
    hs = np.asarray(inputs["hidden_states"], f)
    x = hs[b]                                      # [T, D]
    xT = np.ascontiguousarray(x.T)                 # [D, T]
    d["xT"] = np.ascontiguousarray(
        xT.reshape(DC, 128, T).transpose(1, 0, 2).reshape(128, DC * T))
    xn = np.zeros((128, NCH * 1024), f)
    for m, (t0, t1) in enumerate(CH):
        xn[0:t1 - t0, m * 1024:(m + 1) * 1024] = x[t0:t1]
    d["xN"] = xn
    if use_mask:
        d["maskT"] = np.ascontiguousarray(
            np.asarray(inputs["attention_mask"], f)[b].T)
    return d


# ---------------------------------------------------------------------------
# Runner (jax/axon shard_map over 8 cores)
# ---------------------------------------------------------------------------
def _build_runner(use_mask, ln1_aff, ln2_aff, repeat=1):
    key = (use_mask, ln1_aff, ln2_aff, repeat)
    if key in _RUNNERS:
        return _RUNNERS[key]
    import jax
    from jax.sharding import Mesh, PartitionSpec
    from jax.experimental.shard_map import shard_map
    from concourse import mybir
    from concourse.bass2jax import (_bass_exec_p, install_neuronx_cc_hook,
                                    partition_id_tensor)

    nc = _emit(use_mask, ln1_aff, ln2_aff, repeat)
    install_neuronx_cc_hook()
    partition_name = nc.partition_id_tensor.name if nc.partition_id_tensor else None
    in_names, out_names, out_avals, zero_shapes = [], [], [], []
    for alloc in nc.m.functions[0].allocations:
        if not isinstance(alloc, mybir.MemoryLocationSet):
            continue
        name = alloc.memorylocations[0].name
        if alloc.kind == "ExternalInput":
            if name != partition_name:
                in_names.append(name)
        elif alloc.kind == "ExternalOutput":
            out_names.append(name)
            shape = tuple(alloc.tensor_shape)
            dtype = mybir.dt.np(alloc.dtype)
            out_avals.append(jax.core.ShapedArray(shape, dtype))
            zero_shapes.append((shape, dtype))
    n_params = len(in_names)
    n_outs = len(out_avals)
    all_in_names = list(in_names) + list(out_names)
    if partition_name is not None:
        all_in_names.append(partition_name)

    def _body(*args):
        operands = list(args)
        if partition_name is not None:
            operands.append(partition_id_tensor())
        outs = _bass_exec_p.bind(
            *operands, out_avals=tuple(out_avals), in_names=tuple(all_in_names),
            out_names=tuple(out_names), lowering_input_output_aliases=(),
            sim_require_finite=False, sim_require_nnan=False, nc=nc)
        return tuple(outs)

    devices = jax.devices()[:B]
    mesh = Mesh(np.asarray(devices), ("core",))
    in_specs = (PartitionSpec("core"),) * (n_params + n_outs)
    out_specs = (PartitionSpec("core"),) * n_outs
    sharded = jax.jit(
        shard_map(_body, mesh=mesh, in_specs=in_specs, out_specs=out_specs,
                  check_rep=False),
        keep_unused=True)

    def run(per_core_maps):
        concat_in = [
            np.concatenate([np.asarray(per_core_maps[c][nm]) for c in range(B)], 0)
            for nm in in_names]
        concat_zeros = [np.zeros((B * s[0], *s[1:]), dt) for s, dt in zero_shapes]
        out_arrs = jax.block_until_ready(sharded(*concat_in, *concat_zeros))
        return [
            {nm: np.asarray(out_arrs[i]).reshape(B, *out_avals[i].shape)[c]
             for i, nm in enumerate(out_names)}
            for c in range(B)]

    _RUNNERS[key] = (run, sharded, in_names, zero_shapes, out_names, out_avals)
    return _RUNNERS[key]


def kernel(**inputs):
    use_mask = bool(np.any(np.asarray(inputs["attention_mask"])))
    ln1_aff = not (np.all(np.asarray(inputs["ln1_g"]) == 1.0)
                   and np.all(np.asarray(inputs["ln1_b"]) == 0.0))
    ln2_aff = not (np.all(np.asarray(inputs["ln2_g"]) == 1.0)
                   and np.all(np.asarray(inputs["ln2_b"]) == 0.0))
    run = _build_runner(use_mask, ln1_aff, ln2_aff)[0]
    consts = _host_consts(inputs, ln1_aff, ln2_aff)
    maps = [_prep_core_inputs(inputs, b, use_mask, ln1_aff, ln2_aff, consts)
            for b in range(B)]
    res = run(maps)
    out = np.stack([res[b]["out"] for b in range(B)]).astype(np.float32)
    return out



# revision 68
# speedup vs baseline: 1.0321x; 1.0321x over previous
# Trainium2 Bass kernel for nn_NeuralPromptProducerLayer (moe_routing).
# v2: fp8-e4m3 DoubleRow MLP with resident weights, LayerNorms folded into
# matmuls via rank-1 corrections (normalized activations never materialized),
# fused softmax denominator via an augmented-V ones column, batched DMAs.
import sys
sys.path.insert(0, '/opt/trn_rl_repo')

import numpy as np

B, T, D = 8, 704, 1024
RULES, KSLOT, HD = 16, 4, 64
SD, E = 32, 3
DC = D // 128                        # 8 d-chunks
CH = [(0, 128), (128, 256), (256, 384), (384, 512), (512, 640), (640, 704)]
NCH = len(CH)
HALVES = [(0, 352), (352, 704)]
SEGS = [(0, 128), (128, 192), (192, 704)]
SEG_LENS = [128.0, 64.0, 512.0]
FD1 = 512
FD2 = 4096
C2N = FD2 // 128                     # 32
J1N = FD1 // 128                     # 4

# consts-blob column layout (f32, [128 partitions, CBW])
CB_IDENT = 0          # [128, 128] identity
CB_WENT = 128         # [128, 8*32]  W_ent chunks
CB_SEGW = 384         # [128, 18]    seg/len weights per (m, e)
CB_E4 = 402           # [4, 64]
CB_E4T = 466          # [64, 4]
CB_IOTA = 470         # [64, 1]
CB_WQER = 471         # [64, 32]
CB_WKER = 503         # [32, 32]
CB_WQES = 535         # [64, 64]
CB_WKES = 599         # [32, 16]
CB_RET = 615          # [64, 64]
CB_REF = 679          # [64, 64]
CB_BENT = 743         # [32, 1]
CB_CSE = 744          # [1, 32]
CB_CS1 = 776          # [1, 512]
CB_R1 = 1288          # [1, 512]
CB_OC = 1800          # [128, 1] ones column
CB_OR128 = 1801       # [1, 128] ones row
CB_ORT = 1929         # [1, 704] ones row
CB_UR = 2633          # [1, 260] V-aug ones-col selector
CBW = 2896

_RUNNERS = {}


# ---------------------------------------------------------------------------
# TileContext subclass: this walrus build accepts at most ONE sync-wait per
# instruction; split excess waits onto injected NoOps / extra drains.
# ---------------------------------------------------------------------------
def _make_tile_cls():
    from concourse import tile as _tile
    from concourse import mybir as _mybir
    from concourse.vector_clock import ScopedClock

    class TileContextSplit(_tile.TileContext):
        def _lower_ordered_insts(self, ordered):
            for bb_name in list(ordered.keys()):
                insts = ordered[bb_name]
                out = []
                n_new = 0
                for inst in insts:
                    si = getattr(inst, 'sync_info', None)
                    waits = list(si.on_wait) if (si is not None and si.on_wait) else []
                    if len(waits) > 1:
                        for w in waits[:-1]:
                            nop = _mybir.InstNoOp(name=f"{inst.name}-w{n_new}",
                                                  ins=[], outs=[])
                            nop.engine = inst.engine
                            nop.sync_info = _mybir.SyncInfo(on_wait=[w], on_update=[])
                            out.append(nop)
                            n_new += 1
                        si.on_wait = waits[-1:]
                    out.append(inst)
                ordered[bb_name] = out
            return super()._lower_ordered_insts(ordered)

        def _drain_and_barrier(self, tick_clock, wait_clock):
            nc = self.nc
            drain_inst = nc.sync.drain()
            wait_clock.add_sem_waits(
                drain_inst.ins, ScopedClock({None: tick_clock.global_clock}))
            waits = list(drain_inst.ins.sync_info.on_wait or [])
            if len(waits) > 1:
                drain_inst.ins.sync_info.on_wait = waits[:1]
                rest = waits[1:]
                while rest:
                    extra = nc.sync.drain()
                    extra.ins.sync_info = _mybir.SyncInfo(on_wait=rest[:1],
                                                          on_update=[])
                    rest = rest[1:]
            nc.all_engine_barrier()
            assert self.sems is not None
            popped = nc._tile_sem_poison_stack.pop()
            assert popped is self._sem_poison
            nc.clear_and_free_semaphores(list(self.sems.allocated().values()))
            nc.all_engine_barrier()

    return TileContextSplit


# ---------------------------------------------------------------------------
# Program emission
# ---------------------------------------------------------------------------
def _emit(use_mask, ln1_aff, ln2_aff, repeat=1):
    import concourse.bass as bass
    import concourse.mybir as mybir
    from contextlib import ExitStack

    f32 = mybir.dt.float32
    f32r = mybir.dt.float32r
    bf16 = mybir.dt.bfloat16
    fp8 = mybir.dt.float8e4
    i32 = mybir.dt.int32
    AF = mybir.ActivationFunctionType
    AL = mybir.AluOpType
    DR = mybir.MatmulPerfMode.DoubleRow
    X = mybir.AxisListType.X
    TileContextSplit = _make_tile_cls()

    nc = bass.Bass("TRN2", target_bir_lowering=False, num_devices=B)

    xT_d = nc.declare_dram_parameter("xT", [128, DC * T], f32r, isOutput=False)
    xN_d = nc.declare_dram_parameter("xN", [128, NCH * 1024], f32, isOutput=False)
    Wqkv_d = nc.declare_dram_parameter("Wqkv_g", [RULES, 3 * D * HD], bf16,
                                       isOutput=False)
    bcs_d = nc.declare_dram_parameter("bcs_g", [RULES, 576], f32r, isOutput=False)
    mlpw_d = nc.declare_dram_parameter("mlpw", [128, 40960], fp8, isOutput=False)
    cb_d = nc.declare_dram_parameter("cblob", [128, CBW], f32r, isOutput=False)
    wo_d = nc.declare_dram_parameter("Wo_aug", [HD + 1, D], bf16, isOutput=False)
    if use_mask:
        maskT_d = nc.declare_dram_parameter("maskT", [T, T], f32, isOutput=False)
    out_d = nc.declare_dram_parameter("out", [T, D], f32, isOutput=True)

    with ExitStack() as ctx:
        tc = ctx.enter_context(TileContextSplit(nc, pool_alloc_mode="queue"))
        P = ctx.enter_context(tc.tile_pool(name="main", bufs=1))

        def tile(shape, dt, tag):
            return P.tile(shape, dt, tag=tag, name=f"{tag}_u{nc.next_id()}")

        # ================= phase 0: DMAs + consts =================
        xTc = [tile([128, T], f32r, f"XT{c}") for c in range(DC)]
        for c in range(DC):
            nc.sync.dma_start(xTc[c][:], xT_d[:, c * T:(c + 1) * T])
        CB = tile([128, CBW], f32r, "CB")
        nc.sync.dma_start(CB[:], cb_d[:])
        IDF = CB[0:128, CB_IDENT:CB_IDENT + 128].bitcast(f32)
        MW = tile([128, 40960], fp8, "MW")
        WO = tile([HD + 1, D], bf16, "WO")
        XN = tile([128, NCH * 1024], f32, "XN")
        if use_mask:
            maskT = [tile([128, T], f32, f"MK{m}") for m in range(NCH)]
            for m, (t0, t1) in enumerate(CH):
                nc.sync.dma_start(maskT[m][0:t1 - t0, :], maskT_d[t0:t1, :])


        # const views from the blob (f32r) + small memsets
        OC = CB[0:128, CB_OC:CB_OC + 1]
        OR128 = CB[0:1, CB_OR128:CB_OR128 + 128]
        ORT = CB[0:1, CB_ORT:CB_ORT + T]
        UR = CB[0:1, CB_UR:CB_UR + KSLOT * 65]
        ones4 = CB[0:KSLOT, CB_OC:CB_OC + 1]
        OB = tile([128, 1], bf16, "OB")       # ones column bf16
        nc.gpsimd.memset(OB[:], 1.0)
        O8 = tile([128, 1], fp8, "O8")        # ones column fp8
        nc.gpsimd.memset(O8[:], 1.0)
        eps1 = tile([1, 1], f32, "EPS")
        nc.gpsimd.memset(eps1[:], 1e-5)

        # MLP weight views
        W1v = MW[:, 0:4096].rearrange("p (s i m) -> p s i m", s=4, i=2, m=512)
        W4v = MW[:, 4096:8192].rearrange("p (s i m) -> p s i m", s=2, i=2, m=1024)
        W2v = MW[:, 8192:24576].rearrange("p (c s i m) -> p c s i m",
                                          c=C2N, s=2, i=2, m=128)
        W3v = MW[:, 24576:40960].rearrange("p (j r i m) -> p j r i m",
                                           j=J1N, r=16, i=2, m=128)

        # ================= phase 1: LN1 stats + xe =================
        sqt = [tile([128, T], bf16, f"SQ{c % 3}") for c in range(DC)]
        MUR = tile([1, T], f32, "MUR")
        INVR = tile([1, T], f32r, "INVR")
        SIGR = tile([1, T], f32, "T3")
        VARR = tile([1, T], f32, "T2")
        MUSQ = tile([1, T], f32, "T1")
        MIR = tile([1, T], f32, "T1")
        NMS = tile([1, T], f32r, "NMS")
        XE = tile([SD, T], f32, "XE")
        XET = tile([128, NCH * SD], f32, "XET")
        IVT = tile([128, NCH], f32, "IVT")
        W3C = tile([128, 18], f32, "W3C")
        NEGC = tile([1, E], f32, "NEGC")

        with tc.tile_pool(name="ps_ln1", bufs=1, space="PSUM") as ps_ln1:
            ps_st = ps_ln1
            psx = [ps_ln1.tile([1, h1 - h0], f32, tag=f"px{h}",
                               name=f"px{h}_u{nc.next_id()}")
                   for h, (h0, h1) in enumerate(HALVES)]
            psq = [ps_ln1.tile([1, h1 - h0], f32, tag=f"pq{h}",
                               name=f"pq{h}_u{nc.next_id()}")
                   for h, (h0, h1) in enumerate(HALVES)]
            psxe = [ps_ln1.tile([SD, h1 - h0], f32, tag=f"pe{h}",
                                name=f"pe{h}_u{nc.next_id()}")
                    for h, (h0, h1) in enumerate(HALVES)]
            for c in range(DC):
                went_c = CB[0:128, CB_WENT + SD * c:CB_WENT + SD * (c + 1)]
                for h, (h0, h1) in enumerate(HALVES):
                    nc.tensor.matmul(psx[h][:], OC,
                                     xTc[c][:, h0:h1],
                                     start=(c == 0), stop=(c == DC - 1))
                    nc.tensor.matmul(psxe[h][:], went_c,
                                     xTc[c][:, h0:h1],
                                     start=(c == 0), stop=(c == DC - 1))
                if c % 4 == 0:
                    nc.gpsimd.tensor_mul(sqt[c][:], xTc[c][:].bitcast(f32),
                                         xTc[c][:].bitcast(f32))
                elif c % 4 in (1, 2):
                    nc.vector.tensor_mul(sqt[c][:], xTc[c][:].bitcast(f32),
                                         xTc[c][:].bitcast(f32))
                else:
                    nc.scalar.activation(sqt[c][:], xTc[c][:].bitcast(f32),
                                         AF.Square)
            for c in range(DC):
                for h, (h0, h1) in enumerate(HALVES):
                    nc.tensor.matmul(psq[h][:], OB[:], sqt[c][:, h0:h1],
                                     start=(c == 0), stop=(c == DC - 1))
            # rows: mu, var, sig, inv, mu*inv
            for h, (h0, h1) in enumerate(HALVES):
                nc.vector.tensor_scalar_mul(MUR[0:1, h0:h1], psx[h][:], 1.0 / D)
                nc.vector.tensor_mul(MUSQ[0:1, h0:h1], MUR[0:1, h0:h1],
                                     MUR[0:1, h0:h1])
                nc.vector.scalar_tensor_tensor(VARR[0:1, h0:h1], psq[h][:],
                                               1.0 / D, MUSQ[0:1, h0:h1],
                                               AL.mult, AL.subtract)
                nc.scalar.activation(SIGR[0:1, h0:h1], VARR[0:1, h0:h1],
                                     AF.Sqrt, bias=eps1[:])
                with nc.allow_low_precision("f32r rounding is fine here"):
                    nc.vector.reciprocal(INVR[0:1, h0:h1], SIGR[0:1, h0:h1])
                nc.vector.tensor_mul(MIR[0:1, h0:h1], MUR[0:1, h0:h1],
                                     INVR[0:1, h0:h1].bitcast(f32))
            # xe -> SBUF, then per-chunk transposes
            for h, (h0, h1) in enumerate(HALVES):
                nc.vector.tensor_copy(XE[:, h0:h1], psxe[h][:])
            # inv transposed to columns per t-chunk + seg weights
            for m, (t0, t1) in enumerate(CH):
                ts_ = t1 - t0
                pst = ps_st.tile([128, SD], f32, tag="pst",
                                 name=f"pst_u{nc.next_id()}")
                nc.tensor.transpose(pst[0:ts_, 0:SD], XE[:, t0:t1],
                                    IDF[0:SD, 0:SD])
                nc.vector.tensor_copy(XET[0:ts_, m * SD:(m + 1) * SD],
                                      pst[0:ts_, 0:SD])
                psi = ps_st.tile([128, 1], f32, tag="psi",
                                 name=f"psi_u{nc.next_id()}")
                nc.tensor.transpose(psi[0:ts_, 0:1],
                                    INVR[0:1, t0:t1].bitcast(f32),
                                    IDF[0:1, 0:1])
                nc.vector.tensor_copy(IVT[0:ts_, m:m + 1], psi[0:ts_, 0:1])
                nc.vector.tensor_scalar(
                    W3C[:, 3 * m:3 * (m + 1)],
                    CB[0:128, CB_SEGW + 3 * m:CB_SEGW + 3 * (m + 1)].bitcast(f32),
                    IVT[:, m:m + 1], None, AL.mult)
            # neg corr row [1, E]
            for e, (s0, s1) in enumerate(SEGS):
                nc.vector.reduce_sum(NEGC[0:1, e:e + 1], MIR[0:1, s0:s1], axis=X)
                nc.vector.tensor_scalar_mul(NEGC[0:1, e:e + 1],
                                            NEGC[0:1, e:e + 1],
                                            -1.0 / SEG_LENS[e])

        # ============ phase 2: entities + routing (fp32) ============
        entT = tile([SD, E], f32, "ENTT")
        GR = tile([1, T], f32r, "GR")
        SR = tile([1, T], f32r, "SR")
        SBC = tile([128, T], f32, "SBC")
        ridx_i = tile([1, KSLOT], i32, "RIDXI")
        with tc.tile_pool(name="ps_rt", bufs=2, space="PSUM") as ps_rt:
            def rpt(shape, tag="r"):
                return ps_rt.tile(shape, f32, tag=tag, name=f"rt_u{nc.next_id()}")

            ps_ent = rpt([SD, E])
            for m, (t0, t1) in enumerate(CH):
                ts_ = t1 - t0
                nc.tensor.matmul(ps_ent[:], XET[0:ts_, m * SD:(m + 1) * SD],
                                 W3C[0:ts_, 3 * m:3 * (m + 1)],
                                 start=(m == 0), stop=False)
            nc.tensor.matmul(ps_ent[:], CB[0:1, CB_CSE:CB_CSE + SD].bitcast(f32),
                             NEGC[0:1, :], start=False, stop=True)
            nc.vector.tensor_scalar(entT[:], ps_ent[:],
                                    CB[0:SD, CB_BENT:CB_BENT + 1].bitcast(f32),
                                    None, AL.add)

            cb_qer = CB[0:HD, CB_WQER:CB_WQER + SD]
            cb_ker = CB[0:SD, CB_WKER:CB_WKER + SD]
            cb_qes = CB[0:HD, CB_WQES:CB_WQES + 64]
            cb_kes = CB[0:SD, CB_WKES:CB_WKES + 16]
            cb_reT = CB[0:HD, CB_RET:CB_RET + 64]
            cb_ref = CB[0:64, CB_REF:CB_REF + HD]
            cb_E4 = CB[0:KSLOT, CB_E4:CB_E4 + 64]
            cb_E4T = CB[0:64, CB_E4T:CB_E4T + KSLOT]
            cb_iota = CB[0:64, CB_IOTA:CB_IOTA + 1]

            ps_qer = rpt([SD, 64])
            nc.tensor.matmul(ps_qer[:], cb_qer, cb_reT, start=True, stop=True)
            q_erT = tile([SD, 64], f32, "q_erT")
            nc.vector.tensor_copy(q_erT[:], ps_qer[:])

            ps_ker = rpt([SD, E])
            nc.tensor.matmul(ps_ker[:], cb_ker.bitcast(f32), entT[:],
                             start=True, stop=True)
            k_erT = tile([SD, E], f32, "k_erT")
            nc.vector.tensor_copy(k_erT[:], ps_ker[:])

            ps_ser = rpt([64, E])
            nc.tensor.matmul(ps_ser[:], q_erT[:], k_erT[:], start=True, stop=True)
            s_er = tile([64, E], f32, "s_er")
            nc.vector.tensor_scalar_mul(s_er[:], ps_ser[:],
                                        float(1.0 / np.sqrt(SD)))

            m1 = tile([64, 1], f32, "m1")
            nc.vector.reduce_max(m1[:], s_er[:], axis=X)
            ps_m1T = rpt([1, 64])
            nc.tensor.transpose(ps_m1T[:], m1[:], IDF[0:64, 0:64])
            m1T = tile([1, 64], f32, "m1T")
            nc.vector.tensor_copy(m1T[:], ps_m1T[:])
            mk = tile([1, KSLOT], f32, "mk")
            for k in range(KSLOT):
                nc.vector.reduce_max(mk[:, k:k + 1], m1T[:, k * 16:(k + 1) * 16],
                                     axis=X)
            ps_mkT = rpt([KSLOT, 1])
            nc.tensor.transpose(ps_mkT[:], mk[:], IDF[0:1, 0:1])
            mkT = tile([KSLOT, 1], f32, "mkT")
            nc.vector.tensor_copy(mkT[:], ps_mkT[:])
            ps_Mcol = rpt([64, 1])
            nc.tensor.matmul(ps_Mcol[:], cb_E4.bitcast(f32), mkT[:],
                             start=True, stop=True)
            Mcol = tile([64, 1], f32, "Mcol")
            nc.vector.tensor_copy(Mcol[:], ps_Mcol[:])

            onehot = tile([64, E], f32, "onehot")
            nc.vector.tensor_scalar(onehot[:], s_er[:], Mcol[:], None, AL.is_equal)
            rm = tile([64, 1], f32, "rm")
            nc.vector.reduce_sum(rm[:], onehot[:], axis=X)
            rm4 = tile([64, KSLOT], f32, "rm4")
            nc.vector.tensor_scalar(rm4[:], cb_E4T.bitcast(f32), rm[:], None, AL.mult)

            ps_sel = rpt([KSLOT, 64])
            nc.tensor.matmul(ps_sel[:], rm4[:], cb_ref.bitcast(f32),
                             start=True, stop=True)
            sel = tile([KSLOT, 64], f32, "sel")
            nc.vector.tensor_copy(sel[:], ps_sel[:])
            ps_selT = rpt([64, KSLOT])
            nc.tensor.transpose(ps_selT[:], sel[:], IDF[0:KSLOT, 0:KSLOT])
            selT = tile([64, KSLOT], f32, "selT")
            nc.vector.tensor_copy(selT[:], ps_selT[:])

            ps_qes = rpt([16, KSLOT])
            for k in range(KSLOT):
                nc.tensor.matmul(ps_qes[:, k:k + 1],
                                 cb_qes[:, k * 16:(k + 1) * 16].bitcast(f32),
                                 selT[:, k:k + 1], start=(k == 0),
                                 stop=(k == KSLOT - 1))
            q_esT = tile([16, KSLOT], f32, "q_esT")
            nc.vector.tensor_copy(q_esT[:], ps_qes[:])
            ps_kes = rpt([16, E])
            nc.tensor.matmul(ps_kes[:], cb_kes.bitcast(f32), entT[:],
                             start=True, stop=True)
            k_esT = tile([16, E], f32, "k_esT")
            nc.vector.tensor_copy(k_esT[:], ps_kes[:])
            ps_sesT = rpt([E, KSLOT])
            nc.tensor.matmul(ps_sesT[:], k_esT[:], q_esT[:], start=True, stop=True)
            s_esT = tile([E, KSLOT], f32, "s_esT")
            nc.vector.tensor_scalar_mul(s_esT[:], ps_sesT[:],
                                        float(1.0 / np.sqrt(SD // 2)))
            ps_ses = rpt([KSLOT, E])
            nc.tensor.transpose(ps_ses[:], s_esT[:], IDF[0:E, 0:E])
            s_es = tile([KSLOT, E], f32, "s_es")
            nc.vector.tensor_copy(s_es[:], ps_ses[:])
            em1 = tile([KSLOT, 1], f32, "em1")
            nc.vector.reduce_max(em1[:], s_es[:], axis=X)
            eoh = tile([KSLOT, E], f32, "eoh")
            nc.vector.tensor_scalar(eoh[:], s_es[:], em1[:], None, AL.is_equal)

            ps_crow = rpt([1, E])
            nc.tensor.matmul(ps_crow[:], ones4.bitcast(f32), eoh[:],
                             start=True, stop=True)
            c_row = tile([1, E], f32, "c_row")
            nc.vector.tensor_copy(c_row[:], ps_crow[:])

            ps_ridx = rpt([KSLOT, 1])
            nc.tensor.matmul(ps_ridx[:], rm4[:], cb_iota.bitcast(f32),
                             start=True, stop=True)
            ridx_col = tile([KSLOT, 1], f32, "ridx_col")
            nc.vector.tensor_copy(ridx_col[:], ps_ridx[:])
            ps_ridxT = rpt([1, KSLOT])
            nc.tensor.transpose(ps_ridxT[:], ridx_col[:], IDF[0:KSLOT, 0:KSLOT])
            ridx_f = tile([1, KSLOT], f32, "ridx_f")
            nc.vector.tensor_copy(ridx_f[:], ps_ridxT[:])
            nc.vector.tensor_copy(ridx_i[:], ridx_f[:])

            # gate-count row, s row = gate * inv, nms row = -mu * s
            for e, (s0, s1) in enumerate(SEGS):
                nc.vector.tensor_scalar(GR[0:1, s0:s1],
                                        ORT[0:1, s0:s1].bitcast(f32),
                                        c_row[0:1, e:e + 1], None, AL.mult)
            nc.vector.tensor_mul(SR[0:1, :], GR[0:1, :].bitcast(f32),
                                 INVR[0:1, :].bitcast(f32))
            nc.vector.scalar_tensor_tensor(NMS[0:1, :], MUR[0:1, :], -1.0,
                                           SR[0:1, :].bitcast(f32),
                                           AL.mult, AL.mult)

        # ============ phase 3: gather + s_bc + xs ============
        wqkv = [tile([128, 1536], bf16, f"G{k}") for k in range(KSLOT)]
        bcs = [tile([1, 576], f32r, f"BCS{k}") for k in range(KSLOT)]
        for k in range(KSLOT):
            reg = nc.gpsimd.alloc_register(f"ridx{k}")
            nc.gpsimd.reg_load(reg, ridx_i[0:1, k:k + 1])
            off = nc.gpsimd.snap(reg, donate=True, min_val=0, max_val=RULES - 1)
            src_w = Wqkv_d[bass.ds(off, 1), :].rearrange(
                "a (p f) -> (a p) f", p=128, f=1536)
            nc.gpsimd.dma_start(wqkv[k][:], src_w)
            nc.gpsimd.dma_start(bcs[k][:], bcs_d[bass.ds(off, 1), :])
        # Defer the big XN/WO/MW loads until the routing-gated gather lands:
        # DMA-engine grants are FIFO by request time, so an early request
        # starves the gather. A tiny gated read of each target makes the DMA
        # wait via WAR.
        gate = tile([1, 2], f32, "GATE")
        wq3v = wqkv[3][0:1, 0:4].bitcast(f32)
        nc.gpsimd.memset(XN[0:1, 0:2], 0.0)
        nc.gpsimd.memset(WO[0:1, 0:4], 0.0)
        nc.gpsimd.memset(MW[0:1, 0:8], 0.0)
        nc.vector.scalar_tensor_tensor(gate[:], XN[0:1, 0:2], 0.0, wq3v,
                                       AL.mult, AL.add)
        nc.sync.dma_start(XN[:], xN_d[:])
        nc.vector.scalar_tensor_tensor(gate[:], WO[0:1, 0:4].bitcast(f32), 0.0,
                                       wq3v, AL.mult, AL.add)
        nc.sync.dma_start(WO[:], wo_d[:])
        nc.vector.scalar_tensor_tensor(gate[:], MW[0:1, 0:8].bitcast(f32), 0.0,
                                       wq3v, AL.mult, AL.add)
        for q in range(4):
            nc.sync.dma_start(MW[:, q * 10240:(q + 1) * 10240],
                              mlpw_d[:, q * 10240:(q + 1) * 10240])

        with tc.tile_pool(name="ps_bc", bufs=1, space="PSUM") as ps_bc:
            for h, (h0, h1) in enumerate(HALVES):
                psb = ps_bc.tile([128, h1 - h0], f32, tag=f"sb{h}",
                                 name=f"sb{h}_u{nc.next_id()}")
                nc.tensor.matmul(psb[:], OR128, SR[0:1, h0:h1],
                                 start=True, stop=True)
                nc.vector.tensor_copy(SBC[:, h0:h1], psb[:])

        xs = [tile([128, T], bf16, f"XS{c}") for c in range(DC)]
        for c in range(DC):
            nc.vector.tensor_mul(xs[c][:], xTc[c][:].bitcast(f32), SBC[:])

        # wv_all [128, c=8, k*65+j] from gathered v-parts; csv row; bvsum col
        WVA = tile([128, DC * KSLOT * 65], bf16, "WVA")
        nc.gpsimd.memset(WVA[:], 0.0)
        for k in range(KSLOT):
            nc.vector.tensor_copy(
                WVA.rearrange("p (c k u) -> p c k u", c=DC, k=KSLOT, u=65)
                [:, :, k, 0:64],
                wqkv[k].rearrange("p (c three j) -> p c three j",
                                  c=DC, three=3, j=HD)[:, :, 2, :])
        CSV = tile([1, KSLOT * 65], f32r, "CSV")
        nc.gpsimd.memset(CSV[:].bitcast(f32), 0.0)
        for k in range(KSLOT):
            nc.vector.tensor_copy(CSV[0:1, k * 65:k * 65 + 64],
                                  bcs[k][0:1, 320:384].bitcast(f32))
        if ln1_aff:
            CBV = tile([1, KSLOT * 65], f32r, "CBV")
            nc.gpsimd.memset(CBV[:].bitcast(f32), 0.0)
            for k in range(KSLOT):
                nc.vector.tensor_copy(CBV[0:1, k * 65:k * 65 + 64],
                                      bcs[k][0:1, 512:576].bitcast(f32))
        bv01 = tile([1, HD], f32, "BV01")
        nc.vector.tensor_add(bv01[:], bcs[0][0:1, 128:192].bitcast(f32),
                             bcs[1][0:1, 128:192].bitcast(f32))
        bv23 = tile([1, HD], f32, "BV23")
        nc.vector.tensor_add(bv23[:], bcs[2][0:1, 128:192].bitcast(f32),
                             bcs[3][0:1, 128:192].bitcast(f32))
        bvr = tile([1, HD], f32, "BVR")
        nc.vector.tensor_add(bvr[:], bv01[:], bv23[:])
        BVS = tile([HD, 1], f32, "BVS")
        with tc.tile_pool(name="ps_bv", bufs=1, space="PSUM") as ps_bv:
            psv = ps_bv.tile([HD, 1], f32, tag="bv", name=f"bv_u{nc.next_id()}")
            nc.tensor.transpose(psv[:], bvr[:], IDF[0:1, 0:1])
            nc.vector.tensor_copy(BVS[:], psv[:])

        # ============ phase 4: QKV + V ============
        qT = [tile([HD, T], bf16, f"QT{k}") for k in range(KSLOT)]
        kT = [tile([HD, T], bf16, f"KT{k}") for k in range(KSLOT)]
        V_aug = [tile([128, KSLOT * 65], bf16, f"VA{m}") for m in range(NCH)]
        with tc.tile_pool(name="ps_qkv", bufs=2, space="PSUM") as ps_qkv, \
             tc.tile_pool(name="ps_v", bufs=2, space="PSUM") as ps_v:
            for k in range(KSLOT):
                for h, (h0, h1) in enumerate(HALVES):
                    ps = ps_qkv.tile([128, h1 - h0], f32, tag="qk",
                                     name=f"psqk_u{nc.next_id()}")
                    for c in range(DC):
                        lhs_qk = wqkv[k].rearrange(
                            "p (c three j) -> p c three j",
                            c=DC, three=3, j=HD)[:, c, 0:2, :]
                        nc.tensor.matmul(ps[:], lhs_qk, xs[c][:, h0:h1],
                                         start=(c == 0), stop=False)
                    nc.tensor.matmul(ps[:], bcs[k][0:1, 0:128],
                                     ORT[0:1, h0:h1],
                                     start=False, stop=False)
                    nc.tensor.matmul(ps[:], bcs[k][0:1, 192:320],
                                     NMS[0:1, h0:h1],
                                     start=False, stop=(not ln1_aff))
                    if ln1_aff:
                        nc.tensor.matmul(ps[:], bcs[k][0:1, 384:512],
                                         GR[0:1, h0:h1],
                                         start=False, stop=True)
                    nc.vector.tensor_copy(qT[k][:, h0:h1], ps[0:HD, :])
                    nc.vector.tensor_copy(kT[k][:, h0:h1], ps[HD:128, :])
            for m, (t0, t1) in enumerate(CH):
                ts_ = t1 - t0
                ps = ps_v.tile([128, KSLOT * 65], f32, tag="v",
                               name=f"psv_u{nc.next_id()}")
                for c in range(DC):
                    nc.tensor.matmul(ps[0:ts_, :], xs[c][:, t0:t1],
                                     WVA.rearrange("p (c u) -> p c u",
                                                   c=DC, u=KSLOT * 65)[:, c, :],
                                     start=(c == 0), stop=False)
                nc.tensor.matmul(ps[0:ts_, :], NMS[0:1, t0:t1],
                                 CSV[:], start=False, stop=False)
                nc.tensor.matmul(ps[0:ts_, :], ORT[0:1, t0:t1],
                                 UR, start=False,
                                 stop=(not ln1_aff))
                if ln1_aff:
                    nc.tensor.matmul(ps[0:ts_, :], GR[0:1, t0:t1],
                                     CBV[:], start=False, stop=True)
                nc.scalar.copy(V_aug[m][0:ts_, :], ps[0:ts_, :])

        # ============ phase 5: attention ============
        exp_tags = ["G0", "G1", "G2", "G3", "WVA", "XE"]
        expT = [tile([128, T], bf16, exp_tags[m]) for m in range(NCH)]
        aoT = tile([HD + 1, T], bf16, "AOT")
        nc.gpsimd.memset(aoT[HD:HD + 1, :], 4.0)
        with tc.tile_pool(name="ps_sc", bufs=2, space="PSUM") as ps_sc, \
             tc.tile_pool(name="ps_av", bufs=2, space="PSUM") as ps_av, \
             tc.tile_pool(name="ps_rb", bufs=2, space="PSUM") as ps_rb:
            for h, (h0, h1) in enumerate(HALVES):
                n = h1 - h0
                tmps = []
                for k in range(KSLOT):
                    for m, (t0, t1) in enumerate(CH):
                        ts_ = t1 - t0
                        ps = ps_sc.tile([128, n], f32, tag="sc",
                                        name=f"sc_u{nc.next_id()}")
                        if use_mask:
                            nc.vector.tensor_scalar(ps[0:ts_, :],
                                                    maskT[m][0:ts_, h0:h1],
                                                    8.0, None, AL.mult)
                            first = False
                        else:
                            first = True
                        nc.tensor.matmul(ps[0:ts_, :], kT[k][:, t0:t1],
                                         qT[k][:, h0:h1], start=first, stop=True)
                        nc.scalar.activation(expT[m][0:ts_, h0:h1], ps[0:ts_, :],
                                             AF.Exp, scale=0.125)
                    av = ps_av.tile([HD + 1, n], f32, tag="av",
                                    name=f"av_u{nc.next_id()}")
                    for m, (t0, t1) in enumerate(CH):
                        ts_ = t1 - t0
                        nc.tensor.matmul(av[:], V_aug[m][0:ts_, k * 65:(k + 1) * 65],
                                         expT[m][0:ts_, h0:h1],
                                         start=(m == 0), stop=(m == NCH - 1))
                    rec = tile([1, n], f32r, f"REC{k % 2}")
                    with nc.allow_low_precision("softmax renorm"):
                        nc.vector.reciprocal(rec[:], av[HD:HD + 1, :])
                    rbp = ps_rb.tile([HD, n], f32, tag="rb",
                                     name=f"rb_u{nc.next_id()}")
                    nc.tensor.matmul(rbp[:], OR128[0:1, 0:HD],
                                     rec[:], start=True, stop=True)
                    rb = tile([HD, n], f32, f"RB{k % 2}")
                    nc.scalar.copy(rb[:], rbp[:])
                    tmp = tile([HD, n], f32, f"TMP{k}")
                    nc.vector.tensor_mul(tmp[:], av[0:HD, :], rb[:])
                    tmps.append(tmp)
                nc.gpsimd.tensor_add(tmps[0][:], tmps[0][:], tmps[1][:])
                nc.gpsimd.tensor_add(tmps[2][:], tmps[2][:], tmps[3][:])
                nc.vector.tensor_add(tmps[0][:], tmps[0][:], tmps[2][:])
                nc.vector.tensor_scalar(aoT[0:HD, h0:h1], tmps[0][:], BVS[:],
                                        None, AL.add)

        # ============ phase 6: Wo + residuals (h2q fp8, h2N f32) ============
        h2q = [tile([128, 2, T], fp8, f"KT{p}") for p in range(4)]
        with tc.tile_pool(name="ps_wo", bufs=2, space="PSUM") as ps_wo:
            for c in range(DC):
                for h, (h0, h1) in enumerate(HALVES):
                    ps = ps_wo.tile([128, h1 - h0], f32, tag="woT",
                                    name=f"woT_u{nc.next_id()}")
                    nc.tensor.matmul(ps[:], WO[:, c * 128:(c + 1) * 128],
                                     aoT[:, h0:h1], start=True, stop=True)
                    nc.vector.tensor_add(h2q[c // 2][:, c % 2, h0:h1],
                                         xTc[c][:, h0:h1].bitcast(f32), ps[:])
        # ============ phase 7: LN2 stats on h2q ============
        MU2 = tile([1, T], f32, "MUR")
        NM2 = tile([1, T], f32r, "NMS")
        SG2 = tile([1, T], f32, "T3")
        VA2 = tile([1, T], f32, "T2")
        MQ2 = tile([1, T], f32, "T1")
        IV2 = tile([1, T], f32r, "INVR")
        sq2 = [tile([128, T], bf16, f"SQ{i % 3}") for i in range(DC)]
        with tc.tile_pool(name="ps_ln2", bufs=1, space="PSUM") as ps_ln2:
            px2 = [ps_ln2.tile([1, h1 - h0], f32, tag=f"p2x{h}",
                               name=f"p2x{h}_u{nc.next_id()}")
                   for h, (h0, h1) in enumerate(HALVES)]
            pq2 = [ps_ln2.tile([1, h1 - h0], f32, tag=f"p2q{h}",
                               name=f"p2q{h}_u{nc.next_id()}")
                   for h, (h0, h1) in enumerate(HALVES)]
            for c in range(DC):
                src = h2q[c // 2][:, c % 2, :]
                for h, (h0, h1) in enumerate(HALVES):
                    nc.tensor.matmul(px2[h][:], O8[:], src[:, h0:h1],
                                     start=(c == 0), stop=(c == DC - 1))
                nc.scalar.activation(sq2[c][:], src, AF.Square)
            for c in range(DC):
                for h, (h0, h1) in enumerate(HALVES):
                    nc.tensor.matmul(pq2[h][:], OB[:], sq2[c][:, h0:h1],
                                     start=(c == 0), stop=(c == DC - 1))
            for h, (h0, h1) in enumerate(HALVES):
                nc.vector.tensor_scalar_mul(MU2[0:1, h0:h1], px2[h][:], 1.0 / D)
                nc.vector.tensor_mul(MQ2[0:1, h0:h1], MU2[0:1, h0:h1],
                                     MU2[0:1, h0:h1])
                nc.vector.scalar_tensor_tensor(VA2[0:1, h0:h1], pq2[h][:],
                                               1.0 / D, MQ2[0:1, h0:h1],
                                               AL.mult, AL.subtract)
                nc.scalar.activation(SG2[0:1, h0:h1], VA2[0:1, h0:h1],
                                     AF.Sqrt, bias=eps1[:])
                with nc.allow_low_precision("f32r rounding is fine here"):
                    nc.vector.reciprocal(IV2[0:1, h0:h1], SG2[0:1, h0:h1])
                nc.vector.tensor_scalar_mul(NM2[0:1, h0:h1], MU2[0:1, h0:h1],
                                            -1.0)
        # ============ phase 8: MLP (fp8 DoubleRow) ============
        y1q = [tile([128, 2, T], fp8, f"SQ{p}") for p in range(2)]
        y2_tags = ([f"XS{c}" for c in range(DC)] + [f"G{k}" for k in range(KSLOT)]
                   + ["WVA", "XE", "SBC", "Y2F"])
        if use_mask:
            y2_tags = ([f"MK{m}" for m in range(NCH)]
                       + [f"XS{c}" for c in range(DC)] + ["G0", "G1"])
        y2q = [tile([128, 2, T], fp8, y2_tags[p]) for p in range(16)]
        y3q = [tile([128, 2, T], fp8, f"QT{p}") for p in range(2)]

        with tc.tile_pool(name="ps_i2", bufs=1, space="PSUM") as ps_i2, \
             tc.tile_pool(name="ps_y1", bufs=2, space="PSUM") as ps_y1:
            i2bc = []
            for h, (h0, h1) in enumerate(HALVES):
                pi = ps_i2.tile([128, h1 - h0], f32, tag=f"i2{h}",
                                name=f"i2{h}_u{nc.next_id()}")
                nc.tensor.matmul(pi[:], OR128, IV2[0:1, h0:h1],
                                 start=True, stop=True)
                i2bc.append(pi)
            for j in range(J1N):
                for h, (h0, h1) in enumerate(HALVES):
                    ps = ps_y1.tile([128, h1 - h0], f32, tag="y1",
                                    name=f"y1_u{nc.next_id()}")
                    for s in range(4):
                        nc.tensor.matmul(ps[:], W1v[:, s, :, j * 128:(j + 1) * 128],
                                         h2q[s][:, :, h0:h1], perf_mode=DR,
                                         start=(s == 0), stop=False)
                    nc.tensor.matmul(
                        ps[:], CB[0:1, CB_CS1 + j * 128:CB_CS1 + (j + 1) * 128],
                        NM2[0:1, h0:h1], start=False, stop=(not ln2_aff))
                    if ln2_aff:
                        nc.tensor.matmul(
                            ps[:], CB[0:1, CB_R1 + j * 128:CB_R1 + (j + 1) * 128],
                            SG2[0:1, h0:h1].bitcast(f32r),
                            start=False, stop=True)
                    nc.scalar.activation(y1q[j // 2][:, j % 2, h0:h1], ps[:],
                                         AF.Relu)
            for p in range(2):
                for i in range(2):
                    for h, (h0, h1) in enumerate(HALVES):
                        nc.vector.tensor_mul(y1q[p][:, i, h0:h1],
                                             y1q[p][:, i, h0:h1], i2bc[h][:])

        with tc.tile_pool(name="ps_y2", bufs=3, space="PSUM") as ps_y2, \
             tc.tile_pool(name="ps_y3", bufs=2, space="PSUM") as ps_y3, \
             tc.tile_pool(name="ps_y4", bufs=2, space="PSUM") as ps_y4:
            for m, (t0, t1) in enumerate(CH):
                ts_ = t1 - t0
                for dh in range(2):
                    d0 = dh * 512
                    ps = ps_y4.tile([128, 512], f32, tag="y4",
                                    name=f"woN_u{nc.next_id()}")
                    nc.tensor.matmul(ps[0:ts_, :], aoT[:, t0:t1],
                                     WO[:, d0:d0 + 512], start=True, stop=True)
                    nc.vector.tensor_add(
                        XN[0:ts_, m * 1024 + d0:m * 1024 + d0 + 512],
                        XN[0:ts_, m * 1024 + d0:m * 1024 + d0 + 512],
                        ps[0:ts_, :])
            for c2 in range(C2N):
                for h, (h0, h1) in enumerate(HALVES):
                    ps = ps_y2.tile([128, h1 - h0], f32, tag="y2",
                                    name=f"y2_u{nc.next_id()}")
                    for s in range(2):
                        nc.tensor.matmul(ps[:], W2v[:, c2, s],
                                         y1q[s][:, :, h0:h1], perf_mode=DR,
                                         start=(s == 0), stop=(s == 1))
                    nc.scalar.activation(y2q[c2 // 2][:, c2 % 2, h0:h1], ps[:],
                                         AF.Gelu)
            for j in range(J1N):
                for h, (h0, h1) in enumerate(HALVES):
                    ps = ps_y3.tile([128, h1 - h0], f32, tag="y3",
                                    name=f"y3_u{nc.next_id()}")
                    for r in range(16):
                        nc.tensor.matmul(ps[:], W3v[:, j, r],
                                         y2q[r][:, :, h0:h1], perf_mode=DR,
                                         start=(r == 0), stop=(r == 15))
                    nc.scalar.activation(y3q[j // 2][:, j % 2, h0:h1], ps[:],
                                         AF.Relu)
            for m, (t0, t1) in enumerate(CH):
                ts_ = t1 - t0
                for dh in range(2):
                    d0 = dh * 512
                    ps = ps_y4.tile([128, 512], f32, tag="y4",
                                    name=f"y4_u{nc.next_id()}")
                    for p in range(2):
                        nc.tensor.matmul(ps[0:ts_, :], y3q[p][:, :, t0:t1],
                                         W4v[:, p, :, d0:d0 + 512], perf_mode=DR,
                                         start=(p == 0), stop=(p == 1))
                    nc.vector.tensor_add(
                        XN[0:ts_, m * 1024 + d0:m * 1024 + d0 + 512],
                        XN[0:ts_, m * 1024 + d0:m * 1024 + d0 + 512],
                        ps[0:ts_, :])
                nc.sync.dma_start(out_d[t0:t1, :],
                                  XN[0:ts_, m * 1024:(m + 1) * 1024])

    return nc


# ---------------------------------------------------------------------------
# Host-side input prep
# ---------------------------------------------------------------------------
def _host_consts(inputs, ln1_aff, ln2_aff):
    """Batch-independent tensors (weights), computed once."""
    import ml_dtypes
    f = np.float32
    e4 = ml_dtypes.float8_e4m3
    d = {}

    g1 = np.asarray(inputs["ln1_g"], f) if ln1_aff else None
    g2 = np.asarray(inputs["ln2_g"], f) if ln2_aff else None
    b1 = np.asarray(inputs["ln1_b"], f) if ln1_aff else None
    b2 = np.asarray(inputs["ln2_b"], f) if ln2_aff else None

    # gathered per-rule QKV weights (g1 folded in if affine)
    blks = []
    for Wn in ("Wq", "Wk", "Wv"):
        W = np.asarray(inputs[Wn], f)
        if ln1_aff:
            W = W * g1[:, None]
        blks.append(W.reshape(DC, 128, RULES, HD).transpose(2, 0, 1, 3))
    d["Wqkv_g"] = np.ascontiguousarray(
        np.stack(blks, axis=2).transpose(0, 3, 1, 2, 4)
        .reshape(RULES, 3 * D * HD)).astype(ml_dtypes.bfloat16)

    # bcs row per rule: [bq bk bv csq csk csv cbq cbk cbv] (9*64 = 576)
    bias = np.concatenate(
        [np.asarray(inputs[bn], f).reshape(RULES, HD) for bn in ("bq", "bk", "bv")],
        axis=1)                                             # [R, 192]
    # blks[i] is [R, DC, 128, HD]; column sums over d per rule head
    csums = np.concatenate(
        [blk.reshape(RULES, D, HD).sum(1) for blk in blks], axis=1)  # [R, 192]
    if ln1_aff:
        cb = np.concatenate(
            [np.einsum('d,drh->rh', b1,
                       (np.asarray(inputs[Wn], f) * g1[:, None])
                       .reshape(D, RULES, HD))
             for Wn in ("Wq", "Wk", "Wv")], axis=1)
    else:
        cb = np.zeros((RULES, 192), f)
    d["bcs_g"] = np.ascontiguousarray(np.concatenate([bias, csums, cb], axis=1))

    # Wo_aug bf16
    d["Wo_aug"] = np.ascontiguousarray(np.concatenate(
        [np.asarray(inputs["Wo"], f), np.asarray(inputs["bo"], f)[None, :]],
        0)).astype(ml_dtypes.bfloat16)

    # MLP weights fp8, DoubleRow layouts
    W1 = np.asarray(inputs["fc1_w1"], f)
    if ln2_aff:
        W1 = W1 * g2[:, None]
    W1q = W1.astype(e4)
    W2q = np.asarray(inputs["fc1_w2"], f).astype(e4)
    W3q = np.asarray(inputs["fc2_w1"], f).astype(e4)
    W4q = np.asarray(inputs["fc2_w2"], f).astype(e4)
    # W1 [1024, 512] -> [p, s, i, m]
    w1 = W1q.reshape(4, 2, 128, FD1).transpose(2, 0, 1, 3).reshape(128, 4096)
    # W4 [512, 1024] -> [p, s, i, d]
    w4 = W4q.reshape(2, 2, 128, D).transpose(2, 0, 1, 3).reshape(128, 4096)
    # W2 [512, 4096] -> [p, c2, s, i, m]
    w2 = (W2q.reshape(2, 2, 128, C2N, 128).transpose(2, 3, 0, 1, 4)
          .reshape(128, 16384))
    # W3 [4096, 512] -> [p, j, r, i, m]
    w3 = (W3q.reshape(16, 2, 128, J1N, 128).transpose(2, 3, 0, 1, 4)
          .reshape(128, 16384))
    d["mlpw"] = np.ascontiguousarray(np.concatenate([w1, w4, w2, w3], axis=1))

    # consts blob
    cb_arr = np.zeros((128, CBW), f)
    cb_arr[:, CB_IDENT:CB_IDENT + 128] = np.eye(128, dtype=f)
    went = np.asarray(inputs["W_ent"], f)
    if ln1_aff:
        went = went * g1[:, None]
    cb_arr[:, CB_WENT:CB_WENT + DC * SD] = (
        went.reshape(DC, 128, SD).transpose(1, 0, 2).reshape(128, DC * SD))
    segw = np.zeros((128, NCH * E), f)
    for m, (t0, t1) in enumerate(CH):
        for p in range(t1 - t0):
            t = t0 + p
            for e, (s0, s1) in enumerate(SEGS):
                if s0 <= t < s1:
                    segw[p, m * E + e] = 1.0 / SEG_LENS[e]
    cb_arr[:, CB_SEGW:CB_SEGW + NCH * E] = segw
    E4 = np.kron(np.eye(KSLOT, dtype=f), np.ones((1, RULES), f))
    cb_arr[0:KSLOT, CB_E4:CB_E4 + 64] = E4
    cb_arr[0:64, CB_E4T:CB_E4T + KSLOT] = E4.T
    cb_arr[0:64, CB_IOTA] = (np.arange(64) % RULES).astype(f)
    cb_arr[0:HD, CB_WQER:CB_WQER + SD] = np.asarray(inputs["Wq_er"], f)
    cb_arr[0:SD, CB_WKER:CB_WKER + SD] = np.asarray(inputs["Wk_er"], f)
    Wqes = np.asarray(inputs["Wq_es"], f)          # [K, HD, SD//2]
    cb_arr[0:HD, CB_WQES:CB_WQES + 64] = (
        Wqes.transpose(1, 0, 2).reshape(HD, KSLOT * (SD // 2)))
    cb_arr[0:SD, CB_WKES:CB_WKES + 16] = np.asarray(inputs["Wk_es"], f)
    re = np.asarray(inputs["rules_embed"], f)      # [K, R, HD]
    cb_arr[0:HD, CB_RET:CB_RET + 64] = re.transpose(2, 0, 1).reshape(HD, 64)
    cb_arr[0:64, CB_REF:CB_REF + HD] = re.reshape(64, HD)
    bent = np.asarray(inputs["b_ent"], f)
    if ln1_aff:
        bent = bent + b1 @ went
    cb_arr[0:SD, CB_BENT] = bent
    cb_arr[0, CB_CSE:CB_CSE + SD] = went.sum(0)
    cb_arr[0, CB_CS1:CB_CS1 + FD1] = W1q.astype(f).sum(0)
    if ln2_aff:
        cb_arr[0, CB_R1:CB_R1 + FD1] = b2 @ W1
    cb_arr[:, CB_OC] = 1.0
    cb_arr[0, CB_OR128:CB_OR128 + 128] = 1.0
    cb_arr[0, CB_ORT:CB_ORT + T] = 1.0
    ur = np.zeros(KSLOT * 65, f)
    ur[64::65] = 1.0
    cb_arr[0, CB_UR:CB_UR + KSLOT * 65] = ur
    d["cblob"] = np.ascontiguousarray(cb_arr)
    return d


def _prep_core_inputs(inputs, b, use_mask, ln1_aff, ln2_aff, consts=None):
    f = np.float32
    if consts is None:
        consts = _host_consts(inputs, ln1_aff, ln2_aff)
    d = dict(consts)
    hs = np.asarray(inputs["hidden_states"], f)
    x = hs[b]                                      # [T, D]
    xT = np.ascontiguousarray(x.T)                 # [D, T]
    d["xT"] = np.ascontiguousarray(
        xT.reshape(DC, 128, T).transpose(1, 0, 2).reshape(128, DC * T))
    xn = np.zeros((128, NCH * 1024), f)
    for m, (t0, t1) in enumerate(CH):
        xn[0:t1 - t0, m * 1024:(m + 1) * 1024] = x[t0:t1]
    d["xN"] = xn
    if use_mask:
        d["maskT"] = np.ascontiguousarray(
            np.asarray(inputs["attention_mask"], f)[b].T)
    return d


# ---------------------------------------------------------------------------
# Runner (jax/axon shard_map over 8 cores)
# ---------------------------------------------------------------------------
def _build_runner(use_mask, ln1_aff, ln2_aff, repeat=1):
    key = (use_mask, ln1_aff, ln2_aff, repeat)
    if key in _RUNNERS:
        return _RUNNERS[key]
    import jax
    from jax.sharding import Mesh, PartitionSpec
    from jax.experimental.shard_map import shard_map
    from concourse import mybir
    from concourse.bass2jax import (_bass_exec_p, install_neuronx_cc_hook,
                                    partition_id_tensor)

    nc = _emit(use_mask, ln1_aff, ln2_aff, repeat)
    install_neuronx_cc_hook()
    partition_name = nc.partition_id_tensor.name if nc.partition_id_tensor else None
    in_names, out_names, out_avals, zero_shapes = [], [], [], []
    for alloc in nc.m.functions[0].allocations:
        if not isinstance(alloc, mybir.MemoryLocationSet):
            continue
        name = alloc.memorylocations[0].name
        if alloc.kind == "ExternalInput":
            if name != partition_name:
                in_names.append(name)
        elif alloc.kind == "ExternalOutput":
            out_names.append(name)
            shape = tuple(alloc.tensor_shape)
            dtype = mybir.dt.np(alloc.dtype)
            out_avals.append(jax.core.ShapedArray(shape, dtype))
            zero_shapes.append((shape, dtype))
    n_params = len(in_names)
    n_outs = len(out_avals)
    all_in_names = list(in_names) + list(out_names)
    if partition_name is not None:
        all_in_names.append(partition_name)

    def _body(*args):
        operands = list(args)
        if partition_name is not None:
            operands.append(partition_id_tensor())
        outs = _bass_exec_p.bind(
            *operands, out_avals=tuple(out_avals), in_names=tuple(all_in_names),
            out_names=tuple(out_names), lowering_input_output_aliases=(),
            sim_require_finite=False, sim_require_nnan=False, nc=nc)
        return tuple(outs)

    devices = jax.devices()[:B]
    mesh = Mesh(np.asarray(devices), ("core",))
    in_specs = (PartitionSpec("core"),) * (n_params + n_outs)
    out_specs = (PartitionSpec("core"),) * n_outs
    sharded = jax.jit(
        shard_map(_body, mesh=mesh, in_specs=in_specs, out_specs=out_specs,
                  check_rep=False),
        keep_unused=True)

    def run(per_core_maps):
        concat_in = [
            np.concatenate([np.asarray(per_core_maps[c][nm]) for c in range(B)], 0)
            for nm in in_names]
        concat_zeros = [np.zeros((B * s[0], *s[1:]), dt) for s, dt in zero_shapes]
        out_arrs = jax.block_until_ready(sharded(*concat_in, *concat_zeros))
        return [
            {nm: np.asarray(out_arrs[i]).reshape(B, *out_avals[i].shape)[c]
             for i, nm in enumerate(out_names)}
            for c in range(B)]

    _RUNNERS[key] = (run, sharded, in_names, zero_shapes, out_names, out_avals)
    return _RUNNERS[key]


def kernel(**inputs):
    use_mask = bool(np.any(np.asarray(inputs["attention_mask"])))
    ln1_aff = not (np.all(np.asarray(inputs["ln1_g"]) == 1.0)
                   and np.all(np.asarray(inputs["ln1_b"]) == 0.0))
    ln2_aff = not (np.all(np.asarray(inputs["ln2_g"]) == 1.0)
                   and np.all(np.asarray(inputs["ln2_b"]) == 0.0))
    run = _build_runner(use_mask, ln1_aff, ln2_aff)[0]
    consts = _host_consts(inputs, ln1_aff, ln2_aff)
    maps = [_prep_core_inputs(inputs, b, use_mask, ln1_aff, ln2_aff, consts)
            for b in range(B)]
    res = run(maps)
    out = np.stack([res[b]["out"] for b in range(B)]).astype(np.float32)
    return out



# revision 69
# speedup vs baseline: 1.0532x; 1.0204x over previous
# Trainium2 Bass kernel for nn_NeuralPromptProducerLayer (moe_routing).
# v2: fp8-e4m3 DoubleRow MLP with resident weights, LayerNorms folded into
# matmuls via rank-1 corrections (normalized activations never materialized),
# fused softmax denominator via an augmented-V ones column, batched DMAs.
import sys
sys.path.insert(0, '/opt/trn_rl_repo')

import numpy as np

B, T, D = 8, 704, 1024
RULES, KSLOT, HD = 16, 4, 64
SD, E = 32, 3
DC = D // 128                        # 8 d-chunks
CH = [(0, 128), (128, 256), (256, 384), (384, 512), (512, 640), (640, 704)]
NCH = len(CH)
HALVES = [(0, 352), (352, 704)]
SEGS = [(0, 128), (128, 192), (192, 704)]
SEG_LENS = [128.0, 64.0, 512.0]
FD1 = 512
FD2 = 4096
C2N = FD2 // 128                     # 32
J1N = FD1 // 128                     # 4

# consts-blob column layout (f32, [128 partitions, CBW])
CB_IDENT = 0          # [128, 128] identity
CB_WENT = 128         # [128, 8*32]  W_ent chunks
CB_SEGW = 384         # [128, 18]    seg/len weights per (m, e)
CB_E4 = 402           # [4, 64]
CB_E4T = 466          # [64, 4]
CB_IOTA = 470         # [64, 1]
CB_WQER = 471         # [64, 32]
CB_WKER = 503         # [32, 32]
CB_WQES = 535         # [64, 64]
CB_WKES = 599         # [32, 16]
CB_RET = 615          # [64, 64]
CB_REF = 679          # [64, 64]
CB_BENT = 743         # [32, 1]
CB_CSE = 744          # [1, 32]
CB_CS1 = 776          # [1, 512]
CB_R1 = 1288          # [1, 512]
CB_OC = 1800          # [128, 1] ones column
CB_OR128 = 1801       # [1, 128] ones row
CB_ORT = 1929         # [1, 704] ones row
CB_UR = 2633          # [1, 260] V-aug ones-col selector
CB_OW = 2896          # [128, 8*33] per-chunk [W_ent_c | ones] stacked lhs
CBW = 3160

_RUNNERS = {}


# ---------------------------------------------------------------------------
# TileContext subclass: this walrus build accepts at most ONE sync-wait per
# instruction; split excess waits onto injected NoOps / extra drains.
# ---------------------------------------------------------------------------
def _make_tile_cls():
    from concourse import tile as _tile
    from concourse import mybir as _mybir
    from concourse.vector_clock import ScopedClock

    class TileContextSplit(_tile.TileContext):
        def _lower_ordered_insts(self, ordered):
            for bb_name in list(ordered.keys()):
                insts = ordered[bb_name]
                out = []
                n_new = 0
                for inst in insts:
                    si = getattr(inst, 'sync_info', None)
                    waits = list(si.on_wait) if (si is not None and si.on_wait) else []
                    if len(waits) > 1:
                        for w in waits[:-1]:
                            nop = _mybir.InstNoOp(name=f"{inst.name}-w{n_new}",
                                                  ins=[], outs=[])
                            nop.engine = inst.engine
                            nop.sync_info = _mybir.SyncInfo(on_wait=[w], on_update=[])
                            out.append(nop)
                            n_new += 1
                        si.on_wait = waits[-1:]
                    out.append(inst)
                ordered[bb_name] = out
            return super()._lower_ordered_insts(ordered)

        def _drain_and_barrier(self, tick_clock, wait_clock):
            nc = self.nc
            drain_inst = nc.sync.drain()
            wait_clock.add_sem_waits(
                drain_inst.ins, ScopedClock({None: tick_clock.global_clock}))
            waits = list(drain_inst.ins.sync_info.on_wait or [])
            if len(waits) > 1:
                drain_inst.ins.sync_info.on_wait = waits[:1]
                rest = waits[1:]
                while rest:
                    extra = nc.sync.drain()
                    extra.ins.sync_info = _mybir.SyncInfo(on_wait=rest[:1],
                                                          on_update=[])
                    rest = rest[1:]
            nc.all_engine_barrier()
            assert self.sems is not None
            popped = nc._tile_sem_poison_stack.pop()
            assert popped is self._sem_poison
            nc.clear_and_free_semaphores(list(self.sems.allocated().values()))
            nc.all_engine_barrier()

    return TileContextSplit


# ---------------------------------------------------------------------------
# Program emission
# ---------------------------------------------------------------------------
def _emit(use_mask, ln1_aff, ln2_aff, repeat=1):
    import concourse.bass as bass
    import concourse.mybir as mybir
    from contextlib import ExitStack

    f32 = mybir.dt.float32
    f32r = mybir.dt.float32r
    bf16 = mybir.dt.bfloat16
    fp8 = mybir.dt.float8e4
    i32 = mybir.dt.int32
    AF = mybir.ActivationFunctionType
    AL = mybir.AluOpType
    DR = mybir.MatmulPerfMode.DoubleRow
    X = mybir.AxisListType.X
    TileContextSplit = _make_tile_cls()

    nc = bass.Bass("TRN2", target_bir_lowering=False, num_devices=B)

    xT_d = nc.declare_dram_parameter("xT", [128, DC * T], f32r, isOutput=False)
    xN_d = nc.declare_dram_parameter("xN", [128, NCH * 1024], f32, isOutput=False)
    Wqkv_d = nc.declare_dram_parameter("Wqkv_g", [RULES, 3 * D * HD], bf16,
                                       isOutput=False)
    bcs_d = nc.declare_dram_parameter("bcs_g", [RULES, 576], f32r, isOutput=False)
    mlpw_d = nc.declare_dram_parameter("mlpw", [128, 40960], fp8, isOutput=False)
    cb_d = nc.declare_dram_parameter("cblob", [128, CBW], f32r, isOutput=False)
    wo_d = nc.declare_dram_parameter("Wo_aug", [HD + 1, D], bf16, isOutput=False)
    if use_mask:
        maskT_d = nc.declare_dram_parameter("maskT", [T, T], f32, isOutput=False)
    out_d = nc.declare_dram_parameter("out", [T, D], f32, isOutput=True)

    with ExitStack() as ctx:
        tc = ctx.enter_context(TileContextSplit(nc, pool_alloc_mode="queue"))
        P = ctx.enter_context(tc.tile_pool(name="main", bufs=1))

        def tile(shape, dt, tag):
            return P.tile(shape, dt, tag=tag, name=f"{tag}_u{nc.next_id()}")

        # ================= phase 0: DMAs + consts =================
        xTc = [tile([128, T], f32r, f"XT{c}") for c in range(DC)]
        for c in range(DC):
            nc.sync.dma_start(xTc[c][:], xT_d[:, c * T:(c + 1) * T])
        CB = tile([128, CBW], f32r, "CB")
        nc.sync.dma_start(CB[:], cb_d[:])
        IDF = CB[0:128, CB_IDENT:CB_IDENT + 128].bitcast(f32)
        MW = tile([128, 40960], fp8, "MW")
        WO = tile([HD + 1, D], bf16, "WO")
        XN = tile([128, NCH * 1024], f32, "XN")
        if use_mask:
            maskT = [tile([128, T], f32, f"MK{m}") for m in range(NCH)]
            for m, (t0, t1) in enumerate(CH):
                nc.sync.dma_start(maskT[m][0:t1 - t0, :], maskT_d[t0:t1, :])


        # const views from the blob (f32r) + small memsets
        OC = CB[0:128, CB_OC:CB_OC + 1]
        OR128 = CB[0:1, CB_OR128:CB_OR128 + 128]
        ORT = CB[0:1, CB_ORT:CB_ORT + T]
        UR = CB[0:1, CB_UR:CB_UR + KSLOT * 65]
        ones4 = CB[0:KSLOT, CB_OC:CB_OC + 1]
        OB = tile([128, 1], bf16, "OB")       # ones column bf16
        nc.gpsimd.memset(OB[:], 1.0)
        O8 = tile([128, 1], fp8, "O8")        # ones column fp8
        nc.gpsimd.memset(O8[:], 1.0)
        eps1 = tile([1, 1], f32, "EPS")
        nc.gpsimd.memset(eps1[:], 1e-5)

        # MLP weight views
        W1v = MW[:, 0:4096].rearrange("p (s i m) -> p s i m", s=4, i=2, m=512)
        W4v = MW[:, 4096:8192].rearrange("p (s i m) -> p s i m", s=2, i=2, m=1024)
        W2v = MW[:, 8192:24576].rearrange("p (c s i m) -> p c s i m",
                                          c=C2N, s=2, i=2, m=128)
        W3v = MW[:, 24576:40960].rearrange("p (j r i m) -> p j r i m",
                                           j=J1N, r=16, i=2, m=128)

        # ================= phase 1: LN1 stats + xe =================
        sqt = [tile([128, T], bf16, f"SQ{c % 3}") for c in range(DC)]
        MUR = tile([1, T], f32, "MUR")
        INVR = tile([1, T], f32r, "INVR")
        SIGR = tile([1, T], f32, "T3")
        VARR = tile([1, T], f32, "T2")
        MUSQ = tile([1, T], f32, "T1")
        MIR = tile([1, T], f32, "T1")
        NMS = tile([1, T], f32r, "NMS")
        XE = tile([SD, T], f32, "XE")
        XET = tile([128, NCH * SD], f32, "XET")
        IVT = tile([128, NCH], f32, "IVT")
        W3C = tile([128, 18], f32, "W3C")
        NEGC = tile([1, E], f32, "NEGC")

        with tc.tile_pool(name="ps_ln1", bufs=1, space="PSUM") as ps_ln1:
            ps_st = ps_ln1
            psq = [ps_ln1.tile([1, h1 - h0], f32, tag=f"pq{h}",
                               name=f"pq{h}_u{nc.next_id()}")
                   for h, (h0, h1) in enumerate(HALVES)]
            psxe = [ps_ln1.tile([SD + 1, h1 - h0], f32, tag=f"pe{h}",
                                name=f"pe{h}_u{nc.next_id()}")
                    for h, (h0, h1) in enumerate(HALVES)]
            for c in range(DC):
                ow_c = CB[0:128, CB_OW + 33 * c:CB_OW + 33 * (c + 1)]
                for h, (h0, h1) in enumerate(HALVES):
                    nc.tensor.matmul(psxe[h][:], ow_c,
                                     xTc[c][:, h0:h1],
                                     start=(c == 0), stop=(c == DC - 1))
                if c % 4 == 0:
                    nc.gpsimd.tensor_mul(sqt[c][:], xTc[c][:].bitcast(f32),
                                         xTc[c][:].bitcast(f32))
                elif c % 4 in (1, 2):
                    nc.vector.tensor_mul(sqt[c][:], xTc[c][:].bitcast(f32),
                                         xTc[c][:].bitcast(f32))
                else:
                    nc.scalar.activation(sqt[c][:], xTc[c][:].bitcast(f32),
                                         AF.Square)
            for c in range(DC):
                for h, (h0, h1) in enumerate(HALVES):
                    nc.tensor.matmul(psq[h][:], OB[:], sqt[c][:, h0:h1],
                                     start=(c == 0), stop=(c == DC - 1))
            # rows: mu, var, sig, inv, mu*inv
            for h, (h0, h1) in enumerate(HALVES):
                nc.vector.tensor_scalar_mul(MUR[0:1, h0:h1],
                                            psxe[h][SD:SD + 1, :], 1.0 / D)
                nc.vector.tensor_mul(MUSQ[0:1, h0:h1], MUR[0:1, h0:h1],
                                     MUR[0:1, h0:h1])
                nc.vector.scalar_tensor_tensor(VARR[0:1, h0:h1], psq[h][:],
                                               1.0 / D, MUSQ[0:1, h0:h1],
                                               AL.mult, AL.subtract)
                nc.scalar.activation(SIGR[0:1, h0:h1], VARR[0:1, h0:h1],
                                     AF.Sqrt, bias=eps1[:])
                with nc.allow_low_precision("f32r rounding is fine here"):
                    nc.vector.reciprocal(INVR[0:1, h0:h1], SIGR[0:1, h0:h1])
                nc.vector.tensor_mul(MIR[0:1, h0:h1], MUR[0:1, h0:h1],
                                     INVR[0:1, h0:h1].bitcast(f32))
            # xe -> SBUF, then per-chunk transposes
            for h, (h0, h1) in enumerate(HALVES):
                nc.vector.tensor_copy(XE[:, h0:h1], psxe[h][0:SD, :])
            # inv transposed to columns per t-chunk + seg weights
            for m, (t0, t1) in enumerate(CH):
                ts_ = t1 - t0
                pst = ps_st.tile([128, SD], f32, tag="pst",
                                 name=f"pst_u{nc.next_id()}")
                nc.tensor.transpose(pst[0:ts_, 0:SD], XE[:, t0:t1],
                                    IDF[0:SD, 0:SD])
                nc.vector.tensor_copy(XET[0:ts_, m * SD:(m + 1) * SD],
                                      pst[0:ts_, 0:SD])
                psi = ps_st.tile([128, 1], f32, tag="psi",
                                 name=f"psi_u{nc.next_id()}")
                nc.tensor.transpose(psi[0:ts_, 0:1],
                                    INVR[0:1, t0:t1].bitcast(f32),
                                    IDF[0:1, 0:1])
                nc.vector.tensor_copy(IVT[0:ts_, m:m + 1], psi[0:ts_, 0:1])
                nc.vector.tensor_scalar(
                    W3C[:, 3 * m:3 * (m + 1)],
                    CB[0:128, CB_SEGW + 3 * m:CB_SEGW + 3 * (m + 1)].bitcast(f32),
                    IVT[:, m:m + 1], None, AL.mult)
            # neg corr row [1, E]
            for e, (s0, s1) in enumerate(SEGS):
                nc.vector.reduce_sum(NEGC[0:1, e:e + 1], MIR[0:1, s0:s1], axis=X)
                nc.vector.tensor_scalar_mul(NEGC[0:1, e:e + 1],
                                            NEGC[0:1, e:e + 1],
                                            -1.0 / SEG_LENS[e])

        # ============ phase 2: entities + routing (fp32) ============
        entT = tile([SD, E], f32, "ENTT")
        GR = tile([1, T], f32r, "GR")
        SR = tile([1, T], f32r, "SR")
        SBC = tile([128, T], f32, "SBC")
        ridx_i = tile([1, KSLOT], i32, "RIDXI")
        with tc.tile_pool(name="ps_rt", bufs=2, space="PSUM") as ps_rt:
            def rpt(shape, tag="r"):
                return ps_rt.tile(shape, f32, tag=tag, name=f"rt_u{nc.next_id()}")

            ps_ent = rpt([SD, E])
            for m, (t0, t1) in enumerate(CH):
                ts_ = t1 - t0
                nc.tensor.matmul(ps_ent[:], XET[0:ts_, m * SD:(m + 1) * SD],
                                 W3C[0:ts_, 3 * m:3 * (m + 1)],
                                 start=(m == 0), stop=False)
            nc.tensor.matmul(ps_ent[:], CB[0:1, CB_CSE:CB_CSE + SD].bitcast(f32),
                             NEGC[0:1, :], start=False, stop=True)
            nc.vector.tensor_scalar(entT[:], ps_ent[:],
                                    CB[0:SD, CB_BENT:CB_BENT + 1].bitcast(f32),
                                    None, AL.add)

            cb_qer = CB[0:HD, CB_WQER:CB_WQER + SD]
            cb_ker = CB[0:SD, CB_WKER:CB_WKER + SD]
            cb_qes = CB[0:HD, CB_WQES:CB_WQES + 64]
            cb_kes = CB[0:SD, CB_WKES:CB_WKES + 16]
            cb_reT = CB[0:HD, CB_RET:CB_RET + 64]
            cb_ref = CB[0:64, CB_REF:CB_REF + HD]
            cb_E4 = CB[0:KSLOT, CB_E4:CB_E4 + 64]
            cb_E4T = CB[0:64, CB_E4T:CB_E4T + KSLOT]
            cb_iota = CB[0:64, CB_IOTA:CB_IOTA + 1]

            ps_qer = rpt([SD, 64])
            nc.tensor.matmul(ps_qer[:], cb_qer, cb_reT, start=True, stop=True)
            q_erT = tile([SD, 64], f32, "q_erT")
            nc.vector.tensor_copy(q_erT[:], ps_qer[:])

            ps_ker = rpt([SD, E])
            nc.tensor.matmul(ps_ker[:], cb_ker.bitcast(f32), entT[:],
                             start=True, stop=True)
            k_erT = tile([SD, E], f32, "k_erT")
            nc.vector.tensor_copy(k_erT[:], ps_ker[:])

            ps_ser = rpt([64, E])
            nc.tensor.matmul(ps_ser[:], q_erT[:], k_erT[:], start=True, stop=True)
            s_er = tile([64, E], f32, "s_er")
            nc.vector.tensor_scalar_mul(s_er[:], ps_ser[:],
                                        float(1.0 / np.sqrt(SD)))

            m1 = tile([64, 1], f32, "m1")
            nc.vector.reduce_max(m1[:], s_er[:], axis=X)
            ps_m1T = rpt([1, 64])
            nc.tensor.transpose(ps_m1T[:], m1[:], IDF[0:64, 0:64])
            m1T = tile([1, 64], f32, "m1T")
            nc.vector.tensor_copy(m1T[:], ps_m1T[:])
            mk = tile([1, KSLOT], f32, "mk")
            for k in range(KSLOT):
                nc.vector.reduce_max(mk[:, k:k + 1], m1T[:, k * 16:(k + 1) * 16],
                                     axis=X)
            ps_mkT = rpt([KSLOT, 1])
            nc.tensor.transpose(ps_mkT[:], mk[:], IDF[0:1, 0:1])
            mkT = tile([KSLOT, 1], f32, "mkT")
            nc.vector.tensor_copy(mkT[:], ps_mkT[:])
            ps_Mcol = rpt([64, 1])
            nc.tensor.matmul(ps_Mcol[:], cb_E4.bitcast(f32), mkT[:],
                             start=True, stop=True)
            Mcol = tile([64, 1], f32, "Mcol")
            nc.vector.tensor_copy(Mcol[:], ps_Mcol[:])

            onehot = tile([64, E], f32, "onehot")
            nc.vector.tensor_scalar(onehot[:], s_er[:], Mcol[:], None, AL.is_equal)
            rm = tile([64, 1], f32, "rm")
            nc.vector.reduce_sum(rm[:], onehot[:], axis=X)
            rm4 = tile([64, KSLOT], f32, "rm4")
            nc.vector.tensor_scalar(rm4[:], cb_E4T.bitcast(f32), rm[:], None, AL.mult)

            ps_sel = rpt([KSLOT, 64])
            nc.tensor.matmul(ps_sel[:], rm4[:], cb_ref.bitcast(f32),
                             start=True, stop=True)
            sel = tile([KSLOT, 64], f32, "sel")
            nc.vector.tensor_copy(sel[:], ps_sel[:])
            ps_selT = rpt([64, KSLOT])
            nc.tensor.transpose(ps_selT[:], sel[:], IDF[0:KSLOT, 0:KSLOT])
            selT = tile([64, KSLOT], f32, "selT")
            nc.vector.tensor_copy(selT[:], ps_selT[:])

            ps_qes = rpt([16, KSLOT])
            for k in range(KSLOT):
                nc.tensor.matmul(ps_qes[:, k:k + 1],
                                 cb_qes[:, k * 16:(k + 1) * 16].bitcast(f32),
                                 selT[:, k:k + 1], start=(k == 0),
                                 stop=(k == KSLOT - 1))
            q_esT = tile([16, KSLOT], f32, "q_esT")
            nc.vector.tensor_copy(q_esT[:], ps_qes[:])
            ps_kes = rpt([16, E])
            nc.tensor.matmul(ps_kes[:], cb_kes.bitcast(f32), entT[:],
                             start=True, stop=True)
            k_esT = tile([16, E], f32, "k_esT")
            nc.vector.tensor_copy(k_esT[:], ps_kes[:])
            ps_sesT = rpt([E, KSLOT])
            nc.tensor.matmul(ps_sesT[:], k_esT[:], q_esT[:], start=True, stop=True)
            s_esT = tile([E, KSLOT], f32, "s_esT")
            nc.vector.tensor_scalar_mul(s_esT[:], ps_sesT[:],
                                        float(1.0 / np.sqrt(SD // 2)))
            ps_ses = rpt([KSLOT, E])
            nc.tensor.transpose(ps_ses[:], s_esT[:], IDF[0:E, 0:E])
            s_es = tile([KSLOT, E], f32, "s_es")
            nc.vector.tensor_copy(s_es[:], ps_ses[:])
            em1 = tile([KSLOT, 1], f32, "em1")
            nc.vector.reduce_max(em1[:], s_es[:], axis=X)
            eoh = tile([KSLOT, E], f32, "eoh")
            nc.vector.tensor_scalar(eoh[:], s_es[:], em1[:], None, AL.is_equal)

            ps_crow = rpt([1, E])
            nc.tensor.matmul(ps_crow[:], ones4.bitcast(f32), eoh[:],
                             start=True, stop=True)
            c_row = tile([1, E], f32, "c_row")
            nc.vector.tensor_copy(c_row[:], ps_crow[:])

            ps_ridx = rpt([KSLOT, 1])
            nc.tensor.matmul(ps_ridx[:], rm4[:], cb_iota.bitcast(f32),
                             start=True, stop=True)
            ridx_col = tile([KSLOT, 1], f32, "ridx_col")
            nc.vector.tensor_copy(ridx_col[:], ps_ridx[:])
            ps_ridxT = rpt([1, KSLOT])
            nc.tensor.transpose(ps_ridxT[:], ridx_col[:], IDF[0:KSLOT, 0:KSLOT])
            ridx_f = tile([1, KSLOT], f32, "ridx_f")
            nc.vector.tensor_copy(ridx_f[:], ps_ridxT[:])
            nc.vector.tensor_copy(ridx_i[:], ridx_f[:])

            # gate-count row, s row = gate * inv, nms row = -mu * s
            for e, (s0, s1) in enumerate(SEGS):
                nc.vector.tensor_scalar(GR[0:1, s0:s1],
                                        ORT[0:1, s0:s1].bitcast(f32),
                                        c_row[0:1, e:e + 1], None, AL.mult)
            nc.vector.tensor_mul(SR[0:1, :], GR[0:1, :].bitcast(f32),
                                 INVR[0:1, :].bitcast(f32))
            nc.vector.scalar_tensor_tensor(NMS[0:1, :], MUR[0:1, :], -1.0,
                                           SR[0:1, :].bitcast(f32),
                                           AL.mult, AL.mult)

        # ============ phase 3: gather + s_bc + xs ============
        wqkv = [tile([128, 1536], bf16, f"G{k}") for k in range(KSLOT)]
        bcs = [tile([1, 576], f32r, f"BCS{k}") for k in range(KSLOT)]
        for k in range(KSLOT):
            reg = nc.gpsimd.alloc_register(f"ridx{k}")
            nc.gpsimd.reg_load(reg, ridx_i[0:1, k:k + 1])
            off = nc.gpsimd.snap(reg, donate=True, min_val=0, max_val=RULES - 1)
            src_w = Wqkv_d[bass.ds(off, 1), :].rearrange(
                "a (p f) -> (a p) f", p=128, f=1536)
            nc.gpsimd.dma_start(wqkv[k][:], src_w)
            nc.gpsimd.dma_start(bcs[k][:], bcs_d[bass.ds(off, 1), :])
        # Defer the big XN/WO/MW loads until the routing-gated gather lands:
        # DMA-engine grants are FIFO by request time, so an early request
        # starves the gather. A tiny gated read of each target makes the DMA
        # wait via WAR.
        gate = tile([1, 2], f32, "GATE")
        wq3v = wqkv[3][0:1, 0:4].bitcast(f32)
        nc.gpsimd.memset(XN[0:1, 0:2], 0.0)
        nc.gpsimd.memset(WO[0:1, 0:4], 0.0)
        nc.gpsimd.memset(MW[0:1, 0:8], 0.0)
        nc.vector.scalar_tensor_tensor(gate[:], XN[0:1, 0:2], 0.0, wq3v,
                                       AL.mult, AL.add)
        nc.sync.dma_start(XN[:], xN_d[:])
        nc.vector.scalar_tensor_tensor(gate[:], WO[0:1, 0:4].bitcast(f32), 0.0,
                                       wq3v, AL.mult, AL.add)
        nc.sync.dma_start(WO[:], wo_d[:])
        nc.vector.scalar_tensor_tensor(gate[:], MW[0:1, 0:8].bitcast(f32), 0.0,
                                       wq3v, AL.mult, AL.add)
        for q in range(4):
            nc.sync.dma_start(MW[:, q * 10240:(q + 1) * 10240],
                              mlpw_d[:, q * 10240:(q + 1) * 10240])

        with tc.tile_pool(name="ps_bc", bufs=1, space="PSUM") as ps_bc:
            for h, (h0, h1) in enumerate(HALVES):
                psb = ps_bc.tile([128, h1 - h0], f32, tag=f"sb{h}",
                                 name=f"sb{h}_u{nc.next_id()}")
                nc.tensor.matmul(psb[:], OR128, SR[0:1, h0:h1],
                                 start=True, stop=True)
                nc.vector.tensor_copy(SBC[:, h0:h1], psb[:])

        xs = [tile([128, T], bf16, f"XS{c}") for c in range(DC)]
        for c in range(DC):
            nc.vector.tensor_mul(xs[c][:], xTc[c][:].bitcast(f32), SBC[:])

        # wv_all [128, c=8, k*65+j] from gathered v-parts; csv row; bvsum col
        WVA = tile([128, DC * KSLOT * 65], bf16, "WVA")
        nc.gpsimd.memset(WVA[:], 0.0)
        for k in range(KSLOT):
            nc.vector.tensor_copy(
                WVA.rearrange("p (c k u) -> p c k u", c=DC, k=KSLOT, u=65)
                [:, :, k, 0:64],
                wqkv[k].rearrange("p (c three j) -> p c three j",
                                  c=DC, three=3, j=HD)[:, :, 2, :])
        CSV = tile([1, KSLOT * 65], f32r, "CSV")
        nc.gpsimd.memset(CSV[:].bitcast(f32), 0.0)
        for k in range(KSLOT):
            nc.vector.tensor_copy(CSV[0:1, k * 65:k * 65 + 64],
                                  bcs[k][0:1, 320:384].bitcast(f32))
        if ln1_aff:
            CBV = tile([1, KSLOT * 65], f32r, "CBV")
            nc.gpsimd.memset(CBV[:].bitcast(f32), 0.0)
            for k in range(KSLOT):
                nc.vector.tensor_copy(CBV[0:1, k * 65:k * 65 + 64],
                                      bcs[k][0:1, 512:576].bitcast(f32))
        bv01 = tile([1, HD], f32, "BV01")
        nc.vector.tensor_add(bv01[:], bcs[0][0:1, 128:192].bitcast(f32),
                             bcs[1][0:1, 128:192].bitcast(f32))
        bv23 = tile([1, HD], f32, "BV23")
        nc.vector.tensor_add(bv23[:], bcs[2][0:1, 128:192].bitcast(f32),
                             bcs[3][0:1, 128:192].bitcast(f32))
        bvr = tile([1, HD], f32, "BVR")
        nc.vector.tensor_add(bvr[:], bv01[:], bv23[:])
        BVS = tile([HD, 1], f32, "BVS")
        with tc.tile_pool(name="ps_bv", bufs=1, space="PSUM") as ps_bv:
            psv = ps_bv.tile([HD, 1], f32, tag="bv", name=f"bv_u{nc.next_id()}")
            nc.tensor.transpose(psv[:], bvr[:], IDF[0:1, 0:1])
            nc.vector.tensor_copy(BVS[:], psv[:])

        # ============ phase 4: QKV + V ============
        qT = [tile([HD, T], bf16, f"QT{k}") for k in range(KSLOT)]
        kT = [tile([HD, T], bf16, f"KT{k}") for k in range(KSLOT)]
        V_aug = [tile([128, KSLOT * 65], bf16, f"VA{m}") for m in range(NCH)]
        with tc.tile_pool(name="ps_qkv", bufs=2, space="PSUM") as ps_qkv, \
             tc.tile_pool(name="ps_v", bufs=2, space="PSUM") as ps_v:
            for k in range(KSLOT):
                for h, (h0, h1) in enumerate(HALVES):
                    ps = ps_qkv.tile([128, h1 - h0], f32, tag="qk",
                                     name=f"psqk_u{nc.next_id()}")
                    for c in range(DC):
                        lhs_qk = wqkv[k].rearrange(
                            "p (c three j) -> p c three j",
                            c=DC, three=3, j=HD)[:, c, 0:2, :]
                        nc.tensor.matmul(ps[:], lhs_qk, xs[c][:, h0:h1],
                                         start=(c == 0), stop=False)
                    nc.tensor.matmul(ps[:], bcs[k][0:1, 0:128],
                                     ORT[0:1, h0:h1],
                                     start=False, stop=False)
                    nc.tensor.matmul(ps[:], bcs[k][0:1, 192:320],
                                     NMS[0:1, h0:h1],
                                     start=False, stop=(not ln1_aff))
                    if ln1_aff:
                        nc.tensor.matmul(ps[:], bcs[k][0:1, 384:512],
                                         GR[0:1, h0:h1],
                                         start=False, stop=True)
                    nc.vector.tensor_copy(qT[k][:, h0:h1], ps[0:HD, :])
                    nc.vector.tensor_copy(kT[k][:, h0:h1], ps[HD:128, :])
            for m, (t0, t1) in enumerate(CH):
                ts_ = t1 - t0
                ps = ps_v.tile([128, KSLOT * 65], f32, tag="v",
                               name=f"psv_u{nc.next_id()}")
                for c in range(DC):
                    nc.tensor.matmul(ps[0:ts_, :], xs[c][:, t0:t1],
                                     WVA.rearrange("p (c u) -> p c u",
                                                   c=DC, u=KSLOT * 65)[:, c, :],
                                     start=(c == 0), stop=False)
                nc.tensor.matmul(ps[0:ts_, :], NMS[0:1, t0:t1],
                                 CSV[:], start=False, stop=False)
                nc.tensor.matmul(ps[0:ts_, :], ORT[0:1, t0:t1],
                                 UR, start=False,
                                 stop=(not ln1_aff))
                if ln1_aff:
                    nc.tensor.matmul(ps[0:ts_, :], GR[0:1, t0:t1],
                                     CBV[:], start=False, stop=True)
                nc.scalar.copy(V_aug[m][0:ts_, :], ps[0:ts_, :])

        # ============ phase 5: attention ============
        exp_tags = ["G0", "G1", "G2", "G3", "WVA", "XE"]
        expT = [tile([128, T], bf16, exp_tags[m]) for m in range(NCH)]
        aoT = tile([HD + 1, T], bf16, "AOT")
        nc.gpsimd.memset(aoT[HD:HD + 1, :], 4.0)
        with tc.tile_pool(name="ps_sc", bufs=2, space="PSUM") as ps_sc, \
             tc.tile_pool(name="ps_av", bufs=2, space="PSUM") as ps_av, \
             tc.tile_pool(name="ps_rb", bufs=2, space="PSUM") as ps_rb:
            for h, (h0, h1) in enumerate(HALVES):
                n = h1 - h0
                tmps = []
                for k in range(KSLOT):
                    for m, (t0, t1) in enumerate(CH):
                        ts_ = t1 - t0
                        ps = ps_sc.tile([128, n], f32, tag="sc",
                                        name=f"sc_u{nc.next_id()}")
                        if use_mask:
                            nc.vector.tensor_scalar(ps[0:ts_, :],
                                                    maskT[m][0:ts_, h0:h1],
                                                    8.0, None, AL.mult)
                            first = False
                        else:
                            first = True
                        nc.tensor.matmul(ps[0:ts_, :], kT[k][:, t0:t1],
                                         qT[k][:, h0:h1], start=first, stop=True)
                        nc.scalar.activation(expT[m][0:ts_, h0:h1], ps[0:ts_, :],
                                             AF.Exp, scale=0.125)
                    av = ps_av.tile([HD + 1, n], f32, tag="av",
                                    name=f"av_u{nc.next_id()}")
                    for m, (t0, t1) in enumerate(CH):
                        ts_ = t1 - t0
                        nc.tensor.matmul(av[:], V_aug[m][0:ts_, k * 65:(k + 1) * 65],
                                         expT[m][0:ts_, h0:h1],
                                         start=(m == 0), stop=(m == NCH - 1))
                    rec = tile([1, n], f32r, f"REC{k % 2}")
                    with nc.allow_low_precision("softmax renorm"):
                        nc.vector.reciprocal(rec[:], av[HD:HD + 1, :])
                    rbp = ps_rb.tile([HD, n], f32, tag="rb",
                                     name=f"rb_u{nc.next_id()}")
                    nc.tensor.matmul(rbp[:], OR128[0:1, 0:HD],
                                     rec[:], start=True, stop=True)
                    rb = tile([HD, n], f32, f"RB{k % 2}")
                    nc.scalar.copy(rb[:], rbp[:])
                    tmp = tile([HD, n], f32, f"TMP{k}")
                    nc.vector.tensor_mul(tmp[:], av[0:HD, :], rb[:])
                    tmps.append(tmp)
                nc.gpsimd.tensor_add(tmps[0][:], tmps[0][:], tmps[1][:])
                nc.gpsimd.tensor_add(tmps[2][:], tmps[2][:], tmps[3][:])
                nc.vector.tensor_add(tmps[0][:], tmps[0][:], tmps[2][:])
                nc.vector.tensor_scalar(aoT[0:HD, h0:h1], tmps[0][:], BVS[:],
                                        None, AL.add)

        # ============ phase 6: Wo + residuals (h2q fp8, h2N f32) ============
        h2q = [tile([128, 2, T], fp8, f"KT{p}") for p in range(4)]
        with tc.tile_pool(name="ps_wo", bufs=2, space="PSUM") as ps_wo:
            for c in range(DC):
                for h, (h0, h1) in enumerate(HALVES):
                    ps = ps_wo.tile([128, h1 - h0], f32, tag="woT",
                                    name=f"woT_u{nc.next_id()}")
                    nc.tensor.matmul(ps[:], WO[:, c * 128:(c + 1) * 128],
                                     aoT[:, h0:h1], start=True, stop=True)
                    nc.vector.tensor_add(h2q[c // 2][:, c % 2, h0:h1],
                                         xTc[c][:, h0:h1].bitcast(f32), ps[:])
        # ============ phase 7: LN2 stats on h2q ============
        MU2 = tile([1, T], f32, "MUR")
        NM2 = tile([1, T], f32r, "NMS")
        SG2 = tile([1, T], f32, "T3")
        VA2 = tile([1, T], f32, "T2")
        MQ2 = tile([1, T], f32, "T1")
        IV2 = tile([1, T], f32r, "INVR")
        sq2 = [tile([128, T], bf16, f"SQ{i % 3}") for i in range(DC)]
        with tc.tile_pool(name="ps_ln2", bufs=1, space="PSUM") as ps_ln2:
            px2 = [ps_ln2.tile([1, h1 - h0], f32, tag=f"p2x{h}",
                               name=f"p2x{h}_u{nc.next_id()}")
                   for h, (h0, h1) in enumerate(HALVES)]
            pq2 = [ps_ln2.tile([1, h1 - h0], f32, tag=f"p2q{h}",
                               name=f"p2q{h}_u{nc.next_id()}")
                   for h, (h0, h1) in enumerate(HALVES)]
            for c in range(DC):
                src = h2q[c // 2][:, c % 2, :]
                for h, (h0, h1) in enumerate(HALVES):
                    nc.tensor.matmul(px2[h][:], O8[:], src[:, h0:h1],
                                     start=(c == 0), stop=(c == DC - 1))
                nc.scalar.activation(sq2[c][:], src, AF.Square)
            for c in range(DC):
                for h, (h0, h1) in enumerate(HALVES):
                    nc.tensor.matmul(pq2[h][:], OB[:], sq2[c][:, h0:h1],
                                     start=(c == 0), stop=(c == DC - 1))
            for h, (h0, h1) in enumerate(HALVES):
                nc.vector.tensor_scalar_mul(MU2[0:1, h0:h1], px2[h][:], 1.0 / D)
                nc.vector.tensor_mul(MQ2[0:1, h0:h1], MU2[0:1, h0:h1],
                                     MU2[0:1, h0:h1])
                nc.vector.scalar_tensor_tensor(VA2[0:1, h0:h1], pq2[h][:],
                                               1.0 / D, MQ2[0:1, h0:h1],
                                               AL.mult, AL.subtract)
                nc.scalar.activation(SG2[0:1, h0:h1], VA2[0:1, h0:h1],
                                     AF.Sqrt, bias=eps1[:])
                with nc.allow_low_precision("f32r rounding is fine here"):
                    nc.vector.reciprocal(IV2[0:1, h0:h1], SG2[0:1, h0:h1])
                nc.vector.tensor_scalar_mul(NM2[0:1, h0:h1], MU2[0:1, h0:h1],
                                            -1.0)
        # ============ phase 8: MLP (fp8 DoubleRow) ============
        y1q = [tile([128, 2, T], fp8, f"SQ{p}") for p in range(2)]
        y2_tags = ([f"XS{c}" for c in range(DC)] + [f"G{k}" for k in range(KSLOT)]
                   + ["WVA", "XE", "SBC", "Y2F"])
        if use_mask:
            y2_tags = ([f"MK{m}" for m in range(NCH)]
                       + [f"XS{c}" for c in range(DC)] + ["G0", "G1"])
        y2q = [tile([128, 2, T], fp8, y2_tags[p]) for p in range(16)]
        y3q = [tile([128, 2, T], fp8, f"QT{p}") for p in range(2)]

        with tc.tile_pool(name="ps_i2", bufs=1, space="PSUM") as ps_i2, \
             tc.tile_pool(name="ps_y1", bufs=2, space="PSUM") as ps_y1:
            i2bc = []
            for h, (h0, h1) in enumerate(HALVES):
                pi = ps_i2.tile([128, h1 - h0], f32, tag=f"i2{h}",
                                name=f"i2{h}_u{nc.next_id()}")
                nc.tensor.matmul(pi[:], OR128, IV2[0:1, h0:h1],
                                 start=True, stop=True)
                i2bc.append(pi)
            for j in range(J1N):
                for h, (h0, h1) in enumerate(HALVES):
                    ps = ps_y1.tile([128, h1 - h0], f32, tag="y1",
                                    name=f"y1_u{nc.next_id()}")
                    for s in range(4):
                        nc.tensor.matmul(ps[:], W1v[:, s, :, j * 128:(j + 1) * 128],
                                         h2q[s][:, :, h0:h1], perf_mode=DR,
                                         start=(s == 0), stop=False)
                    nc.tensor.matmul(
                        ps[:], CB[0:1, CB_CS1 + j * 128:CB_CS1 + (j + 1) * 128],
                        NM2[0:1, h0:h1], start=False, stop=(not ln2_aff))
                    if ln2_aff:
                        nc.tensor.matmul(
                            ps[:], CB[0:1, CB_R1 + j * 128:CB_R1 + (j + 1) * 128],
                            SG2[0:1, h0:h1].bitcast(f32r),
                            start=False, stop=True)
                    nc.scalar.activation(y1q[j // 2][:, j % 2, h0:h1], ps[:],
                                         AF.Relu)
            for p in range(2):
                for i in range(2):
                    for h, (h0, h1) in enumerate(HALVES):
                        nc.vector.tensor_mul(y1q[p][:, i, h0:h1],
                                             y1q[p][:, i, h0:h1], i2bc[h][:])

        with tc.tile_pool(name="ps_y2", bufs=3, space="PSUM") as ps_y2, \
             tc.tile_pool(name="ps_y3", bufs=2, space="PSUM") as ps_y3, \
             tc.tile_pool(name="ps_y4", bufs=2, space="PSUM") as ps_y4:
            for m, (t0, t1) in enumerate(CH):
                ts_ = t1 - t0
                for dh in range(2):
                    d0 = dh * 512
                    ps = ps_y4.tile([128, 512], f32, tag="y4",
                                    name=f"woN_u{nc.next_id()}")
                    nc.tensor.matmul(ps[0:ts_, :], aoT[:, t0:t1],
                                     WO[:, d0:d0 + 512], start=True, stop=True)
                    nc.vector.tensor_add(
                        XN[0:ts_, m * 1024 + d0:m * 1024 + d0 + 512],
                        XN[0:ts_, m * 1024 + d0:m * 1024 + d0 + 512],
                        ps[0:ts_, :])
            for c2 in range(C2N):
                for h, (h0, h1) in enumerate(HALVES):
                    ps = ps_y2.tile([128, h1 - h0], f32, tag="y2",
                                    name=f"y2_u{nc.next_id()}")
                    for s in range(2):
                        nc.tensor.matmul(ps[:], W2v[:, c2, s],
                                         y1q[s][:, :, h0:h1], perf_mode=DR,
                                         start=(s == 0), stop=(s == 1))
                    nc.scalar.activation(y2q[c2 // 2][:, c2 % 2, h0:h1], ps[:],
                                         AF.Gelu)
            for p0, p1, mlist in [(0, 384, [0, 1, 2]), (384, 704, [3, 4, 5])]:
                pn = p1 - p0
                for j in range(J1N):
                    ps = ps_y3.tile([128, 384], f32, tag="y3",
                                    name=f"y3_u{nc.next_id()}")
                    for r in range(16):
                        nc.tensor.matmul(ps[:, 0:pn], W3v[:, j, r],
                                         y2q[r][:, :, p0:p1], perf_mode=DR,
                                         start=(r == 0), stop=(r == 15))
                    nc.scalar.activation(y3q[j // 2][:, j % 2, p0:p1],
                                         ps[:, 0:pn], AF.Relu)
                for m in mlist:
                    t0, t1 = CH[m]
                    ts_ = t1 - t0
                    for dh in range(2):
                        d0 = dh * 512
                        ps = ps_y4.tile([128, 512], f32, tag="y4",
                                        name=f"y4_u{nc.next_id()}")
                        for p in range(2):
                            nc.tensor.matmul(ps[0:ts_, :], y3q[p][:, :, t0:t1],
                                             W4v[:, p, :, d0:d0 + 512],
                                             perf_mode=DR,
                                             start=(p == 0), stop=(p == 1))
                        nc.vector.tensor_add(
                            XN[0:ts_, m * 1024 + d0:m * 1024 + d0 + 512],
                            XN[0:ts_, m * 1024 + d0:m * 1024 + d0 + 512],
                            ps[0:ts_, :])
                    nc.sync.dma_start(out_d[t0:t1, :],
                                      XN[0:ts_, m * 1024:(m + 1) * 1024])

    return nc


# ---------------------------------------------------------------------------
# Host-side input prep
# ---------------------------------------------------------------------------
def _host_consts(inputs, ln1_aff, ln2_aff):
    """Batch-independent tensors (weights), computed once."""
    import ml_dtypes
    f = np.float32
    e4 = ml_dtypes.float8_e4m3
    d = {}

    g1 = np.asarray(inputs["ln1_g"], f) if ln1_aff else None
    g2 = np.asarray(inputs["ln2_g"], f) if ln2_aff else None
    b1 = np.asarray(inputs["ln1_b"], f) if ln1_aff else None
    b2 = np.asarray(inputs["ln2_b"], f) if ln2_aff else None

    # gathered per-rule QKV weights (g1 folded in if affine)
    blks = []
    for Wn in ("Wq", "Wk", "Wv"):
        W = np.asarray(inputs[Wn], f)
        if ln1_aff:
            W = W * g1[:, None]
        blks.append(W.reshape(DC, 128, RULES, HD).transpose(2, 0, 1, 3))
    d["Wqkv_g"] = np.ascontiguousarray(
        np.stack(blks, axis=2).transpose(0, 3, 1, 2, 4)
        .reshape(RULES, 3 * D * HD)).astype(ml_dtypes.bfloat16)

    # bcs row per rule: [bq bk bv csq csk csv cbq cbk cbv] (9*64 = 576)
    bias = np.concatenate(
        [np.asarray(inputs[bn], f).reshape(RULES, HD) for bn in ("bq", "bk", "bv")],
        axis=1)                                             # [R, 192]
    # blks[i] is [R, DC, 128, HD]; column sums over d per rule head
    csums = np.concatenate(
        [blk.reshape(RULES, D, HD).sum(1) for blk in blks], axis=1)  # [R, 192]
    if ln1_aff:
        cb = np.concatenate(
            [np.einsum('d,drh->rh', b1,
                       (np.asarray(inputs[Wn], f) * g1[:, None])
                       .reshape(D, RULES, HD))
             for Wn in ("Wq", "Wk", "Wv")], axis=1)
    else:
        cb = np.zeros((RULES, 192), f)
    d["bcs_g"] = np.ascontiguousarray(np.concatenate([bias, csums, cb], axis=1))

    # Wo_aug bf16
    d["Wo_aug"] = np.ascontiguousarray(np.concatenate(
        [np.asarray(inputs["Wo"], f), np.asarray(inputs["bo"], f)[None, :]],
        0)).astype(ml_dtypes.bfloat16)

    # MLP weights fp8, DoubleRow layouts
    W1 = np.asarray(inputs["fc1_w1"], f)
    if ln2_aff:
        W1 = W1 * g2[:, None]
    W1q = W1.astype(e4)
    W2q = np.asarray(inputs["fc1_w2"], f).astype(e4)
    W3q = np.asarray(inputs["fc2_w1"], f).astype(e4)
    W4q = np.asarray(inputs["fc2_w2"], f).astype(e4)
    # W1 [1024, 512] -> [p, s, i, m]
    w1 = W1q.reshape(4, 2, 128, FD1).transpose(2, 0, 1, 3).reshape(128, 4096)
    # W4 [512, 1024] -> [p, s, i, d]
    w4 = W4q.reshape(2, 2, 128, D).transpose(2, 0, 1, 3).reshape(128, 4096)
    # W2 [512, 4096] -> [p, c2, s, i, m]
    w2 = (W2q.reshape(2, 2, 128, C2N, 128).transpose(2, 3, 0, 1, 4)
          .reshape(128, 16384))
    # W3 [4096, 512] -> [p, j, r, i, m]
    w3 = (W3q.reshape(16, 2, 128, J1N, 128).transpose(2, 3, 0, 1, 4)
          .reshape(128, 16384))
    d["mlpw"] = np.ascontiguousarray(np.concatenate([w1, w4, w2, w3], axis=1))

    # consts blob
    cb_arr = np.zeros((128, CBW), f)
    cb_arr[:, CB_IDENT:CB_IDENT + 128] = np.eye(128, dtype=f)
    went = np.asarray(inputs["W_ent"], f)
    if ln1_aff:
        went = went * g1[:, None]
    cb_arr[:, CB_WENT:CB_WENT + DC * SD] = (
        went.reshape(DC, 128, SD).transpose(1, 0, 2).reshape(128, DC * SD))
    went_c = went.reshape(DC, 128, SD)
    for c in range(DC):
        cb_arr[:, CB_OW + 33 * c:CB_OW + 33 * c + SD] = went_c[c]
        cb_arr[:, CB_OW + 33 * c + SD] = 1.0
    segw = np.zeros((128, NCH * E), f)
    for m, (t0, t1) in enumerate(CH):
        for p in range(t1 - t0):
            t = t0 + p
            for e, (s0, s1) in enumerate(SEGS):
                if s0 <= t < s1:
                    segw[p, m * E + e] = 1.0 / SEG_LENS[e]
    cb_arr[:, CB_SEGW:CB_SEGW + NCH * E] = segw
    E4 = np.kron(np.eye(KSLOT, dtype=f), np.ones((1, RULES), f))
    cb_arr[0:KSLOT, CB_E4:CB_E4 + 64] = E4
    cb_arr[0:64, CB_E4T:CB_E4T + KSLOT] = E4.T
    cb_arr[0:64, CB_IOTA] = (np.arange(64) % RULES).astype(f)
    cb_arr[0:HD, CB_WQER:CB_WQER + SD] = np.asarray(inputs["Wq_er"], f)
    cb_arr[0:SD, CB_WKER:CB_WKER + SD] = np.asarray(inputs["Wk_er"], f)
    Wqes = np.asarray(inputs["Wq_es"], f)          # [K, HD, SD//2]
    cb_arr[0:HD, CB_WQES:CB_WQES + 64] = (
        Wqes.transpose(1, 0, 2).reshape(HD, KSLOT * (SD // 2)))
    cb_arr[0:SD, CB_WKES:CB_WKES + 16] = np.asarray(inputs["Wk_es"], f)
    re = np.asarray(inputs["rules_embed"], f)      # [K, R, HD]
    cb_arr[0:HD, CB_RET:CB_RET + 64] = re.transpose(2, 0, 1).reshape(HD, 64)
    cb_arr[0:64, CB_REF:CB_REF + HD] = re.reshape(64, HD)
    bent = np.asarray(inputs["b_ent"], f)
    if ln1_aff:
        bent = bent + b1 @ went
    cb_arr[0:SD, CB_BENT] = bent
    cb_arr[0, CB_CSE:CB_CSE + SD] = went.sum(0)
    cb_arr[0, CB_CS1:CB_CS1 + FD1] = W1q.astype(f).sum(0)
    if ln2_aff:
        cb_arr[0, CB_R1:CB_R1 + FD1] = b2 @ W1
    cb_arr[:, CB_OC] = 1.0
    cb_arr[0, CB_OR128:CB_OR128 + 128] = 1.0
    cb_arr[0, CB_ORT:CB_ORT + T] = 1.0
    ur = np.zeros(KSLOT * 65, f)
    ur[64::65] = 1.0
    cb_arr[0, CB_UR:CB_UR + KSLOT * 65] = ur
    d["cblob"] = np.ascontiguousarray(cb_arr)
    return d


def _prep_core_inputs(inputs, b, use_mask, ln1_aff, ln2_aff, consts=None):
    f = np.float32
    if consts is None:
        consts = _host_consts(inputs, ln1_aff, ln2_aff)
    d = dict(consts)
    hs = np.asarray(inputs["hidden_states"], f)
    x = hs[b]                                      # [T, D]
    xT = np.ascontiguousarray(x.T)                 # [D, T]
    d["xT"] = np.ascontiguousarray(
        xT.reshape(DC, 128, T).transpose(1, 0, 2).reshape(128, DC * T))
    xn = np.zeros((128, NCH * 1024), f)
    for m, (t0, t1) in enumerate(CH):
        xn[0:t1 - t0, m * 1024:(m + 1) * 1024] = x[t0:t1]
    d["xN"] = xn
    if use_mask:
        d["maskT"] = np.ascontiguousarray(
            np.asarray(inputs["attention_mask"], f)[b].T)
    return d


# ---------------------------------------------------------------------------
# Runner (jax/axon shard_map over 8 cores)
# ---------------------------------------------------------------------------
def _build_runner(use_mask, ln1_aff, ln2_aff, repeat=1):
    key = (use_mask, ln1_aff, ln2_aff, repeat)
    if key in _RUNNERS:
        return _RUNNERS[key]
    import jax
    from jax.sharding import Mesh, PartitionSpec
    from jax.experimental.shard_map import shard_map
    from concourse import mybir
    from concourse.bass2jax import (_bass_exec_p, install_neuronx_cc_hook,
                                    partition_id_tensor)

    nc = _emit(use_mask, ln1_aff, ln2_aff, repeat)
    install_neuronx_cc_hook()
    partition_name = nc.partition_id_tensor.name if nc.partition_id_tensor else None
    in_names, out_names, out_avals, zero_shapes = [], [], [], []
    for alloc in nc.m.functions[0].allocations:
        if not isinstance(alloc, mybir.MemoryLocationSet):
            continue
        name = alloc.memorylocations[0].name
        if alloc.kind == "ExternalInput":
            if name != partition_name:
                in_names.append(name)
        elif alloc.kind == "ExternalOutput":
            out_names.append(name)
            shape = tuple(alloc.tensor_shape)
            dtype = mybir.dt.np(alloc.dtype)
            out_avals.append(jax.core.ShapedArray(shape, dtype))
            zero_shapes.append((shape, dtype))
    n_params = len(in_names)
    n_outs = len(out_avals)
    all_in_names = list(in_names) + list(out_names)
    if partition_name is not None:
        all_in_names.append(partition_name)

    def _body(*args):
        operands = list(args)
        if partition_name is not None:
            operands.append(partition_id_tensor())
        outs = _bass_exec_p.bind(
            *operands, out_avals=tuple(out_avals), in_names=tuple(all_in_names),
            out_names=tuple(out_names), lowering_input_output_aliases=(),
            sim_require_finite=False, sim_require_nnan=False, nc=nc)
        return tuple(outs)

    devices = jax.devices()[:B]
    mesh = Mesh(np.asarray(devices), ("core",))
    in_specs = (PartitionSpec("core"),) * (n_params + n_outs)
    out_specs = (PartitionSpec("core"),) * n_outs
    sharded = jax.jit(
        shard_map(_body, mesh=mesh, in_specs=in_specs, out_specs=out_specs,
                  check_rep=False),
        keep_unused=True)

    def run(per_core_maps):
        concat_in = [
            np.concatenate([np.asarray(per_core_maps[c][nm]) for c in range(B)], 0)
            for nm in in_names]
        concat_zeros = [np.zeros((B * s[0], *s[1:]), dt) for s, dt in zero_shapes]
        out_arrs = jax.block_until_ready(sharded(*concat_in, *concat_zeros))
        return [
            {nm: np.asarray(out_arrs[i]).reshape(B, *out_avals[i].shape)[c]
             for i, nm in enumerate(out_names)}
            for c in range(B)]

    _RUNNERS[key] = (run, sharded, in_names, zero_shapes, out_names, out_avals)
    return _RUNNERS[key]


def kernel(**inputs):
    use_mask = bool(np.any(np.asarray(inputs["attention_mask"])))
    ln1_aff = not (np.all(np.asarray(inputs["ln1_g"]) == 1.0)
                   and np.all(np.asarray(inputs["ln1_b"]) == 0.0))
    ln2_aff = not (np.all(np.asarray(inputs["ln2_g"]) == 1.0)
                   and np.all(np.asarray(inputs["ln2_b"]) == 0.0))
    run = _build_runner(use_mask, ln1_aff, ln2_aff)[0]
    consts = _host_consts(inputs, ln1_aff, ln2_aff)
    maps = [_prep_core_inputs(inputs, b, use_mask, ln1_aff, ln2_aff, consts)
            for b in range(B)]
    res = run(maps)
    out = np.stack([res[b]["out"] for b in range(B)]).astype(np.float32)
    return out



# revision 72
# speedup vs baseline: 1.0972x; 1.0418x over previous
# Trainium2 Bass kernel for nn_NeuralPromptProducerLayer (moe_routing).
# v2: fp8-e4m3 DoubleRow MLP with resident weights, LayerNorms folded into
# matmuls via rank-1 corrections (normalized activations never materialized),
# fused softmax denominator via an augmented-V ones column, batched DMAs.
import sys
sys.path.insert(0, '/opt/trn_rl_repo')

import numpy as np

B, T, D = 8, 704, 1024
RULES, KSLOT, HD = 16, 4, 64
SD, E = 32, 3
DC = D // 128                        # 8 d-chunks
CH = [(0, 128), (128, 256), (256, 384), (384, 512), (512, 640), (640, 704)]
NCH = len(CH)
HALVES = [(0, 352), (352, 704)]
SEGS = [(0, 128), (128, 192), (192, 704)]
SEG_LENS = [128.0, 64.0, 512.0]
FD1 = 512
FD2 = 4096
C2N = FD2 // 128                     # 32
J1N = FD1 // 128                     # 4

# consts-blob column layout (f32, [128 partitions, CBW])
CB_IDENT = 0          # [128, 128] identity
CB_WENT = 128         # [128, 8*32]  W_ent chunks
CB_SEGW = 384         # [128, 18]    seg/len weights per (m, e)
CB_E4 = 402           # [4, 64]
CB_E4T = 466          # [64, 4]
CB_IOTA = 470         # [64, 1]
CB_WQER = 471         # [64, 32]
CB_WKER = 503         # [32, 32]
CB_WQES = 535         # [64, 64]
CB_WKES = 599         # [32, 16]
CB_RET = 615          # [64, 64]
CB_REF = 679          # [64, 64]
CB_BENT = 743         # [32, 1]
CB_CSE = 744          # [1, 32]
CB_CS1 = 776          # [1, 512]
CB_R1 = 1288          # [1, 512]
CB_OC = 1800          # [128, 1] ones column
CB_OR128 = 1801       # [1, 128] ones row
CB_ORT = 1929         # [1, 704] ones row
CB_UR = 2633          # [1, 260] V-aug ones-col selector
CB_OW = 2896          # [128, 8*33] per-chunk [W_ent_c | ones] stacked lhs
CBW = 3160

_RUNNERS = {}


# ---------------------------------------------------------------------------
# TileContext subclass: this walrus build accepts at most ONE sync-wait per
# instruction; split excess waits onto injected NoOps / extra drains.
# ---------------------------------------------------------------------------
def _make_tile_cls():
    from concourse import tile as _tile
    from concourse import mybir as _mybir
    from concourse.vector_clock import ScopedClock

    class TileContextSplit(_tile.TileContext):
        def _lower_ordered_insts(self, ordered):
            for bb_name in list(ordered.keys()):
                insts = ordered[bb_name]
                out = []
                n_new = 0
                for inst in insts:
                    si = getattr(inst, 'sync_info', None)
                    waits = list(si.on_wait) if (si is not None and si.on_wait) else []
                    if len(waits) > 1:
                        for w in waits[:-1]:
                            nop = _mybir.InstNoOp(name=f"{inst.name}-w{n_new}",
                                                  ins=[], outs=[])
                            nop.engine = inst.engine
                            nop.sync_info = _mybir.SyncInfo(on_wait=[w], on_update=[])
                            out.append(nop)
                            n_new += 1
                        si.on_wait = waits[-1:]
                    out.append(inst)
                ordered[bb_name] = out
            return super()._lower_ordered_insts(ordered)

        def _drain_and_barrier(self, tick_clock, wait_clock):
            nc = self.nc
            drain_inst = nc.sync.drain()
            wait_clock.add_sem_waits(
                drain_inst.ins, ScopedClock({None: tick_clock.global_clock}))
            waits = list(drain_inst.ins.sync_info.on_wait or [])
            if len(waits) > 1:
                drain_inst.ins.sync_info.on_wait = waits[:1]
                rest = waits[1:]
                while rest:
                    extra = nc.sync.drain()
                    extra.ins.sync_info = _mybir.SyncInfo(on_wait=rest[:1],
                                                          on_update=[])
                    rest = rest[1:]
            nc.all_engine_barrier()
            assert self.sems is not None
            popped = nc._tile_sem_poison_stack.pop()
            assert popped is self._sem_poison
            nc.clear_and_free_semaphores(list(self.sems.allocated().values()))
            nc.all_engine_barrier()

    return TileContextSplit


# ---------------------------------------------------------------------------
# Program emission
# ---------------------------------------------------------------------------
def _emit(use_mask, ln1_aff, ln2_aff, repeat=1):
    import concourse.bass as bass
    import concourse.mybir as mybir
    from contextlib import ExitStack

    f32 = mybir.dt.float32
    f32r = mybir.dt.float32r
    bf16 = mybir.dt.bfloat16
    fp8 = mybir.dt.float8e4
    i32 = mybir.dt.int32
    AF = mybir.ActivationFunctionType
    AL = mybir.AluOpType
    DR = mybir.MatmulPerfMode.DoubleRow
    X = mybir.AxisListType.X
    TileContextSplit = _make_tile_cls()

    nc = bass.Bass("TRN2", target_bir_lowering=False, num_devices=B)

    xT_d = nc.declare_dram_parameter("xT", [128, DC * T], f32r, isOutput=False)
    xN_d = nc.declare_dram_parameter("xN", [128, NCH * 1024], f32, isOutput=False)
    Wqkv_d = nc.declare_dram_parameter("Wqkv_g", [RULES, 3 * D * HD], bf16,
                                       isOutput=False)
    bcs_d = nc.declare_dram_parameter("bcs_g", [RULES, 576], f32r, isOutput=False)
    mlpw_d = nc.declare_dram_parameter("mlpw", [128, 40960], fp8, isOutput=False)
    cb_d = nc.declare_dram_parameter("cblob", [128, CBW], f32r, isOutput=False)
    wo_d = nc.declare_dram_parameter("Wo_aug", [HD + 1, D], bf16, isOutput=False)
    if use_mask:
        maskT_d = nc.declare_dram_parameter("maskT", [T, T], f32, isOutput=False)
    out_d = nc.declare_dram_parameter("out", [T, D], f32, isOutput=True)

    with ExitStack() as ctx:
        tc = ctx.enter_context(TileContextSplit(nc, pool_alloc_mode="queue"))
        P = ctx.enter_context(tc.tile_pool(name="main", bufs=1))

        def tile(shape, dt, tag):
            return P.tile(shape, dt, tag=tag, name=f"{tag}_u{nc.next_id()}")

        # ================= phase 0: DMAs + consts =================
        xTc = [tile([128, T], f32r, f"XT{c}") for c in range(DC)]
        for c in range(DC):
            nc.sync.dma_start(xTc[c][:], xT_d[:, c * T:(c + 1) * T])
        CB = tile([128, CBW], f32r, "CB")
        nc.sync.dma_start(CB[:], cb_d[:])
        IDF = CB[0:128, CB_IDENT:CB_IDENT + 128].bitcast(f32)
        MW = tile([128, 40960], fp8, "MW")
        WO = tile([HD + 1, D], bf16, "WO")
        XN = tile([128, NCH * 1024], f32, "XN")
        if use_mask:
            maskT = [tile([128, T], f32, f"MK{m}") for m in range(NCH)]
            for m, (t0, t1) in enumerate(CH):
                nc.sync.dma_start(maskT[m][0:t1 - t0, :], maskT_d[t0:t1, :])


        # const views from the blob (f32r) + small memsets
        OC = CB[0:128, CB_OC:CB_OC + 1]
        OR128 = CB[0:1, CB_OR128:CB_OR128 + 128]
        ORT = CB[0:1, CB_ORT:CB_ORT + T]
        UR = CB[0:1, CB_UR:CB_UR + KSLOT * 65]
        ones4 = CB[0:KSLOT, CB_OC:CB_OC + 1]
        OB = tile([128, 1], bf16, "OB")       # ones column bf16
        nc.gpsimd.memset(OB[:], 1.0)
        O8 = tile([128, 1], fp8, "O8")        # ones column fp8
        nc.gpsimd.memset(O8[:], 1.0)
        eps1 = tile([1, 1], f32, "EPS")
        nc.gpsimd.memset(eps1[:], 1e-5)

        # MLP weight views
        W1v = MW[:, 0:4096].rearrange("p (s i m) -> p s i m", s=4, i=2, m=512)
        W4v = MW[:, 4096:8192].rearrange("p (s i m) -> p s i m", s=2, i=2, m=1024)
        W2v = MW[:, 8192:24576].rearrange("p (c s i m) -> p c s i m",
                                          c=C2N, s=2, i=2, m=128)
        W3v = MW[:, 24576:40960].rearrange("p (j r i m) -> p j r i m",
                                           j=J1N, r=16, i=2, m=128)

        # ================= phase 1: LN1 stats + xe =================
        sqt = [tile([128, T], bf16, f"SQ{c % 3}") for c in range(DC)]
        MUR = tile([1, T], f32, "MUR")
        INVR = tile([1, T], f32r, "INVR")
        SIGR = tile([1, T], f32, "T3")
        VARR = tile([1, T], f32, "T2")
        MUSQ = tile([1, T], f32, "T1")
        MIR = tile([1, T], f32, "T1")
        NMS = tile([1, T], f32r, "NMS")
        XE = tile([SD, T], f32, "XE")
        XET = tile([128, NCH * SD], f32, "XET")
        IVT = tile([128, NCH], f32, "IVT")
        W3C = tile([128, 18], f32, "W3C")
        NEGC = tile([1, E], f32, "NEGC")

        with tc.tile_pool(name="ps_ln1", bufs=1, space="PSUM") as ps_ln1:
            ps_st = ps_ln1
            psq = [ps_ln1.tile([1, h1 - h0], f32, tag=f"pq{h}",
                               name=f"pq{h}_u{nc.next_id()}")
                   for h, (h0, h1) in enumerate(HALVES)]
            psxe = [ps_ln1.tile([SD + 1, h1 - h0], f32, tag=f"pe{h}",
                                name=f"pe{h}_u{nc.next_id()}")
                    for h, (h0, h1) in enumerate(HALVES)]
            for c in range(DC):
                ow_c = CB[0:128, CB_OW + 33 * c:CB_OW + 33 * (c + 1)]
                for h, (h0, h1) in enumerate(HALVES):
                    nc.tensor.matmul(psxe[h][:], ow_c,
                                     xTc[c][:, h0:h1],
                                     start=(c == 0), stop=(c == DC - 1))
                if c % 4 == 0:
                    nc.gpsimd.tensor_mul(sqt[c][:], xTc[c][:].bitcast(f32),
                                         xTc[c][:].bitcast(f32))
                elif c % 4 in (1, 2):
                    nc.vector.tensor_mul(sqt[c][:], xTc[c][:].bitcast(f32),
                                         xTc[c][:].bitcast(f32))
                else:
                    nc.scalar.activation(sqt[c][:], xTc[c][:].bitcast(f32),
                                         AF.Square)
            for c in range(DC):
                for h, (h0, h1) in enumerate(HALVES):
                    nc.tensor.matmul(psq[h][:], OB[:], sqt[c][:, h0:h1],
                                     start=(c == 0), stop=(c == DC - 1))
            # rows: mu, var, sig, inv, mu*inv
            for h, (h0, h1) in enumerate(HALVES):
                nc.vector.tensor_scalar_mul(MUR[0:1, h0:h1],
                                            psxe[h][SD:SD + 1, :], 1.0 / D)
                nc.vector.tensor_mul(MUSQ[0:1, h0:h1], MUR[0:1, h0:h1],
                                     MUR[0:1, h0:h1])
                nc.vector.scalar_tensor_tensor(VARR[0:1, h0:h1], psq[h][:],
                                               1.0 / D, MUSQ[0:1, h0:h1],
                                               AL.mult, AL.subtract)
                nc.scalar.activation(SIGR[0:1, h0:h1], VARR[0:1, h0:h1],
                                     AF.Sqrt, bias=eps1[:])
                with nc.allow_low_precision("f32r rounding is fine here"):
                    nc.vector.reciprocal(INVR[0:1, h0:h1], SIGR[0:1, h0:h1])
                nc.vector.tensor_mul(MIR[0:1, h0:h1], MUR[0:1, h0:h1],
                                     INVR[0:1, h0:h1].bitcast(f32))
            # xe -> SBUF, then per-chunk transposes
            for h, (h0, h1) in enumerate(HALVES):
                nc.vector.tensor_copy(XE[:, h0:h1], psxe[h][0:SD, :])
            # inv transposed to columns per t-chunk + seg weights
            for m, (t0, t1) in enumerate(CH):
                ts_ = t1 - t0
                pst = ps_st.tile([128, SD], f32, tag="pst",
                                 name=f"pst_u{nc.next_id()}")
                nc.tensor.transpose(pst[0:ts_, 0:SD], XE[:, t0:t1],
                                    IDF[0:SD, 0:SD])
                nc.vector.tensor_copy(XET[0:ts_, m * SD:(m + 1) * SD],
                                      pst[0:ts_, 0:SD])
                psi = ps_st.tile([128, 1], f32, tag="psi",
                                 name=f"psi_u{nc.next_id()}")
                nc.tensor.transpose(psi[0:ts_, 0:1],
                                    INVR[0:1, t0:t1].bitcast(f32),
                                    IDF[0:1, 0:1])
                nc.vector.tensor_copy(IVT[0:ts_, m:m + 1], psi[0:ts_, 0:1])
                nc.vector.tensor_scalar(
                    W3C[:, 3 * m:3 * (m + 1)],
                    CB[0:128, CB_SEGW + 3 * m:CB_SEGW + 3 * (m + 1)].bitcast(f32),
                    IVT[:, m:m + 1], None, AL.mult)
            # neg corr row [1, E]
            for e, (s0, s1) in enumerate(SEGS):
                nc.vector.reduce_sum(NEGC[0:1, e:e + 1], MIR[0:1, s0:s1], axis=X)
                nc.vector.tensor_scalar_mul(NEGC[0:1, e:e + 1],
                                            NEGC[0:1, e:e + 1],
                                            -1.0 / SEG_LENS[e])

        # ============ phase 2: entities + routing (fp32) ============
        entT = tile([SD, E], f32, "ENTT")
        GR = tile([1, T], f32r, "GR")
        SR = tile([1, T], f32r, "SR")
        SBC = tile([128, T], f32, "SBC")
        ridx_i = tile([1, KSLOT], i32, "RIDXI")
        with tc.tile_pool(name="ps_rt", bufs=2, space="PSUM") as ps_rt:
            def rpt(shape, tag="r"):
                return ps_rt.tile(shape, f32, tag=tag, name=f"rt_u{nc.next_id()}")

            ps_ent = rpt([SD, E])
            for m, (t0, t1) in enumerate(CH):
                ts_ = t1 - t0
                nc.tensor.matmul(ps_ent[:], XET[0:ts_, m * SD:(m + 1) * SD],
                                 W3C[0:ts_, 3 * m:3 * (m + 1)],
                                 start=(m == 0), stop=False)
            nc.tensor.matmul(ps_ent[:], CB[0:1, CB_CSE:CB_CSE + SD].bitcast(f32),
                             NEGC[0:1, :], start=False, stop=True)
            nc.vector.tensor_scalar(entT[:], ps_ent[:],
                                    CB[0:SD, CB_BENT:CB_BENT + 1].bitcast(f32),
                                    None, AL.add)

            cb_qer = CB[0:HD, CB_WQER:CB_WQER + SD]
            cb_ker = CB[0:SD, CB_WKER:CB_WKER + SD]
            cb_qes = CB[0:HD, CB_WQES:CB_WQES + 64]
            cb_kes = CB[0:SD, CB_WKES:CB_WKES + 16]
            cb_reT = CB[0:HD, CB_RET:CB_RET + 64]
            cb_ref = CB[0:64, CB_REF:CB_REF + HD]
            cb_E4 = CB[0:KSLOT, CB_E4:CB_E4 + 64]
            cb_E4T = CB[0:64, CB_E4T:CB_E4T + KSLOT]
            cb_iota = CB[0:64, CB_IOTA:CB_IOTA + 1]

            ps_qer = rpt([SD, 64])
            nc.tensor.matmul(ps_qer[:], cb_qer, cb_reT, start=True, stop=True)
            q_erT = tile([SD, 64], f32, "q_erT")
            nc.vector.tensor_copy(q_erT[:], ps_qer[:])

            ps_ker = rpt([SD, E])
            nc.tensor.matmul(ps_ker[:], cb_ker.bitcast(f32), entT[:],
                             start=True, stop=True)
            k_erT = tile([SD, E], f32, "k_erT")
            nc.vector.tensor_copy(k_erT[:], ps_ker[:])

            ps_ser = rpt([64, E])
            nc.tensor.matmul(ps_ser[:], q_erT[:], k_erT[:], start=True, stop=True)
            s_er = tile([64, E], f32, "s_er")
            nc.vector.tensor_scalar_mul(s_er[:], ps_ser[:],
                                        float(1.0 / np.sqrt(SD)))

            m1 = tile([64, 1], f32, "m1")
            nc.vector.reduce_max(m1[:], s_er[:], axis=X)
            ps_m1T = rpt([1, 64])
            nc.tensor.transpose(ps_m1T[:], m1[:], IDF[0:64, 0:64])
            m1T = tile([1, 64], f32, "m1T")
            nc.vector.tensor_copy(m1T[:], ps_m1T[:])
            mk = tile([1, KSLOT], f32, "mk")
            for k in range(KSLOT):
                nc.vector.reduce_max(mk[:, k:k + 1], m1T[:, k * 16:(k + 1) * 16],
                                     axis=X)
            ps_mkT = rpt([KSLOT, 1])
            nc.tensor.transpose(ps_mkT[:], mk[:], IDF[0:1, 0:1])
            mkT = tile([KSLOT, 1], f32, "mkT")
            nc.vector.tensor_copy(mkT[:], ps_mkT[:])
            ps_Mcol = rpt([64, 1])
            nc.tensor.matmul(ps_Mcol[:], cb_E4.bitcast(f32), mkT[:],
                             start=True, stop=True)
            Mcol = tile([64, 1], f32, "Mcol")
            nc.vector.tensor_copy(Mcol[:], ps_Mcol[:])

            onehot = tile([64, E], f32, "onehot")
            nc.vector.tensor_scalar(onehot[:], s_er[:], Mcol[:], None, AL.is_equal)
            rm = tile([64, 1], f32, "rm")
            nc.vector.reduce_sum(rm[:], onehot[:], axis=X)
            rm4 = tile([64, KSLOT], f32, "rm4")
            nc.vector.tensor_scalar(rm4[:], cb_E4T.bitcast(f32), rm[:], None, AL.mult)

            ps_sel = rpt([KSLOT, 64])
            nc.tensor.matmul(ps_sel[:], rm4[:], cb_ref.bitcast(f32),
                             start=True, stop=True)
            sel = tile([KSLOT, 64], f32, "sel")
            nc.vector.tensor_copy(sel[:], ps_sel[:])
            ps_selT = rpt([64, KSLOT])
            nc.tensor.transpose(ps_selT[:], sel[:], IDF[0:KSLOT, 0:KSLOT])
            selT = tile([64, KSLOT], f32, "selT")
            nc.vector.tensor_copy(selT[:], ps_selT[:])

            ps_qes = rpt([16, KSLOT])
            for k in range(KSLOT):
                nc.tensor.matmul(ps_qes[:, k:k + 1],
                                 cb_qes[:, k * 16:(k + 1) * 16].bitcast(f32),
                                 selT[:, k:k + 1], start=(k == 0),
                                 stop=(k == KSLOT - 1))
            q_esT = tile([16, KSLOT], f32, "q_esT")
            nc.vector.tensor_copy(q_esT[:], ps_qes[:])
            ps_kes = rpt([16, E])
            nc.tensor.matmul(ps_kes[:], cb_kes.bitcast(f32), entT[:],
                             start=True, stop=True)
            k_esT = tile([16, E], f32, "k_esT")
            nc.vector.tensor_copy(k_esT[:], ps_kes[:])
            ps_sesT = rpt([E, KSLOT])
            nc.tensor.matmul(ps_sesT[:], k_esT[:], q_esT[:], start=True, stop=True)
            s_esT = tile([E, KSLOT], f32, "s_esT")
            nc.vector.tensor_scalar_mul(s_esT[:], ps_sesT[:],
                                        float(1.0 / np.sqrt(SD // 2)))
            ps_ses = rpt([KSLOT, E])
            nc.tensor.transpose(ps_ses[:], s_esT[:], IDF[0:E, 0:E])
            s_es = tile([KSLOT, E], f32, "s_es")
            nc.vector.tensor_copy(s_es[:], ps_ses[:])
            em1 = tile([KSLOT, 1], f32, "em1")
            nc.vector.reduce_max(em1[:], s_es[:], axis=X)
            eoh = tile([KSLOT, E], f32, "eoh")
            nc.vector.tensor_scalar(eoh[:], s_es[:], em1[:], None, AL.is_equal)

            ps_crow = rpt([1, E])
            nc.tensor.matmul(ps_crow[:], ones4.bitcast(f32), eoh[:],
                             start=True, stop=True)
            c_row = tile([1, E], f32, "c_row")
            nc.vector.tensor_copy(c_row[:], ps_crow[:])

            ps_ridx = rpt([KSLOT, 1])
            nc.tensor.matmul(ps_ridx[:], rm4[:], cb_iota.bitcast(f32),
                             start=True, stop=True)
            ridx_col = tile([KSLOT, 1], f32, "ridx_col")
            nc.vector.tensor_copy(ridx_col[:], ps_ridx[:])
            ps_ridxT = rpt([1, KSLOT])
            nc.tensor.transpose(ps_ridxT[:], ridx_col[:], IDF[0:KSLOT, 0:KSLOT])
            ridx_f = tile([1, KSLOT], f32, "ridx_f")
            nc.vector.tensor_copy(ridx_f[:], ps_ridxT[:])
            nc.vector.tensor_copy(ridx_i[:], ridx_f[:])

            # gate-count row, s row = gate * inv, nms row = -mu * s
            for e, (s0, s1) in enumerate(SEGS):
                nc.vector.tensor_scalar(GR[0:1, s0:s1],
                                        ORT[0:1, s0:s1].bitcast(f32),
                                        c_row[0:1, e:e + 1], None, AL.mult)
            nc.vector.tensor_mul(SR[0:1, :], GR[0:1, :].bitcast(f32),
                                 INVR[0:1, :].bitcast(f32))
            nc.vector.scalar_tensor_tensor(NMS[0:1, :], MUR[0:1, :], -1.0,
                                           SR[0:1, :].bitcast(f32),
                                           AL.mult, AL.mult)

        # ============ phase 3: gather + s_bc + xs ============
        wqkv = [tile([128, 1536], bf16, f"G{k}") for k in range(KSLOT)]
        bcs = [tile([1, 576], f32r, f"BCS{k}") for k in range(KSLOT)]
        for k in range(KSLOT):
            reg = nc.gpsimd.alloc_register(f"ridx{k}")
            nc.gpsimd.reg_load(reg, ridx_i[0:1, k:k + 1])
            off = nc.gpsimd.snap(reg, donate=True, min_val=0, max_val=RULES - 1)
            src_w = Wqkv_d[bass.ds(off, 1), :].rearrange(
                "a (p f) -> (a p) f", p=128, f=1536)
            nc.gpsimd.dma_start(wqkv[k][:], src_w)
            nc.gpsimd.dma_start(bcs[k][:], bcs_d[bass.ds(off, 1), :])
        # Defer the big XN/WO/MW loads until the routing-gated gather lands:
        # DMA-engine grants are FIFO by request time, so an early request
        # starves the gather. A tiny gated read of each target makes the DMA
        # wait via WAR.
        gate = tile([1, 2], f32, "GATE")
        wq3v = wqkv[3][0:1, 0:4].bitcast(f32)
        nc.gpsimd.memset(XN[0:1, 0:2], 0.0)
        nc.gpsimd.memset(WO[0:1, 0:4], 0.0)
        nc.gpsimd.memset(MW[0:1, 0:8], 0.0)
        nc.vector.scalar_tensor_tensor(gate[:], XN[0:1, 0:2], 0.0, wq3v,
                                       AL.mult, AL.add)
        nc.sync.dma_start(XN[:], xN_d[:])
        nc.vector.scalar_tensor_tensor(gate[:], WO[0:1, 0:4].bitcast(f32), 0.0,
                                       wq3v, AL.mult, AL.add)
        nc.sync.dma_start(WO[:], wo_d[:])
        nc.vector.scalar_tensor_tensor(gate[:], MW[0:1, 0:8].bitcast(f32), 0.0,
                                       wq3v, AL.mult, AL.add)
        for q in range(4):
            nc.sync.dma_start(MW[:, q * 10240:(q + 1) * 10240],
                              mlpw_d[:, q * 10240:(q + 1) * 10240])

        with tc.tile_pool(name="ps_bc", bufs=1, space="PSUM") as ps_bc:
            for h, (h0, h1) in enumerate(HALVES):
                psb = ps_bc.tile([128, h1 - h0], f32, tag=f"sb{h}",
                                 name=f"sb{h}_u{nc.next_id()}")
                nc.tensor.matmul(psb[:], OR128, SR[0:1, h0:h1],
                                 start=True, stop=True)
                nc.vector.tensor_copy(SBC[:, h0:h1], psb[:])

        xs = [tile([128, T], bf16, f"XS{c}") for c in range(DC)]
        for c in range(DC):
            nc.vector.tensor_mul(xs[c][:], xTc[c][:].bitcast(f32), SBC[:])

        # wv_all [128, c=8, k*65+j] from gathered v-parts; csv row; bvsum col
        WVA = tile([128, DC * KSLOT * 65], bf16, "WVA")
        nc.gpsimd.memset(WVA[:], 0.0)
        for k in range(KSLOT):
            nc.vector.tensor_copy(
                WVA.rearrange("p (c k u) -> p c k u", c=DC, k=KSLOT, u=65)
                [:, :, k, 0:64],
                wqkv[k].rearrange("p (c three j) -> p c three j",
                                  c=DC, three=3, j=HD)[:, :, 2, :])
        CSV = tile([1, KSLOT * 65], f32r, "CSV")
        nc.gpsimd.memset(CSV[:].bitcast(f32), 0.0)
        for k in range(KSLOT):
            nc.vector.tensor_copy(CSV[0:1, k * 65:k * 65 + 64],
                                  bcs[k][0:1, 320:384].bitcast(f32))
        if ln1_aff:
            CBV = tile([1, KSLOT * 65], f32r, "CBV")
            nc.gpsimd.memset(CBV[:].bitcast(f32), 0.0)
            for k in range(KSLOT):
                nc.vector.tensor_copy(CBV[0:1, k * 65:k * 65 + 64],
                                      bcs[k][0:1, 512:576].bitcast(f32))
        bv01 = tile([1, HD], f32, "BV01")
        nc.vector.tensor_add(bv01[:], bcs[0][0:1, 128:192].bitcast(f32),
                             bcs[1][0:1, 128:192].bitcast(f32))
        bv23 = tile([1, HD], f32, "BV23")
        nc.vector.tensor_add(bv23[:], bcs[2][0:1, 128:192].bitcast(f32),
                             bcs[3][0:1, 128:192].bitcast(f32))
        bvr = tile([1, HD], f32, "BVR")
        nc.vector.tensor_add(bvr[:], bv01[:], bv23[:])
        BVS = tile([HD, 1], f32, "BVS")
        with tc.tile_pool(name="ps_bv", bufs=1, space="PSUM") as ps_bv:
            psv = ps_bv.tile([HD, 1], f32, tag="bv", name=f"bv_u{nc.next_id()}")
            nc.tensor.transpose(psv[:], bvr[:], IDF[0:1, 0:1])
            nc.vector.tensor_copy(BVS[:], psv[:])

        # ============ phase 4: QKV + V ============
        qT = [tile([HD, T], bf16, f"QT{k}") for k in range(KSLOT)]
        kT = [tile([HD, T], bf16, f"KT{k}") for k in range(KSLOT)]
        V_aug = [tile([128, KSLOT * 65], bf16, f"VA{m}") for m in range(NCH)]
        with tc.tile_pool(name="ps_qkv", bufs=2, space="PSUM") as ps_qkv, \
             tc.tile_pool(name="ps_v", bufs=2, space="PSUM") as ps_v:
            for k in range(KSLOT):
                for h, (h0, h1) in enumerate(HALVES):
                    ps = ps_qkv.tile([128, h1 - h0], f32, tag="qk",
                                     name=f"psqk_u{nc.next_id()}")
                    for c in range(DC):
                        lhs_qk = wqkv[k].rearrange(
                            "p (c three j) -> p c three j",
                            c=DC, three=3, j=HD)[:, c, 0:2, :]
                        nc.tensor.matmul(ps[:], lhs_qk, xs[c][:, h0:h1],
                                         start=(c == 0), stop=False)
                    nc.tensor.matmul(ps[:], bcs[k][0:1, 0:128],
                                     ORT[0:1, h0:h1],
                                     start=False, stop=False)
                    nc.tensor.matmul(ps[:], bcs[k][0:1, 192:320],
                                     NMS[0:1, h0:h1],
                                     start=False, stop=(not ln1_aff))
                    if ln1_aff:
                        nc.tensor.matmul(ps[:], bcs[k][0:1, 384:512],
                                         GR[0:1, h0:h1],
                                         start=False, stop=True)
                    nc.vector.tensor_copy(qT[k][:, h0:h1], ps[0:HD, :])
                    nc.vector.tensor_copy(kT[k][:, h0:h1], ps[HD:128, :])
            for m, (t0, t1) in enumerate(CH):
                ts_ = t1 - t0
                ps = ps_v.tile([128, KSLOT * 65], f32, tag="v",
                               name=f"psv_u{nc.next_id()}")
                for c in range(DC):
                    nc.tensor.matmul(ps[0:ts_, :], xs[c][:, t0:t1],
                                     WVA.rearrange("p (c u) -> p c u",
                                                   c=DC, u=KSLOT * 65)[:, c, :],
                                     start=(c == 0), stop=False)
                nc.tensor.matmul(ps[0:ts_, :], NMS[0:1, t0:t1],
                                 CSV[:], start=False, stop=False)
                nc.tensor.matmul(ps[0:ts_, :], ORT[0:1, t0:t1],
                                 UR, start=False,
                                 stop=(not ln1_aff))
                if ln1_aff:
                    nc.tensor.matmul(ps[0:ts_, :], GR[0:1, t0:t1],
                                     CBV[:], start=False, stop=True)
                nc.scalar.copy(V_aug[m][0:ts_, :], ps[0:ts_, :])

        # ============ phase 5: attention ============
        exp_tags = ["G0", "G1", "G2", "G3", "WVA", "XE"]
        expT = [tile([128, T], bf16, exp_tags[m]) for m in range(NCH)]
        aoT = tile([HD + 1, T], bf16, "AOT")
        nc.gpsimd.memset(aoT[HD:HD + 1, :], 4.0)
        with tc.tile_pool(name="ps_sc", bufs=2, space="PSUM") as ps_sc, \
             tc.tile_pool(name="ps_av", bufs=2, space="PSUM") as ps_av, \
             tc.tile_pool(name="ps_rb", bufs=2, space="PSUM") as ps_rb:
            for h, (h0, h1) in enumerate(HALVES):
                n = h1 - h0
                tmps = []
                for k in range(KSLOT):
                    for m, (t0, t1) in enumerate(CH):
                        ts_ = t1 - t0
                        ps = ps_sc.tile([128, n], f32, tag="sc",
                                        name=f"sc_u{nc.next_id()}")
                        if use_mask:
                            nc.vector.tensor_scalar(ps[0:ts_, :],
                                                    maskT[m][0:ts_, h0:h1],
                                                    8.0, None, AL.mult)
                            first = False
                        else:
                            first = True
                        nc.tensor.matmul(ps[0:ts_, :], kT[k][:, t0:t1],
                                         qT[k][:, h0:h1], start=first, stop=True)
                        nc.scalar.activation(expT[m][0:ts_, h0:h1], ps[0:ts_, :],
                                             AF.Exp, scale=0.125)
                    av = ps_av.tile([HD + 1, n], f32, tag="av",
                                    name=f"av_u{nc.next_id()}")
                    for m, (t0, t1) in enumerate(CH):
                        ts_ = t1 - t0
                        nc.tensor.matmul(av[:], V_aug[m][0:ts_, k * 65:(k + 1) * 65],
                                         expT[m][0:ts_, h0:h1],
                                         start=(m == 0), stop=(m == NCH - 1))
                    rec = tile([1, n], f32r, f"REC{k % 2}")
                    with nc.allow_low_precision("softmax renorm"):
                        nc.vector.reciprocal(rec[:], av[HD:HD + 1, :])
                    rbp = ps_rb.tile([HD, n], f32, tag="rb",
                                     name=f"rb_u{nc.next_id()}")
                    nc.tensor.matmul(rbp[:], OR128[0:1, 0:HD],
                                     rec[:], start=True, stop=True)
                    rb = tile([HD, n], f32, f"RB{k % 2}")
                    nc.scalar.copy(rb[:], rbp[:])
                    tmp = tile([HD, n], f32, f"TMP{k}")
                    nc.vector.tensor_mul(tmp[:], av[0:HD, :], rb[:])
                    tmps.append(tmp)
                nc.gpsimd.tensor_add(tmps[0][:], tmps[0][:], tmps[1][:])
                nc.gpsimd.tensor_add(tmps[2][:], tmps[2][:], tmps[3][:])
                nc.vector.tensor_add(tmps[0][:], tmps[0][:], tmps[2][:])
                nc.vector.tensor_scalar(aoT[0:HD, h0:h1], tmps[0][:], BVS[:],
                                        None, AL.add)

        # ============ phase 6: Wo + residuals (h2q fp8, h2N f32) ============
        h2q = [tile([128, 2, T], fp8, f"KT{p}") for p in range(4)]
        with tc.tile_pool(name="ps_wo", bufs=2, space="PSUM") as ps_wo:
            for c in range(DC):
                for h, (h0, h1) in enumerate(HALVES):
                    ps = ps_wo.tile([128, h1 - h0], f32, tag="woT",
                                    name=f"woT_u{nc.next_id()}")
                    nc.tensor.matmul(ps[:], WO[:, c * 128:(c + 1) * 128],
                                     aoT[:, h0:h1], start=True, stop=True)
                    nc.vector.tensor_add(h2q[c // 2][:, c % 2, h0:h1],
                                         xTc[c][:, h0:h1].bitcast(f32), ps[:])
        # ============ phase 7: LN2 stats on h2q ============
        MU2 = tile([1, T], f32, "MUR")
        NM2 = tile([1, T], f32r, "NMS")
        SG2 = tile([1, T], f32, "T3")
        VA2 = tile([1, T], f32, "T2")
        MQ2 = tile([1, T], f32, "T1")
        IV2 = tile([1, T], f32r, "INVR")
        sq2 = [tile([128, T], bf16, f"SQ{i % 3}") for i in range(DC)]
        with tc.tile_pool(name="ps_ln2", bufs=1, space="PSUM") as ps_ln2:
            px2 = [ps_ln2.tile([1, h1 - h0], f32, tag=f"p2x{h}",
                               name=f"p2x{h}_u{nc.next_id()}")
                   for h, (h0, h1) in enumerate(HALVES)]
            pq2 = [ps_ln2.tile([1, h1 - h0], f32, tag=f"p2q{h}",
                               name=f"p2q{h}_u{nc.next_id()}")
                   for h, (h0, h1) in enumerate(HALVES)]
            for c in range(DC):
                src = h2q[c // 2][:, c % 2, :]
                for h, (h0, h1) in enumerate(HALVES):
                    nc.tensor.matmul(px2[h][:], O8[:], src[:, h0:h1],
                                     start=(c == 0), stop=(c == DC - 1))
                nc.scalar.activation(sq2[c][:], src, AF.Square)
            for c in range(DC):
                for h, (h0, h1) in enumerate(HALVES):
                    nc.tensor.matmul(pq2[h][:], OB[:], sq2[c][:, h0:h1],
                                     start=(c == 0), stop=(c == DC - 1))
            for h, (h0, h1) in enumerate(HALVES):
                nc.vector.tensor_scalar_mul(MU2[0:1, h0:h1], px2[h][:], 1.0 / D)
                nc.vector.tensor_mul(MQ2[0:1, h0:h1], MU2[0:1, h0:h1],
                                     MU2[0:1, h0:h1])
                nc.vector.scalar_tensor_tensor(VA2[0:1, h0:h1], pq2[h][:],
                                               1.0 / D, MQ2[0:1, h0:h1],
                                               AL.mult, AL.subtract)
                nc.scalar.activation(SG2[0:1, h0:h1], VA2[0:1, h0:h1],
                                     AF.Sqrt, bias=eps1[:])
                with nc.allow_low_precision("f32r rounding is fine here"):
                    nc.vector.reciprocal(IV2[0:1, h0:h1], SG2[0:1, h0:h1])
                nc.vector.tensor_scalar_mul(NM2[0:1, h0:h1], MU2[0:1, h0:h1],
                                            -1.0)
        # ============ phase 8: MLP (fp8 DoubleRow) ============
        y1q = [tile([128, 2, T], fp8, f"SQ{p}") for p in range(2)]
        y2_tags = ([f"XS{c}" for c in range(DC)] + [f"G{k}" for k in range(KSLOT)]
                   + ["WVA", "XE", "SBC", "Y2F"])
        if use_mask:
            y2_tags = ([f"MK{m}" for m in range(NCH)]
                       + [f"XS{c}" for c in range(DC)] + ["G0", "G1"])
        y2q = [tile([128, 2, T], fp8, y2_tags[p]) for p in range(16)]
        y3q = [tile([128, 2, T], fp8, f"QT{p}") for p in range(2)]

        with tc.tile_pool(name="ps_i2", bufs=1, space="PSUM") as ps_i2, \
             tc.tile_pool(name="ps_y1", bufs=2, space="PSUM") as ps_y1:
            i2bc = []
            for h, (h0, h1) in enumerate(HALVES):
                pi = ps_i2.tile([128, h1 - h0], f32, tag=f"i2{h}",
                                name=f"i2{h}_u{nc.next_id()}")
                nc.tensor.matmul(pi[:], OR128, IV2[0:1, h0:h1],
                                 start=True, stop=True)
                i2bc.append(pi)
            for j in range(J1N):
                for h, (h0, h1) in enumerate(HALVES):
                    ps = ps_y1.tile([128, h1 - h0], f32, tag="y1",
                                    name=f"y1_u{nc.next_id()}")
                    for s in range(4):
                        nc.tensor.matmul(ps[:], W1v[:, s, :, j * 128:(j + 1) * 128],
                                         h2q[s][:, :, h0:h1], perf_mode=DR,
                                         start=(s == 0), stop=False)
                    nc.tensor.matmul(
                        ps[:], CB[0:1, CB_CS1 + j * 128:CB_CS1 + (j + 1) * 128],
                        NM2[0:1, h0:h1], start=False, stop=(not ln2_aff))
                    if ln2_aff:
                        nc.tensor.matmul(
                            ps[:], CB[0:1, CB_R1 + j * 128:CB_R1 + (j + 1) * 128],
                            SG2[0:1, h0:h1].bitcast(f32r),
                            start=False, stop=True)
                    nc.scalar.activation(y1q[j // 2][:, j % 2, h0:h1], ps[:],
                                         AF.Relu)
            for p in range(2):
                for i in range(2):
                    for h, (h0, h1) in enumerate(HALVES):
                        nc.vector.tensor_mul(y1q[p][:, i, h0:h1],
                                             y1q[p][:, i, h0:h1], i2bc[h][:])

        with tc.tile_pool(name="ps_y2", bufs=2, space="PSUM") as ps_y2, \
             tc.tile_pool(name="ps_y3", bufs=2, space="PSUM") as ps_y3, \
             tc.tile_pool(name="ps_y4", bufs=2, space="PSUM") as ps_y4:
            for m, (t0, t1) in enumerate(CH):
                ts_ = t1 - t0
                for dh in range(2):
                    d0 = dh * 512
                    ps = ps_y4.tile([128, 512], f32, tag="y4",
                                    name=f"woN_u{nc.next_id()}")
                    nc.tensor.matmul(ps[0:ts_, :], aoT[:, t0:t1],
                                     WO[:, d0:d0 + 512], start=True, stop=True)
                    nc.vector.tensor_add(
                        XN[0:ts_, m * 1024 + d0:m * 1024 + d0 + 512],
                        XN[0:ts_, m * 1024 + d0:m * 1024 + d0 + 512],
                        ps[0:ts_, :])
            for c2 in range(C2N):
                ps = ps_y2.tile([128, 1024], f32, tag="y2",
                                name=f"y2_u{nc.next_id()}")
                for h, (h0, h1) in enumerate(HALVES):
                    pw = ps[:, 512 * h:512 * h + (h1 - h0)]
                    for s in range(2):
                        nc.tensor.matmul(pw, W2v[:, c2, s],
                                         y1q[s][:, :, h0:h1], perf_mode=DR,
                                         start=(s == 0), stop=(s == 1))
                # one wide gelu over both halves (strided read of the padded
                # PSUM tile; halves sit at bank-aligned 512 offsets)
                psw = ps.rearrange("p (two n) -> p two n", two=2, n=512)
                nc.scalar.activation(
                    y2q[c2 // 2][:, c2 % 2, :].rearrange(
                        "p (two n) -> p two n", two=2, n=352),
                    psw[:, :, 0:352], AF.Gelu)
            for p0, p1, mlist in [(0, 384, [0, 1, 2]), (384, 704, [3, 4, 5])]:
                pn = p1 - p0
                for j in range(J1N):
                    ps = ps_y3.tile([128, 384], f32, tag="y3",
                                    name=f"y3_u{nc.next_id()}")
                    for r in range(16):
                        nc.tensor.matmul(ps[:, 0:pn], W3v[:, j, r],
                                         y2q[r][:, :, p0:p1], perf_mode=DR,
                                         start=(r == 0), stop=(r == 15))
                    nc.scalar.activation(y3q[j // 2][:, j % 2, p0:p1],
                                         ps[:, 0:pn], AF.Relu)
                for m in mlist:
                    t0, t1 = CH[m]
                    ts_ = t1 - t0
                    for dh in range(2):
                        d0 = dh * 512
                        ps = ps_y4.tile([128, 512], f32, tag="y4",
                                        name=f"y4_u{nc.next_id()}")
                        for p in range(2):
                            nc.tensor.matmul(ps[0:ts_, :], y3q[p][:, :, t0:t1],
                                             W4v[:, p, :, d0:d0 + 512],
                                             perf_mode=DR,
                                             start=(p == 0), stop=(p == 1))
                        nc.vector.tensor_add(
                            XN[0:ts_, m * 1024 + d0:m * 1024 + d0 + 512],
                            XN[0:ts_, m * 1024 + d0:m * 1024 + d0 + 512],
                            ps[0:ts_, :])
                    nc.sync.dma_start(out_d[t0:t1, :],
                                      XN[0:ts_, m * 1024:(m + 1) * 1024])

    return nc


# ---------------------------------------------------------------------------
# Host-side input prep
# ---------------------------------------------------------------------------
def _host_consts(inputs, ln1_aff, ln2_aff):
    """Batch-independent tensors (weights), computed once."""
    import ml_dtypes
    f = np.float32
    e4 = ml_dtypes.float8_e4m3
    d = {}

    g1 = np.asarray(inputs["ln1_g"], f) if ln1_aff else None
    g2 = np.asarray(inputs["ln2_g"], f) if ln2_aff else None
    b1 = np.asarray(inputs["ln1_b"], f) if ln1_aff else None
    b2 = np.asarray(inputs["ln2_b"], f) if ln2_aff else None

    # gathered per-rule QKV weights (g1 folded in if affine)
    blks = []
    for Wn in ("Wq", "Wk", "Wv"):
        W = np.asarray(inputs[Wn], f)
        if ln1_aff:
            W = W * g1[:, None]
        blks.append(W.reshape(DC, 128, RULES, HD).transpose(2, 0, 1, 3))
    d["Wqkv_g"] = np.ascontiguousarray(
        np.stack(blks, axis=2).transpose(0, 3, 1, 2, 4)
        .reshape(RULES, 3 * D * HD)).astype(ml_dtypes.bfloat16)

    # bcs row per rule: [bq bk bv csq csk csv cbq cbk cbv] (9*64 = 576)
    bias = np.concatenate(
        [np.asarray(inputs[bn], f).reshape(RULES, HD) for bn in ("bq", "bk", "bv")],
        axis=1)                                             # [R, 192]
    # blks[i] is [R, DC, 128, HD]; column sums over d per rule head
    csums = np.concatenate(
        [blk.reshape(RULES, D, HD).sum(1) for blk in blks], axis=1)  # [R, 192]
    if ln1_aff:
        cb = np.concatenate(
            [np.einsum('d,drh->rh', b1,
                       (np.asarray(inputs[Wn], f) * g1[:, None])
                       .reshape(D, RULES, HD))
             for Wn in ("Wq", "Wk", "Wv")], axis=1)
    else:
        cb = np.zeros((RULES, 192), f)
    d["bcs_g"] = np.ascontiguousarray(np.concatenate([bias, csums, cb], axis=1))

    # Wo_aug bf16
    d["Wo_aug"] = np.ascontiguousarray(np.concatenate(
        [np.asarray(inputs["Wo"], f), np.asarray(inputs["bo"], f)[None, :]],
        0)).astype(ml_dtypes.bfloat16)

    # MLP weights fp8, DoubleRow layouts
    W1 = np.asarray(inputs["fc1_w1"], f)
    if ln2_aff:
        W1 = W1 * g2[:, None]
    W1q = W1.astype(e4)
    W2q = np.asarray(inputs["fc1_w2"], f).astype(e4)
    W3q = np.asarray(inputs["fc2_w1"], f).astype(e4)
    W4q = np.asarray(inputs["fc2_w2"], f).astype(e4)
    # W1 [1024, 512] -> [p, s, i, m]
    w1 = W1q.reshape(4, 2, 128, FD1).transpose(2, 0, 1, 3).reshape(128, 4096)
    # W4 [512, 1024] -> [p, s, i, d]
    w4 = W4q.reshape(2, 2, 128, D).transpose(2, 0, 1, 3).reshape(128, 4096)
    # W2 [512, 4096] -> [p, c2, s, i, m]
    w2 = (W2q.reshape(2, 2, 128, C2N, 128).transpose(2, 3, 0, 1, 4)
          .reshape(128, 16384))
    # W3 [4096, 512] -> [p, j, r, i, m]
    w3 = (W3q.reshape(16, 2, 128, J1N, 128).transpose(2, 3, 0, 1, 4)
          .reshape(128, 16384))
    d["mlpw"] = np.ascontiguousarray(np.concatenate([w1, w4, w2, w3], axis=1))

    # consts blob
    cb_arr = np.zeros((128, CBW), f)
    cb_arr[:, CB_IDENT:CB_IDENT + 128] = np.eye(128, dtype=f)
    went = np.asarray(inputs["W_ent"], f)
    if ln1_aff:
        went = went * g1[:, None]
    cb_arr[:, CB_WENT:CB_WENT + DC * SD] = (
        went.reshape(DC, 128, SD).transpose(1, 0, 2).reshape(128, DC * SD))
    went_c = went.reshape(DC, 128, SD)
    for c in range(DC):
        cb_arr[:, CB_OW + 33 * c:CB_OW + 33 * c + SD] = went_c[c]
        cb_arr[:, CB_OW + 33 * c + SD] = 1.0
    segw = np.zeros((128, NCH * E), f)
    for m, (t0, t1) in enumerate(CH):
        for p in range(t1 - t0):
            t = t0 + p
            for e, (s0, s1) in enumerate(SEGS):
                if s0 <= t < s1:
                    segw[p, m * E + e] = 1.0 / SEG_LENS[e]
    cb_arr[:, CB_SEGW:CB_SEGW + NCH * E] = segw
    E4 = np.kron(np.eye(KSLOT, dtype=f), np.ones((1, RULES), f))
    cb_arr[0:KSLOT, CB_E4:CB_E4 + 64] = E4
    cb_arr[0:64, CB_E4T:CB_E4T + KSLOT] = E4.T
    cb_arr[0:64, CB_IOTA] = (np.arange(64) % RULES).astype(f)
    cb_arr[0:HD, CB_WQER:CB_WQER + SD] = np.asarray(inputs["Wq_er"], f)
    cb_arr[0:SD, CB_WKER:CB_WKER + SD] = np.asarray(inputs["Wk_er"], f)
    Wqes = np.asarray(inputs["Wq_es"], f)          # [K, HD, SD//2]
    cb_arr[0:HD, CB_WQES:CB_WQES + 64] = (
        Wqes.transpose(1, 0, 2).reshape(HD, KSLOT * (SD // 2)))
    cb_arr[0:SD, CB_WKES:CB_WKES + 16] = np.asarray(inputs["Wk_es"], f)
    re = np.asarray(inputs["rules_embed"], f)      # [K, R, HD]
    cb_arr[0:HD, CB_RET:CB_RET + 64] = re.transpose(2, 0, 1).reshape(HD, 64)
    cb_arr[0:64, CB_REF:CB_REF + HD] = re.reshape(64, HD)
    bent = np.asarray(inputs["b_ent"], f)
    if ln1_aff:
        bent = bent + b1 @ went
    cb_arr[0:SD, CB_BENT] = bent
    cb_arr[0, CB_CSE:CB_CSE + SD] = went.sum(0)
    cb_arr[0, CB_CS1:CB_CS1 + FD1] = W1q.astype(f).sum(0)
    if ln2_aff:
        cb_arr[0, CB_R1:CB_R1 + FD1] = b2 @ W1
    cb_arr[:, CB_OC] = 1.0
    cb_arr[0, CB_OR128:CB_OR128 + 128] = 1.0
    cb_arr[0, CB_ORT:CB_ORT + T] = 1.0
    ur = np.zeros(KSLOT * 65, f)
    ur[64::65] = 1.0
    cb_arr[0, CB_UR:CB_UR + KSLOT * 65] = ur
    d["cblob"] = np.ascontiguousarray(cb_arr)
    return d


def _prep_core_inputs(inputs, b, use_mask, ln1_aff, ln2_aff, consts=None):
    f = np.float32
    if consts is None:
        consts = _host_consts(inputs, ln1_aff, ln2_aff)
    d = dict(consts)
    hs = np.asarray(inputs["hidden_states"], f)
    x = hs[b]                                      # [T, D]
    xT = np.ascontiguousarray(x.T)                 # [D, T]
    d["xT"] = np.ascontiguousarray(
        xT.reshape(DC, 128, T).transpose(1, 0, 2).reshape(128, DC * T))
    xn = np.zeros((128, NCH * 1024), f)
    for m, (t0, t1) in enumerate(CH):
        xn[0:t1 - t0, m * 1024:(m + 1) * 1024] = x[t0:t1]
    d["xN"] = xn
    if use_mask:
        d["maskT"] = np.ascontiguousarray(
            np.asarray(inputs["attention_mask"], f)[b].T)
    return d


# ---------------------------------------------------------------------------
# Runner (jax/axon shard_map over 8 cores)
# ---------------------------------------------------------------------------
def _build_runner(use_mask, ln1_aff, ln2_aff, repeat=1):
    key = (use_mask, ln1_aff, ln2_aff, repeat)
    if key in _RUNNERS:
        return _RUNNERS[key]
    import jax
    from jax.sharding import Mesh, PartitionSpec
    from jax.experimental.shard_map import shard_map
    from concourse import mybir
    from concourse.bass2jax import (_bass_exec_p, install_neuronx_cc_hook,
                                    partition_id_tensor)

    nc = _emit(use_mask, ln1_aff, ln2_aff, repeat)
    install_neuronx_cc_hook()
    partition_name = nc.partition_id_tensor.name if nc.partition_id_tensor else None
    in_names, out_names, out_avals, zero_shapes = [], [], [], []
    for alloc in nc.m.functions[0].allocations:
        if not isinstance(alloc, mybir.MemoryLocationSet):
            continue
        name = alloc.memorylocations[0].name
        if alloc.kind == "ExternalInput":
            if name != partition_name:
                in_names.append(name)
        elif alloc.kind == "ExternalOutput":
            out_names.append(name)
            shape = tuple(alloc.tensor_shape)
            dtype = mybir.dt.np(alloc.dtype)
            out_avals.append(jax.core.ShapedArray(shape, dtype))
            zero_shapes.append((shape, dtype))
    n_params = len(in_names)
    n_outs = len(out_avals)
    all_in_names = list(in_names) + list(out_names)
    if partition_name is not None:
        all_in_names.append(partition_name)

    def _body(*args):
        operands = list(args)
        if partition_name is not None:
            operands.append(partition_id_tensor())
        outs = _bass_exec_p.bind(
            *operands, out_avals=tuple(out_avals), in_names=tuple(all_in_names),
            out_names=tuple(out_names), lowering_input_output_aliases=(),
            sim_require_finite=False, sim_require_nnan=False, nc=nc)
        return tuple(outs)

    devices = jax.devices()[:B]
    mesh = Mesh(np.asarray(devices), ("core",))
    in_specs = (PartitionSpec("core"),) * (n_params + n_outs)
    out_specs = (PartitionSpec("core"),) * n_outs
    sharded = jax.jit(
        shard_map(_body, mesh=mesh, in_specs=in_specs, out_specs=out_specs,
                  check_rep=False),
        keep_unused=True)

    def run(per_core_maps):
        concat_in = [
            np.concatenate([np.asarray(per_core_maps[c][nm]) for c in range(B)], 0)
            for nm in in_names]
        concat_zeros = [np.zeros((B * s[0], *s[1:]), dt) for s, dt in zero_shapes]
        out_arrs = jax.block_until_ready(sharded(*concat_in, *concat_zeros))
        return [
            {nm: np.asarray(out_arrs[i]).reshape(B, *out_avals[i].shape)[c]
             for i, nm in enumerate(out_names)}
            for c in range(B)]

    _RUNNERS[key] = (run, sharded, in_names, zero_shapes, out_names, out_avals)
    return _RUNNERS[key]


def kernel(**inputs):
    use_mask = bool(np.any(np.asarray(inputs["attention_mask"])))
    ln1_aff = not (np.all(np.asarray(inputs["ln1_g"]) == 1.0)
                   and np.all(np.asarray(inputs["ln1_b"]) == 0.0))
    ln2_aff = not (np.all(np.asarray(inputs["ln2_g"]) == 1.0)
                   and np.all(np.asarray(inputs["ln2_b"]) == 0.0))
    run = _build_runner(use_mask, ln1_aff, ln2_aff)[0]
    consts = _host_consts(inputs, ln1_aff, ln2_aff)
    maps = [_prep_core_inputs(inputs, b, use_mask, ln1_aff, ln2_aff, consts)
            for b in range(B)]
    res = run(maps)
    out = np.stack([res[b]["out"] for b in range(B)]).astype(np.float32)
    return out

